# revision 31
# baseline (speedup 1.0000x reference)
"""Trainium2 Bass kernel for nn_ARPrior (stacked causal-prior MLPs).

Network (per sample, latent D=32, L=31 stacked layers):
    zin = z[:, :31]
    h1[l] = relu(W1m[l] @ zin + b1[l])   # [128], W1m causally masked
    h2[l] = relu(W2[l]  @ h1[l] + b2[l]) # [64]
    out[l] = Wout[l] @ h2[l] + bout[l]   # [2]  (mu, logvar)
    mus = [bout0[0], out[:,0]]; lvs = [bout0[1], out[:,1]]

Mapping (pure data parallel, batch 65536 sharded 8 ways -> 8192/core):
  - L1: K=31 -> 4 layers packed in the 128x128 PE array via row tiling
    (tile_position=(32i,0)), each writing its own PSUM bank.
  - L2: K=128, M=64 -> 2 layers packed via column tiling
    (tile_position=(0,0)/(0,64)) into one PSUM bank.
  - L3: M=2 per layer -> all 31 layers' output weights embedded in a
    block-diagonal [128,64] stationary per layer-pair, accumulated over
    16 matmuls into a single [64,512] PSUM tile per batch tile. Output
    columns are pre-arranged [mu(32) | logvar(32)]; bias adds bout/bout0.
  - The PSUM drain (bias+relu+fp16 cast, one pass over every h1/h2
    element) is the throughput wall (~283us/core combined across both
    drain-capable engines); ops are split greedily between ScalarE
    (activation Relu w/ per-partition bias AP) and VectorE (tensor_scalar
    add+max). Fine-grained single-bank PSUM rotation (4 L1 + 2 L2 + 2 L3
    slots) keeps the TensorE free of long stalls so the HAM clock gate
    stays at 2.4 GHz; coarser multi-bank drain variants measured slower
    despite fewer drain ops. Default variant: 5 L1 + 2 L2 + 1 shared L3
    psum slots, inline L3 accumulation. Final tuning: ACT-table prewarm,
    boot DMAs spread over the three DGE paths (w1 split so a 64KB first
    chunk ungates the first matmul; the zero half of w1s never
    transferred), final-tile output stores on two rings, and the greedy
    DVE/ACT split recalibrated to measured busy-per-op (682/604 ns, flag
    c2), and w2 split the same way as w1 (pairs 0-3 first) to kill a
    ~1.4us both-engine stall at t~14.5us waiting on the 512KB w2
    transfer: 297.2us/core (297,155/297,187 ns across two runs), engines
    balanced within 1.4us. Forcing drain-engine placement (pe2 flag, off)
    measured 308us - the greedy balancer's freedom beats deterministic
    pairing.
  - Profiled balance (per core, ~306us wall before the trims): TensorE
    union-busy 257us (mms ~203ns effective; tile_position pairs co-issue
    ~20-30% of the time), DVE 242us, ACT 249us. All three engines ~80-92%
    busy - a sharp local optimum. Two structural variants measured WORSE: d2 (2-bank
    [128,1024] L1 psums, bias folded via ones row -> half the L1 drain
    ops) hit 513us because halving the PSUM slot count starves the PE
    (mm latency ~633ns = slot head-wait/cold); l3cs (L3 split into two
    M=32 col-tiled chains sharing one rhs) hit 424us from doubled L3
    mm/sem traffic inflating every op ~20%. On TRN2 the drain floor is
    hard: only DVE (0.96GHz) + ACT (1.2GHz) can read PSUM at 1
    elem/lane/cyc (fp32 PSUM is 1x on DVE; 16-bit PSUM matmul output that
    would enable 2x_1P reads is TRN3-only; DMA/GPSIMD have no PSUM port).
  - All compute in fp16 (1 PE cycle/row vs 2 for fp32), fp32 PSUM/bias,
    rel err ~5e-4.
  - skw (current default, 262-264.5us/core, 11% over the 297us non-skw
    build): software-pipelined emission. L2 work of group g-1 and L3 work
    of group g-2 are emitted after group g's L1 section (queues cross tile
    boundaries; osb/output DMAs emit when the q==15 L3 of an odd tile is
    processed; post-loop flush drains the queues). This removes the
    in-order head-of-line blocking that previously idled all three
    engines ~13%: with skw, every instruction's dependencies were
    produced a full group (~2us) earlier, so the drain engines run
    saturated (mid-run idle 0.6-1.3us vs ~35us before; DVE 247us busy,
    ACT 249.7us busy, PE 215.8us on a 264.5us wall). Remaining idle is
    boot (~5-8us DMA/DGE init + HAM grant at ~17-22us) and tail (~8us
    final flush + output DMA receipts); both measured resistant: inlining
    the last tile cost +2us, tail-splitting osb is drain-work-negative.
  - Measured dead ends (2026-08-09 session), do not revisit without new
    evidence: (1) m2/m3 layer-paired [128,1024] L1 psums + relu-only
    1024-col drains (b1 folded via the zT4 ones row, 2-step skewed flat
    pipeline, parity-pinned drain engines): the 1024-col drain saves only
    5-8% engine time (DVE 1214ns vs 2x682, ACT 1106 vs 2x604) and the
    reduced PE load (<80% duty) makes the HAM K=8/8 grant a boot-time
    race that some cores lose entirely (whole run at 1.2GHz, 403us) plus
    mid-run K=4/8 relapses; best stable m3 = 299-311us. The HAM grant
    does NOT follow from continuous PE busy at K=4/8 (18 back-to-back
    zero-data warmup matmuls never tripped it; saturating real-data boot
    moved it only 60->26us); the fine-grained non-skw baseline earns it
    at ~17us reliably, and skw keeps PE >80% so it sticks. (2) fp8
    DoubleRow (2x PE): e4m3 quantization is ~2.2% rms per tensor and
    error propagates through the linear layers undiminished -> blows the
    2e-2 gate (fp16 build is at 5e-4). (3) c3 recalibration of the
    greedy split to raw measured per-op times (742/735) regressed to
    276us - the raw averages embed queue effects; c2's 682/604 ratio
    (~1.13 = true DVE/ACT throughput ratio) assigns correctly. (4)
    period-8 drain-engine patterns (DVE 3xh1+5xh2 / ACT 5xh1+3xh2):
    per-step makespan lumpiness beats the better period-average - 325us.
  - Roofline context: 48.8M hidden elems/core must cross PSUM->SBUF on
    exactly two engines (DVE 0.96GHz + ACT 1.2GHz, 1 fp32 elem/lane/cyc;
    DMA/GPSIMD have no PSUM port, 16-bit PSUM is TRN3-only) -> ~178us
    zero-overhead drain floor, ~225-245us with real per-op overhead.
    At 264.5us wall with drains 94% busy, remaining headroom is ~15-20us
    (boot+tail+per-op overhead), not a structural factor.

Host does all weight masking/stacking/transposing; device output is
[64, 8192] f32 per core, host concatenates and transposes.
"""

import sys

if "/opt/trn_rl_repo" not in sys.path:
    sys.path.insert(0, "/opt/trn_rl_repo")

import numpy as np

B = 65536
D = 32
L = 31
NCORES = 8
BC = B // NCORES  # 8192 per-core batch
FD = 1024             # legacy constant; HF = FD//2 = 512 is the batch tile
NT = BC // (FD // 2)  # 16 batch tiles of 512 per core

# layer pairs for L2/L3; last pair duplicates layer 30 (its W3 block is zero)
PAIRS = [(2 * q, min(2 * q + 1, L - 1)) for q in range(16)]

F16 = np.float16

DEDUP_LDW = False  # delete redundant InstLdweights post-schedule
# Tuned via interleaved A/B on hardware: l3s = one shared L3 psum bank
# (partition halves of one [128,512] tile serve two batch tiles) freeing a
# 5th L1 psum slot; sbp = deeper SBUF pools; l3i = L3 accumulation matmuls
# emitted inline after each pair drain instead of as a tail chain.
# d2* variants: 2-bank [128,1024] L1 psum tiles (2 layers each, bias folded
# into the matmul via the zT4 ones row) so PSUM slots free in bursts of two
# and adjacent row-tiled L1 matmuls co-issue into the PE array. Measured
# 513us: halving the PSUM slot count starves the PE in ~1.2us bursts and the
# HAM clock gate drops it to 1.2 GHz (mm dur ~633ns = cold latency). The
# 8-way single-bank rotation is load-bearing; keep it.
# pw = ACT spline-table prewarm overlapping input DMA; l3cs = L3 emitted as
# two M=32 col-tiled accumulation chains sharing one rhs (broadcast
# co-insert) — measured 424us: doubling L3 mm/sem count inflates every op's
# effective latency ~20%; do not use.
VARIANT = "l3s-sbq-l3i-zs-pw-c2-skw-sem1-pd"

_NC_CACHE = {}
LAST_RESULT = None  # BassKernelResults of the most recent run (for test.py)


def _dedup_ldweights(nc):
    """Remove LDWEIGHTS that reload the exact weights already resident in the
    same PE-array region. Runs after Tile scheduling (instruction order and
    semaphores final) and before Bacc lowering. Conservative: any overlapping
    region load or tiling-mode change invalidates, and only sync-free
    duplicates are deleted.
    """
    import concourse.mybir as mybir

    PE = mybir.EngineType.PE
    removed = 0
    for bb in nc.m.functions[0].blocks:
        loaded = {}
        cur_mode = None
        todel = []
        for ins in bb.instructions:
            if getattr(ins, "engine", None) != PE:
                continue
            tn = type(ins).__name__
            if tn == "InstLdweights":
                tp = ins.tile_position or (0, 0)
                tsz = ins.tile_size or (128, 128)
                if tsz != cur_mode:
                    loaded.clear()
                    cur_mode = tsz
                region = (tp[0], tp[0] + tsz[0], tp[1], tp[1] + tsz[1])
                ap = ins.ins[0]
                sig = (
                    getattr(ap, "memref", None),
                    getattr(ap, "offset", None),
                    str(getattr(ap, "ap", None)),
                    str(getattr(ap, "dtype", None)),
                    tuple(tp),
                    tuple(tsz),
                )
                si = ins.sync_info
                clean = si is None or (not si.on_wait and not si.on_update)
                if loaded.get(region) == sig and clean:
                    todel.append(ins)
                    removed += 1
                    continue
                for rk in list(loaded):
                    if not (
                        rk[1] <= region[0]
                        or region[1] <= rk[0]
                        or rk[3] <= region[2]
                        or region[3] <= rk[2]
                    ):
                        del loaded[rk]
                loaded[region] = sig
            elif tn == "InstMatmult":
                tsz = ins.tile_size or (128, 128)
                if tuple(tsz) != (cur_mode and tuple(cur_mode)):
                    if tsz != cur_mode:
                        loaded.clear()
                        cur_mode = tsz
        for ins in todel:
            bb.instructions.remove(ins)
            nc.inst_map.pop(ins.name, None)
    return removed


def _merge_redundant_self_waits(nc):
    """Post-scheduling pass: on the strict-FIFO drain engines (ACT/DVE), a
    wait on the engine's OWN completion semaphore whose threshold is <= the
    number of increments already emitted earlier in the same queue is
    trivially satisfied (serial in-order execution also subsumes every
    same-engine data hazard). Tile emits such self-waits carried on the
    drain op while pushing the REAL cross-engine wait (psum-fill) onto a
    standalone InstEventSemaphore right before it (~250 per engine, ~40-60ns
    each of saturated engine-queue time). Drop the redundant self-wait,
    migrate the standalone's wait onto the drain, delete the standalone.
    """
    import concourse.mybir as mybir

    ENGS = (mybir.EngineType.Activation, mybir.EngineType.DVE)
    changed = 0
    for fn in nc.m.functions:
        for bb in fn.blocks:
            for ENG in ENGS:
                q = [
                    i
                    for i in bb.instructions
                    if getattr(i, "engine", None) == ENG
                ]
                own = set()
                bad = set()
                for ins in q:
                    si = ins.sync_info
                    for u in (si.on_update or []) if si else []:
                        if u.update_mode == "sem-inc":
                            own.add(u.id)
                        else:
                            bad.add(u.id)
                own -= bad  # only sems this queue monotonically increments
                counts = {}
                prev = None
                todel = []
                for ins in q:
                    si = ins.sync_info
                    waits = list(si.on_wait or []) if si else []
                    if (
                        si is not None
                        and len(waits) == 1
                        and type(ins).__name__ != "InstEventSemaphore"
                        and waits[0].wait_mode == "sem-ge-imm"
                        and waits[0].id in own
                        and waits[0].wait_value <= counts.get(waits[0].id, 0)
                    ):
                        psi = prev.sync_info if prev is not None else None
                        if (
                            prev is not None
                            and type(prev).__name__ == "InstEventSemaphore"
                            and psi is not None
                            and len(psi.on_wait or []) == 1
                            and not (psi.on_update or [])
                        ):
                            ins.sync_info = mybir.SyncInfo(
                                on_wait=list(psi.on_wait),
                                on_update=list(si.on_update or []),
                            )
                            todel.append(prev)
                        else:
                            ins.sync_info = mybir.SyncInfo(
                                on_wait=[],
                                on_update=list(si.on_update or []),
                            )
                        changed += 1
                    si2 = ins.sync_info
                    for u in (si2.on_update or []) if si2 else []:
                        if u.update_mode == "sem-inc" and u.id in own:
                            counts[u.id] = counts.get(u.id, 0) + (
                                u.update_value or 1
                            )
                    prev = ins
                for d in todel:
                    bb.instructions.remove(d)
                    nc.inst_map.pop(d.name, None)
    return changed


def _build_nc():
    import concourse.mybir as mybir
    from concourse import bacc, tile

    flags = set(VARIANT.split("-"))

    f32 = mybir.dt.float32
    f16 = mybir.dt.float16
    ADD = mybir.AluOpType.add
    MAX = mybir.AluOpType.max
    RELU = mybir.ActivationFunctionType.Relu

    nc = bacc.Bacc("TRN2", target_bir_lowering=False, debug=False)

    zT4_d = nc.declare_dram_parameter("zT4", [128, BC], f16, isOutput=False)
    w1_d = nc.declare_dram_parameter("w1s", [128, 16 * 128], f16, isOutput=False)
    w2_d = nc.declare_dram_parameter("w2s", [128, 16 * 128], f16, isOutput=False)
    w3_d = nc.declare_dram_parameter("w3s", [128, 16 * 64], f16, isOutput=False)
    b1_d = nc.declare_dram_parameter("b1s", [128, L], f32, isOutput=False)
    b2_d = nc.declare_dram_parameter("b2s", [128, 16], f32, isOutput=False)
    b3_d = nc.declare_dram_parameter("b3s", [128, 1], f32, isOutput=False)
    out_d = nc.declare_dram_parameter("out", [64, BC], f32, isOutput=True)

    # greedy DVE/ACT balance for PSUM drains (calibrated ns per op at FD)
    if "s1" in flags:
        eng_time = [0.0, 285.0]
    elif "s2" in flags:
        eng_time = [329.0, 0.0]
    else:
        eng_time = [0.0, 0.0]

    if "cal" in flags:
        DVE_NS, ACT_NS = 800.0, 683.0
    elif "rA" in flags:
        DVE_NS, ACT_NS = 700.0, 570.0
    elif "rD" in flags:
        DVE_NS, ACT_NS = 658.0, 610.0
    elif "c2" in flags:
        # measured engine-busy per op (incl. queue effects) from the
        # 2026-08-07 trace: DVE 240.8us/353 ops, ACT 246.3us/408 ops
        DVE_NS, ACT_NS = 682.0, 604.0
    else:
        DVE_NS, ACT_NS = 658.0, 570.0

    if "pd" in flags:
        # mixed 512/1024-col drains: fd-aware costs (decode+access+stream)
        def dve_cost(fd):
            return 170.0 + fd / 0.96

        def act_cost(fd):
            return 175.0 + fd / 1.2
    else:
        def dve_cost(fd):
            return DVE_NS

        def act_cost(fd):
            return ACT_NS

    HF = FD // 2  # single-matmul moving dim (PSUM bank limit)

    with tile.TileContext(nc) as tc:
        with (
            tc.tile_pool(name="const", bufs=1) as const,
            tc.tile_pool(
                name="l1ps",
                bufs=(
                    2 if "pd" in flags
                    else 2 if "m1" in flags
                    else 4 if "l2b3" in flags
                    else 6 if "l16" in flags
                    else 4 if "l3s2" in flags
                    else 5 if "l3s" in flags
                    else 4
                ),
                space="PSUM",
            ) as l1ps,
            tc.tile_pool(name="l1pB", bufs=1, space="PSUM") as l1pB,
            tc.tile_pool(name="h1sp", bufs=2) as h1sp,
            tc.tile_pool(
                name="l2ps",
                bufs=(3 if "l2b3" in flags else 1 if "l16" in flags else 2),
                space="PSUM",
            ) as l2ps,
            tc.tile_pool(
                name="l3ps",
                bufs=(2 if "l3s2" in flags else 1 if "l3s" in flags else 2),
                space="PSUM",
            ) as l3ps,
            tc.tile_pool(
                name="h1p",
                bufs=(
                    6 if "m1" in flags
                    else 18 if "sbq" in flags
                    else 14 if "sbp" in flags
                    else 10
                ),
            ) as h1p,
            tc.tile_pool(
                name="h2p",
                bufs=(26 if "sbq" in flags else 20 if "sbp" in flags else 18),
            ) as h2p,
            tc.tile_pool(
                name="outp",
                bufs=(6 if "sbq" in flags else 5 if "sbp" in flags else 3),
            ) as outp,
        ):
            if "pw" in flags:
                # ACT spline-table prewarm: the one-time ACT_TABLE_LOAD
                # (~2.7us) overlaps the input DMA instead of delaying the
                # first real drain.
                warm = const.tile([128, 1], f32, name="warm")
                nc.vector.memset(warm[:], 0.0)
                warm2 = const.tile([128, 1], f32, name="warm2")
                nc.scalar.activation(warm2[:], warm[:], RELU, bias=0.0)

            zslices = []
            if "zs" in flags:
                # per-tile z slices: first L1 matmul waits only on slice 0
                zt0 = const.tile([128, HF], f16, name="zt_s0")
                nc.sync.dma_start(zt0[:], zT4_d[:, 0:HF])
                zslices.append(zt0)
            else:
                zt_all = const.tile([128, BC], f16, name="zt_all")
                nc.sync.dma_start(zt_all[:], zT4_d[:, :])
            # Boot DMAs spread across the three DGE paths (sync HWDGE ring,
            # scalar HWDGE ring, gpsimd SWDGE) in first-use order so their
            # ~2us fixed completion latencies overlap: w1 gates the first L1
            # matmul, b1 the first L1 drain, w2+b2 the first L2. w1s only
            # populates group slots 0-7 (cols 0:1024); the zero half is never
            # transferred, and groups 0-1 ride a small first chunk so the
            # first matmul is gated by zt0, not the full weight load.
            w1ta = const.tile([128, 2 * 128], f16, name="w1ta")
            nc.scalar.dma_start(w1ta[:], w1_d[:, 0 : 2 * 128])

            def w1sl(g, r0, r1):
                if g < 2:
                    return w1ta[r0:r1, 128 * g : 128 * (g + 1)]
                return w1tb[r0:r1, 128 * (g - 2) : 128 * (g - 1)]
            b1t = const.tile([128, L], f32, name="b1t")
            nc.gpsimd.dma_start(b1t[:], b1_d[:, :])
            b2t = const.tile([128, 16], f32, name="b2t")
            nc.gpsimd.dma_start(b2t[:], b2_d[:, :])
            # w2 split like w1: pairs 0-3 (128KB) land before the w1 bulk
            # so the first L2 matmuls are not gated at ~15us by the full
            # 512KB transfer (both drain engines measured a ~1.4us stall
            # at t~14.5us waiting on it).
            w2ta = const.tile([128, 4 * 128], f16, name="w2ta")
            nc.scalar.dma_start(w2ta[:], w2_d[:, 0 : 4 * 128])
            w1tb = const.tile([128, 6 * 128], f16, name="w1tb")
            nc.scalar.dma_start(w1tb[:], w1_d[:, 2 * 128 : 8 * 128])
            w2tb = const.tile([128, 12 * 128], f16, name="w2tb")
            nc.scalar.dma_start(w2tb[:], w2_d[:, 4 * 128 : 16 * 128])

            def w2sl(q, c0, c1):
                if q < 4:
                    return w2ta[:, 128 * q + c0 : 128 * q + c1]
                return w2tb[:, 128 * (q - 4) + c0 : 128 * (q - 4) + c1]
            w3t = const.tile([128, 16 * 64], f16, name="w3t")
            nc.sync.dma_start(w3t[:], w3_d[:, :])
            b3t = const.tile([128, 1], f32, name="b3t")
            nc.gpsimd.dma_start(b3t[:], b3_d[:, :])
            if "zs" in flags:
                for ti in range(1, NT):
                    zti = const.tile([128, HF], f16, name=f"zt_s{ti}")
                    nc.sync.dma_start(zti[:], zT4_d[:, HF * ti : HF * (ti + 1)])
                    zslices.append(zti)

            alt_ctr = [0]

            def drain(dst, src, bias_ap, relu=True, force=None):
                fd = src.shape[-1]
                if force is not None:
                    use_dve = force == 0
                elif "alt" in flags:
                    alt_ctr[0] += 1
                    use_dve = alt_ctr[0] % 2 == 0
                else:
                    use_dve = (
                        eng_time[0] + dve_cost(fd) <= eng_time[1] + act_cost(fd)
                    )
                if use_dve:
                    eng_time[0] += dve_cost(fd)
                    if relu and bias_ap is None:
                        nc.vector.tensor_scalar(dst, src, 0.0, None, MAX)
                    elif relu:
                        nc.vector.tensor_scalar(dst, src, bias_ap, 0.0, ADD, MAX)
                    else:
                        nc.vector.tensor_scalar(dst, src, bias_ap, None, ADD)
                else:
                    eng_time[1] += act_cost(fd)
                    fn = RELU if relu else mybir.ActivationFunctionType.Identity
                    nc.scalar.activation(
                        dst, src, fn, bias=0.0 if bias_ap is None else bias_ap
                    )
                return 0 if use_dve else 1

            ps3s = None
            # skw: software-pipelined emission — L2 work of group g-1 and L3
            # work of group g-2 are emitted after group g's L1 section, so
            # neither the PE nor the drain engines head-of-line block on
            # drains emitted in the same group. Queues carry (t, q, ...)
            # across group and tile boundaries; osb/output DMAs emit when
            # the q==15 L3 of an odd tile is processed.
            skw = "skw" in flags
            pend_l2 = []
            pend_l3 = []
            ps3h = [None]

            def skw_emit_l2(ents):
                out = []
                for ent in ents:
                    tt, q, ha, hb = ent
                    ps2 = l2ps.tile(
                        [128, HF], f32, tag="l2", name=f"ps2_{tt}_{q}"
                    )
                    nc.tensor.matmul(
                        ps2[0:64, :],
                        lhsT=w2sl(q, 0, 64),
                        rhs=ha[:],
                        start=True,
                        stop=True,
                        tile_position=(0, 0),
                    )
                    nc.tensor.matmul(
                        ps2[64:128, :],
                        lhsT=w2sl(q, 64, 128),
                        rhs=hb[:],
                        start=True,
                        stop=True,
                        tile_position=(0, 64),
                    )
                    h2 = h2p.tile([128, HF], f16, tag="h2", name=f"h2_{tt}_{q}")
                    drain(h2[:], ps2[:], b2t[:, q : q + 1])
                    out.append((tt, q, h2))
                return out

            def skw_emit_l3(ents):
                for tt, q, h2 in ents:
                    po = 64 * (tt % 2)
                    if q == 0 and tt % 2 == 0:
                        ps3h[0] = l3ps.tile(
                            [128, HF], f32, tag="l3", name=f"ps3s_{tt}"
                        )
                    nc.tensor.matmul(
                        ps3h[0][po : po + 64, :],
                        lhsT=w3t[:, 64 * q : 64 * (q + 1)],
                        rhs=h2[:],
                        start=(q == 0),
                        stop=(q == 15),
                        tile_position=(0, po),
                    )
                    if q == 15 and tt % 2 == 1:
                        osb = outp.tile(
                            [128, HF], f32, tag="o", name=f"osb_{tt}"
                        )
                        drain(osb[:], ps3h[0][:], b3t[:, 0:1], relu=False)
                        nc.sync.dma_start(
                            out_d[:, HF * (tt - 1) : HF * tt], osb[0:64, :]
                        )
                        eng = nc.scalar if tt == NT - 1 else nc.sync
                        eng.dma_start(
                            out_d[:, HF * tt : HF * (tt + 1)], osb[64:128, :]
                        )

            for t in range(NT):
                zt = zslices[t][:] if "zs" in flags else zt_all[:, HF * t : HF * (t + 1)]
                h2_tiles = []
                if "l3s" in flags and t % 2 == 0 and not skw:
                    ps3s = l3ps.tile(
                        [128, HF], f32, tag="l3", name=f"ps3s_{t}"
                    )
                if "fg" in flags:
                    # fine-grain: L1 half-group (2 layers) -> its L2 pair ->
                    # inline L3, shortening the h1->L2 dependency distance
                    for q in range(16):
                        fl = [x for x in (2 * q, 2 * q + 1) if x < L]
                        h1f = []
                        for lyr in fl:
                            gg, ii = divmod(lyr, 4)
                            ro = 32 * ii
                            ps = l1ps.tile(
                                [128, HF], f32, tag="l1", name=f"ps1_{t}_{lyr}"
                            )
                            nc.tensor.matmul(
                                ps[:],
                                lhsT=w1sl(gg, ro, ro + L),
                                rhs=zt[ro : ro + L, :],
                                start=True,
                                stop=True,
                                tile_position=(ro, 0),
                            )
                            h1 = h1p.tile(
                                [128, HF], f16, tag="h1", name=f"h1_{t}_{lyr}"
                            )
                            drain(h1[:], ps[:], b1t[:, lyr : lyr + 1])
                            h1f.append(h1)
                        if len(fl) == 1:
                            h1f.append(h1f[0])
                        ha, hb = h1f
                        ps2 = l2ps.tile(
                            [128, HF], f32, tag="l2", name=f"ps2_{t}_{q}"
                        )
                        nc.tensor.matmul(
                            ps2[0:64, :],
                            lhsT=w2sl(q, 0, 64),
                            rhs=ha[:],
                            start=True,
                            stop=True,
                            tile_position=(0, 0),
                        )
                        nc.tensor.matmul(
                            ps2[64:128, :],
                            lhsT=w2sl(q, 64, 128),
                            rhs=hb[:],
                            start=True,
                            stop=True,
                            tile_position=(0, 64),
                        )
                        h2 = h2p.tile(
                            [128, HF], f16, tag="h2", name=f"h2_{t}_{q}"
                        )
                        drain(h2[:], ps2[:], b2t[:, q : q + 1])
                        h2_tiles.append(h2)
                        po = 64 * (t % 2)
                        nc.tensor.matmul(
                            ps3s[po : po + 64, :],
                            lhsT=w3t[:, 64 * q : 64 * (q + 1)],
                            rhs=h2[:],
                            start=(q == 0),
                            stop=(q == 15),
                            tile_position=(0, po),
                        )
                for g in ([] if "fg" in flags else range(8)):
                    lyrs = [x for x in range(4 * g, 4 * g + 4) if x < L]
                    h1_tiles = []
                    if "m1" in flags:
                        # 4 concurrent row-tiled MMs (K=32 incl. bias row)
                        # into two [128,1024] psum tiles; one bias-free relu
                        # drain per tile covers 2 layers.
                        h1d = []
                        for half in range(2):
                            hl = lyrs[2 * half : 2 * half + 2]
                            if not hl:
                                continue
                            ps = l1ps.tile(
                                [128, 2 * HF], f32, tag="l1",
                                name=f"ps1_{t}_{g}_{half}",
                            )
                            for k, lyr in enumerate(hl):
                                ro = 64 * half + 32 * k
                                nc.tensor.matmul(
                                    ps[:, HF * k : HF * (k + 1)],
                                    lhsT=w1sl(g, ro, ro + 32),
                                    rhs=zt[ro : ro + 32, :],
                                    start=True,
                                    stop=True,
                                    tile_position=(ro, 0),
                                )
                            h1 = h1p.tile(
                                [128, 2 * HF], f16, tag="h1",
                                name=f"h1_{t}_{g}_{half}",
                            )
                            drain(h1[:], ps[:], None)
                            h1d.append(h1)
                        h1_tiles = [h1d[0][:, 0:HF], h1d[0][:, HF : 2 * HF]]
                        if len(h1d) > 1:
                            h1_tiles += [h1d[1][:, 0:HF], h1d[1][:, HF : 2 * HF]]
                        else:
                            h1_tiles += [h1_tiles[0], h1_tiles[0]]
                        if len(lyrs) == 3:
                            h1_tiles[3] = h1d[1][:, 0:HF]
                    elif "pd" in flags:
                        # paired L1: layers 2j/2j+1 of the group share one
                        # [128,1024] 2-bank psum (b1 folded via the zT4 ones
                        # row, K=32) -> ONE relu-only drain per pair. Boot
                        # tiles drain per-half to keep the fine-grained
                        # cadence that earns the HAM K=8/8 grant.
                        for half in range(2):
                            hl = lyrs[2 * half : 2 * half + 2]
                            if len(hl) == 2:
                                ps = l1ps.tile(
                                    [128, 2 * HF], f32, tag="l1",
                                    name=f"ps1_{t}_{g}_{half}",
                                )
                                for k, lyr in enumerate(hl):
                                    ro = 32 * (2 * half + k)
                                    nc.tensor.matmul(
                                        ps[:, HF * k : HF * (k + 1)],
                                        lhsT=w1sl(g, ro, ro + 32),
                                        rhs=zt[ro : ro + 32, :],
                                        start=True,
                                        stop=True,
                                        tile_position=(ro, 0),
                                    )
                                h1 = h1p.tile(
                                    [128, 2 * HF], f16, tag="h1",
                                    name=f"h1_{t}_{g}_{half}",
                                )
                                if t < 2:
                                    drain(h1[:, 0:HF], ps[:, 0:HF], None)
                                    drain(
                                        h1[:, HF : 2 * HF],
                                        ps[:, HF : 2 * HF],
                                        None,
                                    )
                                else:
                                    drain(h1[:], ps[:], None)
                                h1_tiles.append(h1[:, 0:HF])
                                h1_tiles.append(h1[:, HF : 2 * HF])
                            elif len(hl) == 1:
                                ro = 32 * (2 * half)
                                ps = l1pB.tile(
                                    [128, HF], f32, tag="l1b",
                                    name=f"ps1b_{t}",
                                )
                                nc.tensor.matmul(
                                    ps[:],
                                    lhsT=w1sl(g, ro, ro + 32),
                                    rhs=zt[ro : ro + 32, :],
                                    start=True,
                                    stop=True,
                                    tile_position=(ro, 0),
                                )
                                h1 = h1sp.tile(
                                    [128, HF], f16, tag="h1b", name=f"h1b_{t}"
                                )
                                drain(h1[:], ps[:], None)
                                h1_tiles.append(h1[:])
                                h1_tiles.append(h1[:])
                    else:
                        prev_e = None
                        for i, lyr in enumerate(lyrs):
                            ro = 32 * i
                            ps = l1ps.tile(
                                [128, HF], f32, tag="l1", name=f"ps1_{t}_{lyr}"
                            )
                            nc.tensor.matmul(
                                ps[:],
                                lhsT=w1sl(g, ro, ro + L),
                                rhs=zt[ro : ro + L, :],
                                start=True,
                                stop=True,
                                tile_position=(ro, 0),
                            )
                            h1 = h1p.tile(
                                [128, HF], f16, tag="h1", name=f"h1_{t}_{lyr}"
                            )
                            # pe2: the two h1 drains feeding one L2 pair go to
                            # different engines so they finish ~together and
                            # the col-tiled L2 matmuls co-issue (co-insert).
                            fe = (
                                1 - prev_e
                                if ("pe2" in flags and i % 2 == 1)
                                else None
                            )
                            prev_e = drain(
                                h1[:], ps[:], b1t[:, lyr : lyr + 1], force=fe
                            )
                            h1_tiles.append(h1)
                        if len(lyrs) == 3:
                            h1_tiles.append(h1_tiles[2])
                    if skw:
                        new_l3 = skw_emit_l2(pend_l2)  # L2 of group g-1
                        del pend_l2[:]
                        skw_emit_l3(pend_l3)           # L3 of group g-2
                        del pend_l3[:]
                        pend_l3.extend(new_l3)
                        for j in range(2):
                            q = 2 * g + j
                            pend_l2.append(
                                (t, q, h1_tiles[2 * j], h1_tiles[2 * j + 1])
                            )
                        continue
                    for j in range(2):
                        q = 2 * g + j
                        ha = h1_tiles[2 * j]
                        hb = h1_tiles[2 * j + 1]
                        ps2 = l2ps.tile(
                            [128, HF], f32, tag="l2", name=f"ps2_{t}_{q}"
                        )
                        nc.tensor.matmul(
                            ps2[0:64, :],
                            lhsT=w2sl(q, 0, 64),
                            rhs=ha[:],
                            start=True,
                            stop=True,
                            tile_position=(0, 0),
                        )
                        if not ("p15" in flags and q == 15):
                            nc.tensor.matmul(
                                ps2[64:128, :],
                                lhsT=w2sl(q, 64, 128),
                                rhs=hb[:],
                                start=True,
                                stop=True,
                                tile_position=(0, 64),
                            )
                        h2 = h2p.tile(
                            [128, HF], f16, tag="h2", name=f"h2_{t}_{q}"
                        )
                        drain(h2[:], ps2[:], b2t[:, q : q + 1])
                        h2_tiles.append(h2)
                        if "l3i" in flags:
                            po = 64 * (t % 2)
                            if "l3cs" in flags:
                                # two M=32 col-tiled chains sharing one rhs:
                                # the insert column broadcasts across both
                                # col-groups, ~halving L3 PE time.
                                for cs in range(2):
                                    nc.tensor.matmul(
                                        ps3s[po + 32 * cs : po + 32 * (cs + 1), :],
                                        lhsT=w3t[
                                            :, 64 * q + 32 * cs : 64 * q + 32 * (cs + 1)
                                        ],
                                        rhs=h2[:],
                                        start=(q == 0),
                                        stop=(q == 15),
                                        tile_position=(0, po + 32 * cs),
                                    )
                            else:
                                nc.tensor.matmul(
                                    ps3s[po : po + 64, :],
                                    lhsT=w3t[:, 64 * q : 64 * (q + 1)],
                                    rhs=h2[:],
                                    start=(q == 0),
                                    stop=(q == 15),
                                    tile_position=(0, po),
                                )
                if "l3s" in flags and not skw:
                    po = 64 * (t % 2)
                    if "l3i" not in flags:
                        for q in range(16):
                            nc.tensor.matmul(
                                ps3s[po : po + 64, :],
                                lhsT=w3t[:, 64 * q : 64 * (q + 1)],
                                rhs=h2_tiles[q][:],
                                start=(q == 0),
                                stop=(q == 15),
                                tile_position=(0, po),
                            )
                    if "ose" in flags:
                        osb = outp.tile([64, HF], f32, tag="o", name=f"osb_{t}")
                        drain(
                            osb[:], ps3s[po : po + 64, :],
                            b3t[po : po + 64, 0:1], relu=False,
                        )
                        nc.sync.dma_start(
                            out_d[:, HF * t : HF * (t + 1)], osb[:]
                        )
                    elif t % 2 == 1:
                        osb = outp.tile([128, HF], f32, tag="o", name=f"osb_{t}")
                        drain(osb[:], ps3s[:], b3t[:, 0:1], relu=False)
                        nc.sync.dma_start(
                            out_d[:, HF * (t - 1) : HF * t], osb[0:64, :]
                        )
                        # final tile: second store on the scalar HWDGE ring so
                        # the two ~2us completion receipts overlap at the tail
                        # (earlier tiles stay off the ACT queue - a DMA trigger
                        # there costs ~650ns of drain-critical ACT time).
                        eng = nc.scalar if t == NT - 1 else nc.sync
                        eng.dma_start(
                            out_d[:, HF * t : HF * (t + 1)], osb[64:128, :]
                        )
                elif not skw:
                    ps3 = l3ps.tile([64, HF], f32, tag="l3", name=f"ps3_{t}")
                    for q in range(16):
                        nc.tensor.matmul(
                            ps3[:],
                            lhsT=w3t[:, 64 * q : 64 * (q + 1)],
                            rhs=h2_tiles[q][:],
                            start=(q == 0),
                            stop=(q == 15),
                        )
                    osb = outp.tile([64, HF], f32, tag="o", name=f"osb_{t}")
                    drain(osb[:], ps3[:], b3t[0:64, 0:1], relu=False)
                    nc.sync.dma_start(out_d[:, HF * t : HF * (t + 1)], osb[:])
            if skw:
                while pend_l2 or pend_l3:
                    new_l3 = skw_emit_l2(pend_l2)
                    del pend_l2[:]
                    skw_emit_l3(pend_l3)
                    del pend_l3[:]
                    pend_l3.extend(new_l3)

    if DEDUP_LDW:
        n = _dedup_ldweights(nc)
        print(f"dedup_ldweights removed {n}")
    if "sem1" in flags:
        # event semaphores only exist once compile() creates them; run the
        # merge pass right after the second generate_event_semaphores call
        # (sems final, still pre-ISA-codegen)
        orig_ges = nc.generate_event_semaphores
        ges_calls = [0]

        def _ges_wrapped():
            orig_ges()
            ges_calls[0] += 1
            if ges_calls[0] == 2:
                n = _merge_redundant_self_waits(nc)
                print(f"merge_redundant_self_waits: {n}")

        nc.generate_event_semaphores = _ges_wrapped
    nc.finalize()
    return nc


def _build_nc_m2():
    """Layer-paired L1 build: layers 2k/2k+1 share one [128,1024] 2-bank PSUM
    tile (column halves), b1 folded into the K=32 matmul via the ones row
    already present in zT4/w1s, so ONE relu-only drain covers both layers.
    Halves the dominant L1 drain-op count (31 -> 15 big + 1 single per batch
    tile) while keeping a fine-grained slot rotation (2x2-bank L1A + 1-bank
    L1B + 2 L2 + 1 shared L3 = 8 banks) so the PE never idles long enough to
    drop the HAM clock. Side effect: an h1 pair lands in one SBUF tile, so
    the two col-tiled L2 matmuls of pair q become ready together and co-issue.
    """
    import concourse.mybir as mybir
    from concourse import bacc, tile

    flags = set(VARIANT.split("-"))

    f32 = mybir.dt.float32
    f16 = mybir.dt.float16
    ADD = mybir.AluOpType.add
    MAX = mybir.AluOpType.max
    RELU = mybir.ActivationFunctionType.Relu

    nc = bacc.Bacc("TRN2", target_bir_lowering=False, debug=False)

    zT4_d = nc.declare_dram_parameter("zT4", [128, BC], f16, isOutput=False)
    w1_d = nc.declare_dram_parameter("w1s", [128, 16 * 128], f16, isOutput=False)
    w2_d = nc.declare_dram_parameter("w2s", [128, 16 * 128], f16, isOutput=False)
    w3_d = nc.declare_dram_parameter("w3s", [128, 16 * 64], f16, isOutput=False)
    b1_d = nc.declare_dram_parameter("b1s", [128, L], f32, isOutput=False)
    b2_d = nc.declare_dram_parameter("b2s", [128, 16], f32, isOutput=False)
    b3_d = nc.declare_dram_parameter("b3s", [128, 1], f32, isOutput=False)
    out_d = nc.declare_dram_parameter("out", [64, BC], f32, isOutput=True)

    # fd-aware greedy DVE/ACT drain balance (decode+access overhead + stream)
    eng_time = [0.0, 0.0]

    def dve_cost(fd):
        return 170.0 + fd / 0.96

    def act_cost(fd):
        return 175.0 + fd / 1.2

    HF = FD // 2

    with tile.TileContext(nc) as tc:
        with (
            tc.tile_pool(name="const", bufs=1) as const,
            tc.tile_pool(name="l1pA", bufs=2, space="PSUM") as l1pA,
            tc.tile_pool(name="l1pB", bufs=1, space="PSUM") as l1pB,
            tc.tile_pool(name="l2ps", bufs=2, space="PSUM") as l2ps,
            tc.tile_pool(name="l3ps", bufs=1, space="PSUM") as l3ps,
            tc.tile_pool(name="h1p", bufs=8) as h1p,
            tc.tile_pool(name="h1sp", bufs=2) as h1sp,
            tc.tile_pool(name="h2p", bufs=10) as h2p,
            tc.tile_pool(name="outp", bufs=4) as outp,
        ):
            # ACT spline-table prewarm overlapping input DMA
            warm = const.tile([128, 1], f32, name="warm")
            nc.vector.memset(warm[:], 0.0)
            warm2 = const.tile([128, 1], f32, name="warm2")
            nc.scalar.activation(warm2[:], warm[:], RELU, bias=0.0)

            zslices = []
            zt0 = const.tile([128, HF], f16, name="zt_s0")
            nc.sync.dma_start(zt0[:], zT4_d[:, 0:HF])
            zslices.append(zt0)
            # boot DMAs spread over the three DGE paths in first-use order
            w1ta = const.tile([128, 2 * 128], f16, name="w1ta")
            nc.scalar.dma_start(w1ta[:], w1_d[:, 0 : 2 * 128])

            def w1sl(g, r0, r1):
                if g < 2:
                    return w1ta[r0:r1, 128 * g : 128 * (g + 1)]
                return w1tb[r0:r1, 128 * (g - 2) : 128 * (g - 1)]

            b2t = const.tile([128, 16], f32, name="b2t")
            nc.gpsimd.dma_start(b2t[:], b2_d[:, :])
            w2ta = const.tile([128, 4 * 128], f16, name="w2ta")
            nc.scalar.dma_start(w2ta[:], w2_d[:, 0 : 4 * 128])
            w1tb = const.tile([128, 6 * 128], f16, name="w1tb")
            nc.scalar.dma_start(w1tb[:], w1_d[:, 2 * 128 : 8 * 128])
            w2tb = const.tile([128, 12 * 128], f16, name="w2tb")
            nc.scalar.dma_start(w2tb[:], w2_d[:, 4 * 128 : 16 * 128])

            def w2sl(q, c0, c1):
                if q < 4:
                    return w2ta[:, 128 * q + c0 : 128 * q + c1]
                return w2tb[:, 128 * (q - 4) + c0 : 128 * (q - 4) + c1]

            w3t = const.tile([128, 16 * 64], f16, name="w3t")
            nc.sync.dma_start(w3t[:], w3_d[:, :])
            b3t = const.tile([128, 1], f32, name="b3t")
            nc.gpsimd.dma_start(b3t[:], b3_d[:, :])
            for ti in range(1, NT):
                zti = const.tile([128, HF], f16, name=f"zt_s{ti}")
                nc.sync.dma_start(zti[:], zT4_d[:, HF * ti : HF * (ti + 1)])
                zslices.append(zti)

            def drain(dst, src, bias_ap, relu=True):
                fd = src.shape[-1]
                use_dve = eng_time[0] + dve_cost(fd) <= eng_time[1] + act_cost(fd)
                if use_dve:
                    eng_time[0] += dve_cost(fd)
                    if relu and bias_ap is None:
                        nc.vector.tensor_scalar(dst, src, 0.0, None, MAX)
                    elif relu:
                        nc.vector.tensor_scalar(dst, src, bias_ap, 0.0, ADD, MAX)
                    else:
                        nc.vector.tensor_scalar(dst, src, bias_ap, None, ADD)
                else:
                    eng_time[1] += act_cost(fd)
                    fn = RELU if relu else mybir.ActivationFunctionType.Identity
                    nc.scalar.activation(
                        dst, src, fn, bias=0.0 if bias_ap is None else bias_ap
                    )

            ps3s = None
            for t in range(NT):
                zt = zslices[t]
                po = 64 * (t % 2)
                if t % 2 == 0:
                    ps3s = l3ps.tile([128, HF], f32, tag="l3", name=f"ps3s_{t}")
                for q in range(16):
                    if q < 15:
                        a, b = 2 * q, 2 * q + 1
                        g = q // 2
                        roa = 32 * (a % 4)
                        rob = 32 * (b % 4)
                        ps = l1pA.tile(
                            [128, 2 * HF], f32, tag="l1", name=f"ps1_{t}_{q}"
                        )
                        # K=32 incl. ones row -> b1 folded; different row
                        # bands -> the two mms co-issue back-to-back
                        nc.tensor.matmul(
                            ps[:, 0:HF],
                            lhsT=w1sl(g, roa, roa + 32),
                            rhs=zt[roa : roa + 32, :],
                            start=True,
                            stop=True,
                            tile_position=(roa, 0),
                        )
                        nc.tensor.matmul(
                            ps[:, HF : 2 * HF],
                            lhsT=w1sl(g, rob, rob + 32),
                            rhs=zt[rob : rob + 32, :],
                            start=True,
                            stop=True,
                            tile_position=(rob, 0),
                        )
                        h1 = h1p.tile(
                            [128, 2 * HF], f16, tag="h1", name=f"h1_{t}_{q}"
                        )
                        drain(h1[:], ps[:], None)
                        ha = h1[:, 0:HF]
                        hb = h1[:, HF : 2 * HF]
                    else:
                        # layer 30 single via the 1-bank B slot; q=15 pair
                        # duplicates it (w3s zero-masks the dup's output)
                        ps = l1pB.tile([128, HF], f32, tag="l1b", name=f"ps1b_{t}")
                        nc.tensor.matmul(
                            ps[:],
                            lhsT=w1sl(7, 64, 96),
                            rhs=zt[64:96, :],
                            start=True,
                            stop=True,
                            tile_position=(64, 0),
                        )
                        h1 = h1sp.tile([128, HF], f16, tag="h1b", name=f"h1b_{t}")
                        drain(h1[:], ps[:], None)
                        ha = h1[:]
                        hb = h1[:]
                    ps2 = l2ps.tile([128, HF], f32, tag="l2", name=f"ps2_{t}_{q}")
                    nc.tensor.matmul(
                        ps2[0:64, :],
                        lhsT=w2sl(q, 0, 64),
                        rhs=ha,
                        start=True,
                        stop=True,
                        tile_position=(0, 0),
                    )
                    nc.tensor.matmul(
                        ps2[64:128, :],
                        lhsT=w2sl(q, 64, 128),
                        rhs=hb,
                        start=True,
                        stop=True,
                        tile_position=(0, 64),
                    )
                    h2 = h2p.tile([128, HF], f16, tag="h2", name=f"h2_{t}_{q}")
                    drain(h2[:], ps2[:], b2t[:, q : q + 1])
                    nc.tensor.matmul(
                        ps3s[po : po + 64, :],
                        lhsT=w3t[:, 64 * q : 64 * (q + 1)],
                        rhs=h2[:],
                        start=(q == 0),
                        stop=(q == 15),
                        tile_position=(0, po),
                    )
                if t % 2 == 1:
                    osb = outp.tile([128, HF], f32, tag="o", name=f"osb_{t}")
                    drain(osb[:], ps3s[:], b3t[:, 0:1], relu=False)
                    nc.sync.dma_start(
                        out_d[:, HF * (t - 1) : HF * t], osb[0:64, :]
                    )
                    eng = nc.scalar if t == NT - 1 else nc.sync
                    eng.dma_start(
                        out_d[:, HF * t : HF * (t + 1)], osb[64:128, :]
                    )

    nc.finalize()
    return nc


def _build_nc_m3():
    """m2 drains + 2-step software-pipelined emission.

    Flat stream over global steps s = 16*t + k (k = layer-pair index). At
    step s we emit: L1 fills for step s, the h1 drain for step s-1, and the
    L2 fills + h2 drain + inline L3 matmul for step s-2. Every PE
    instruction's dependencies were produced >=1 full step (~1.9us) earlier,
    so the in-order PE queue never head-of-line blocks on a drain, PE idle
    stays in thin slices (HAM keeps K=8/8), and each drain engine always has
    a ready op (one [128,1024] h1 drain + one [128,512] h2 drain per step).
    """
    import concourse.mybir as mybir
    from concourse import bacc, tile

    flags = set(VARIANT.split("-"))

    f32 = mybir.dt.float32
    f16 = mybir.dt.float16
    ADD = mybir.AluOpType.add
    MAX = mybir.AluOpType.max
    RELU = mybir.ActivationFunctionType.Relu

    nc = bacc.Bacc("TRN2", target_bir_lowering=False, debug=False)

    zT4_d = nc.declare_dram_parameter("zT4", [128, BC], f16, isOutput=False)
    w1_d = nc.declare_dram_parameter("w1s", [128, 16 * 128], f16, isOutput=False)
    w2_d = nc.declare_dram_parameter("w2s", [128, 16 * 128], f16, isOutput=False)
    w3_d = nc.declare_dram_parameter("w3s", [128, 16 * 64], f16, isOutput=False)
    b1_d = nc.declare_dram_parameter("b1s", [128, L], f32, isOutput=False)
    b2_d = nc.declare_dram_parameter("b2s", [128, 16], f32, isOutput=False)
    b3_d = nc.declare_dram_parameter("b3s", [128, 1], f32, isOutput=False)
    out_d = nc.declare_dram_parameter("out", [64, BC], f32, isOutput=True)

    eng_time = [0.0, 0.0]

    def dve_cost(fd):
        return 170.0 + fd / 0.96

    def act_cost(fd):
        return 175.0 + fd / 1.2

    HF = FD // 2

    with tile.TileContext(nc) as tc:
        with (
            tc.tile_pool(name="const", bufs=1) as const,
            tc.tile_pool(name="l1pA", bufs=2, space="PSUM") as l1pA,
            tc.tile_pool(name="l2ps", bufs=3, space="PSUM") as l2ps,
            tc.tile_pool(name="l3ps", bufs=1, space="PSUM") as l3ps,
            tc.tile_pool(name="h1p", bufs=8) as h1p,
            tc.tile_pool(name="h1sp", bufs=2) as h1sp,
            tc.tile_pool(name="h2p", bufs=10) as h2p,
            tc.tile_pool(name="outp", bufs=4) as outp,
        ):
            warm = const.tile([128, 1], f32, name="warm")
            nc.vector.memset(warm[:], 0.0)
            warm2 = const.tile([128, 1], f32, name="warm2")
            nc.scalar.activation(warm2[:], warm[:], RELU, bias=0.0)

            zslices = []
            zt0 = const.tile([128, HF], f16, name="zt_s0")
            nc.sync.dma_start(zt0[:], zT4_d[:, 0:HF])
            zslices.append(zt0)
            w1ta = const.tile([128, 2 * 128], f16, name="w1ta")
            nc.scalar.dma_start(w1ta[:], w1_d[:, 0 : 2 * 128])

            def w1sl(g, r0, r1):
                if g < 2:
                    return w1ta[r0:r1, 128 * g : 128 * (g + 1)]
                return w1tb[r0:r1, 128 * (g - 2) : 128 * (g - 1)]

            b2t = const.tile([128, 16], f32, name="b2t")
            nc.gpsimd.dma_start(b2t[:], b2_d[:, :])
            w2ta = const.tile([128, 4 * 128], f16, name="w2ta")
            nc.scalar.dma_start(w2ta[:], w2_d[:, 0 : 4 * 128])
            w1tb = const.tile([128, 6 * 128], f16, name="w1tb")
            nc.scalar.dma_start(w1tb[:], w1_d[:, 2 * 128 : 8 * 128])
            w2tb = const.tile([128, 12 * 128], f16, name="w2tb")
            nc.scalar.dma_start(w2tb[:], w2_d[:, 4 * 128 : 16 * 128])

            def w2sl(q, c0, c1):
                if q < 4:
                    return w2ta[:, 128 * q + c0 : 128 * q + c1]
                return w2tb[:, 128 * (q - 4) + c0 : 128 * (q - 4) + c1]

            w3t = const.tile([128, 16 * 64], f16, name="w3t")
            nc.sync.dma_start(w3t[:], w3_d[:, :])
            b3t = const.tile([128, 1], f32, name="b3t")
            nc.gpsimd.dma_start(b3t[:], b3_d[:, :])
            for ti in range(1, NT):
                zti = const.tile([128, HF], f16, name=f"zt_s{ti}")
                nc.sync.dma_start(zti[:], zT4_d[:, HF * ti : HF * (ti + 1)])
                zslices.append(zti)

            def drain(dst, src, bias_ap, relu=True, force=None):
                fd = src.shape[-1]
                if force is not None:
                    use_dve = force == 0
                else:
                    use_dve = (
                        eng_time[0] + dve_cost(fd) <= eng_time[1] + act_cost(fd)
                    )
                if use_dve:
                    eng_time[0] += dve_cost(fd)
                    if relu and bias_ap is None:
                        nc.vector.tensor_scalar(dst, src, 0.0, None, MAX)
                    elif relu:
                        nc.vector.tensor_scalar(dst, src, bias_ap, 0.0, ADD, MAX)
                    else:
                        nc.vector.tensor_scalar(dst, src, bias_ap, None, ADD)
                else:
                    eng_time[1] += act_cost(fd)
                    fn = RELU if relu else mybir.ActivationFunctionType.Identity
                    nc.scalar.activation(
                        dst, src, fn, bias=0.0 if bias_ap is None else bias_ap
                    )

            S = NT * 16
            BOOT_STEPS = 32  # first 2 tiles: PE-saturating redundant fills
            # + baseline-like single-bank drains to win the HAM K=8/8 grant
            pend = {}  # step -> (ps_tile, h1_tile or None yet, t, k)
            ps3s = None

            def emit_l1_fills(s):
                t, k = divmod(s, 16)
                zt = zslices[t]
                # during boot, emit each fill twice (redundant recompute,
                # same-band so strictly serial): keeps the PE 100% saturated
                # at K=4/8 so the HAM monitor grants K=8/8 early and
                # deterministically, like the baseline's PE-bound boot
                rep = 2 if s < 16 else 1
                if k < 15:
                    a = 2 * k
                    g = k // 2
                    roa = 32 * (a % 4)
                    rob = roa + 32
                    ps = l1pA.tile([128, 2 * HF], f32, tag="l1", name=f"ps1_{t}_{k}")
                    for _ in range(rep):
                        nc.tensor.matmul(
                            ps[:, 0:HF],
                            lhsT=w1sl(g, roa, roa + 32),
                            rhs=zt[roa : roa + 32, :],
                            start=True,
                            stop=True,
                            tile_position=(roa, 0),
                        )
                    for _ in range(rep):
                        nc.tensor.matmul(
                            ps[:, HF : 2 * HF],
                            lhsT=w1sl(g, rob, rob + 32),
                            rhs=zt[rob : rob + 32, :],
                            start=True,
                            stop=True,
                            tile_position=(rob, 0),
                        )
                else:
                    ps = l2ps.tile([128, HF], f32, tag="l2", name=f"ps1b_{t}")
                    nc.tensor.matmul(
                        ps[:],
                        lhsT=w1sl(7, 64, 96),
                        rhs=zt[64:96, :],
                        start=True,
                        stop=True,
                        tile_position=(64, 0),
                    )
                pend[s] = [ps, None, t, k]

            def emit_h1_drain(s):
                ent = pend[s]
                ps, _, t, k = ent
                f1 = s % 2
                if k < 15:
                    h1 = h1p.tile([128, 2 * HF], f16, tag="h1", name=f"h1_{t}_{k}")
                    if s < BOOT_STEPS:
                        drain(h1[:, 0:HF], ps[:, 0:HF], None, force=s % 2)
                        drain(
                            h1[:, HF : 2 * HF],
                            ps[:, HF : 2 * HF],
                            None,
                            force=(s + 1) % 2,
                        )
                    else:
                        drain(h1[:], ps[:], None, force=f1)
                else:
                    h1 = h1sp.tile([128, HF], f16, tag="h1b", name=f"h1b_{t}")
                    drain(h1[:], ps[:], None, force=f1)
                ent[1] = h1

            h2pend = {}  # step -> (h2_tile, t, q)

            def emit_l2(s):
                ps, h1, t, k = pend.pop(s)
                q = k
                if k < 15:
                    ha = h1[:, 0:HF]
                    hb = h1[:, HF : 2 * HF]
                else:
                    ha = h1[:]
                    hb = h1[:]
                ps2 = l2ps.tile([128, HF], f32, tag="l2", name=f"ps2_{t}_{q}")
                rep = 2 if s < 16 else 1
                for _ in range(rep):
                    nc.tensor.matmul(
                        ps2[0:64, :],
                        lhsT=w2sl(q, 0, 64),
                        rhs=ha,
                        start=True,
                        stop=True,
                        tile_position=(0, 0),
                    )
                for _ in range(rep):
                    nc.tensor.matmul(
                        ps2[64:128, :],
                        lhsT=w2sl(q, 64, 128),
                        rhs=hb,
                        start=True,
                        stop=True,
                        tile_position=(0, 64),
                    )
                h2 = h2p.tile([128, HF], f16, tag="h2", name=f"h2_{t}_{q}")
                drain(h2[:], ps2[:], b2t[:, q : q + 1], force=(s + 1) % 2)
                h2pend[s] = (h2, t, q)

            def emit_l3(s):
                nonlocal ps3s
                h2, t, q = h2pend.pop(s)
                po = 64 * (t % 2)
                if q == 0 and t % 2 == 0:
                    ps3s = l3ps.tile([128, HF], f32, tag="l3", name=f"ps3s_{t}")
                nc.tensor.matmul(
                    ps3s[po : po + 64, :],
                    lhsT=w3t[:, 64 * q : 64 * (q + 1)],
                    rhs=h2[:],
                    start=(q == 0),
                    stop=(q == 15),
                    tile_position=(0, po),
                )
                if q == 15 and t % 2 == 1:
                    osb = outp.tile([128, HF], f32, tag="o", name=f"osb_{t}")
                    drain(osb[:], ps3s[:], b3t[:, 0:1], relu=False)
                    nc.sync.dma_start(
                        out_d[:, HF * (t - 1) : HF * t], osb[0:64, :]
                    )
                    eng = nc.scalar if t == NT - 1 else nc.sync
                    eng.dma_start(
                        out_d[:, HF * t : HF * (t + 1)], osb[64:128, :]
                    )

            # Per-step emission order tuned for the in-order PE queue: L2
            # fills first (deps resolved 2 steps ago - never block), L1
            # fills second (slot waits overlap L2 streams), L3 last and 5
            # steps deep so its h2 is long-drained. Drains interleave on
            # the engine side in the same cadence.
            for s in range(S + 5):
                if 1 <= s <= S:
                    emit_h1_drain(s - 1)
                if 2 <= s <= S + 1:
                    emit_l2(s - 2)
                if s < S:
                    emit_l1_fills(s)
                if 5 <= s <= S + 4:
                    emit_l3(s - 5)

    nc.finalize()
    return nc


def _build_nc_d2():
    """Pair-granular build: L1 layers 2q/2q+1 share one 2-bank PSUM tile.

    Per batch tile of 512 and pair q: two row-tiled K=32 L1 matmuls (bias via
    ones row) -> one [128,1024] relu drain -> two col-tiled L2 matmuls ->
    [128,512] bias+relu drain -> inline L3 accumulation matmul. The paired
    slot-free events let consecutive PE matmuls co-issue (tile_position
    concurrency), and halving the drain op count trims DVE/ACT overhead.
    """
    import concourse.mybir as mybir
    from concourse import bacc, tile

    flags = set(VARIANT.split("-"))

    f32 = mybir.dt.float32
    f16 = mybir.dt.float16
    ADD = mybir.AluOpType.add
    MAX = mybir.AluOpType.max
    RELU = mybir.ActivationFunctionType.Relu

    nc = bacc.Bacc("TRN2", target_bir_lowering=False, debug=False)

    zT4_d = nc.declare_dram_parameter("zT4", [128, BC], f16, isOutput=False)
    w1_d = nc.declare_dram_parameter("w1s", [128, 16 * 128], f16, isOutput=False)
    w2_d = nc.declare_dram_parameter("w2s", [128, 16 * 128], f16, isOutput=False)
    w3_d = nc.declare_dram_parameter("w3s", [128, 16 * 64], f16, isOutput=False)
    b1_d = nc.declare_dram_parameter("b1s", [128, L], f32, isOutput=False)
    b2_d = nc.declare_dram_parameter("b2s", [128, 16], f32, isOutput=False)
    b3_d = nc.declare_dram_parameter("b3s", [128, 1], f32, isOutput=False)
    out_d = nc.declare_dram_parameter("out", [64, BC], f32, isOutput=True)

    HF = FD // 2

    # greedy DVE/ACT drain balance, fd-aware effective ns per op
    eng_time = [0.0, 0.0]
    PAD = 80.0 if "pad" in flags else 0.0

    def dve_cost(fd):
        return (120.0 + fd) / 0.96 + PAD

    def act_cost(fd):
        return (172.0 + fd) / 1.2 + PAD

    with tile.TileContext(nc) as tc:
        with (
            tc.tile_pool(name="const", bufs=1) as const,
            tc.tile_pool(
                name="l1ps", bufs=(3 if "l1b3" in flags else 2), space="PSUM"
            ) as l1ps,
            tc.tile_pool(
                name="l2ps", bufs=(2 if "l1b3" in flags else 3), space="PSUM"
            ) as l2ps,
            tc.tile_pool(name="l3ps", bufs=1, space="PSUM") as l3ps,
            tc.tile_pool(name="h1p", bufs=10) as h1p,
            tc.tile_pool(name="h2p", bufs=20) as h2p,
            tc.tile_pool(name="outp", bufs=4) as outp,
        ):
            # ACT spline-table prewarm: dummy relu so the one-time
            # ACT_TABLE_LOAD overlaps the input DMA instead of delaying the
            # first real drain.
            warm = const.tile([128, 1], f32, name="warm")
            nc.vector.memset(warm[:], 0.0)
            warm2 = const.tile([128, 1], f32, name="warm2")
            nc.scalar.activation(warm2[:], warm[:], RELU, bias=0.0)

            zslices = []
            zt0 = const.tile([128, HF], f16, name="zt_s0")
            nc.sync.dma_start(zt0[:], zT4_d[:, 0:HF])
            zslices.append(zt0)
            w1t = const.tile([128, 16 * 128], f16, name="w1t")
            nc.sync.dma_start(w1t[:], w1_d[:, :])
            w2t = const.tile([128, 16 * 128], f16, name="w2t")
            nc.sync.dma_start(w2t[:], w2_d[:, :])
            w3t = const.tile([128, 16 * 64], f16, name="w3t")
            nc.sync.dma_start(w3t[:], w3_d[:, :])
            b2t = const.tile([128, 16], f32, name="b2t")
            nc.sync.dma_start(b2t[:], b2_d[:, :])
            b3t = const.tile([128, 1], f32, name="b3t")
            nc.sync.dma_start(b3t[:], b3_d[:, :])
            for ti in range(1, NT):
                zti = const.tile([128, HF], f16, name=f"zt_s{ti}")
                nc.sync.dma_start(zti[:], zT4_d[:, HF * ti : HF * (ti + 1)])
                zslices.append(zti)

            def drain(dst, src, bias_ap, relu=True):
                fd = src.shape[-1]
                use_dve = eng_time[0] + dve_cost(fd) <= eng_time[1] + act_cost(fd)
                if use_dve:
                    eng_time[0] += dve_cost(fd)
                    if relu and bias_ap is None:
                        nc.vector.tensor_scalar(dst, src, 0.0, None, MAX)
                    elif relu:
                        nc.vector.tensor_scalar(dst, src, bias_ap, 0.0, ADD, MAX)
                    else:
                        nc.vector.tensor_scalar(dst, src, bias_ap, None, ADD)
                else:
                    eng_time[1] += act_cost(fd)
                    fn = RELU if relu else mybir.ActivationFunctionType.Identity
                    nc.scalar.activation(
                        dst, src, fn, bias=0.0 if bias_ap is None else bias_ap
                    )

            ps3s = None
            for t in range(NT):
                zt = zslices[t]
                po = 64 * (t % 2)
                if t % 2 == 0:
                    ps3s = l3ps.tile([128, HF], f32, tag="l3", name=f"ps3s_{t}")
                for q in range(16):
                    ps = l1ps.tile([128, 2 * HF], f32, tag="l1", name=f"ps1_{t}_{q}")
                    for k in range(2):
                        lyr = 2 * q + k  # lyr 31 hits the zero band of w1s
                        gg, ii = divmod(lyr, 4)
                        ro = 32 * ii
                        nc.tensor.matmul(
                            ps[:, HF * k : HF * (k + 1)],
                            lhsT=w1t[ro : ro + 32, 128 * gg : 128 * (gg + 1)],
                            rhs=zt[ro : ro + 32, :],
                            start=True,
                            stop=True,
                            tile_position=(ro, 0),
                        )
                    h1 = h1p.tile([128, 2 * HF], f16, tag="h1", name=f"h1_{t}_{q}")
                    drain(h1[:], ps[:], None)
                    ps2 = l2ps.tile([128, HF], f32, tag="l2", name=f"ps2_{t}_{q}")
                    nc.tensor.matmul(
                        ps2[0:64, :],
                        lhsT=w2sl(q, 0, 64),
                        rhs=h1[:, 0:HF],
                        start=True,
                        stop=True,
                        tile_position=(0, 0),
                    )
                    nc.tensor.matmul(
                        ps2[64:128, :],
                        lhsT=w2sl(q, 64, 128),
                        rhs=h1[:, HF : 2 * HF],
                        start=True,
                        stop=True,
                        tile_position=(0, 64),
                    )
                    h2 = h2p.tile([128, HF], f16, tag="h2", name=f"h2_{t}_{q}")
                    drain(h2[:], ps2[:], b2t[:, q : q + 1])
                    nc.tensor.matmul(
                        ps3s[po : po + 64, :],
                        lhsT=w3t[:, 64 * q : 64 * (q + 1)],
                        rhs=h2[:],
                        start=(q == 0),
                        stop=(q == 15),
                        tile_position=(0, po),
                    )
                if t % 2 == 1:
                    osb = outp.tile([128, HF], f32, tag="o", name=f"osb_{t}")
                    drain(osb[:], ps3s[:], b3t[:, 0:1], relu=False)
                    nc.sync.dma_start(out_d[:, HF * (t - 1) : HF * t], osb[0:64, :])
                    nc.sync.dma_start(
                        out_d[:, HF * t : HF * (t + 1)], osb[64:128, :]
                    )

    nc.finalize()
    return nc


def _get_nc():
    if VARIANT not in _NC_CACHE:
        if VARIANT.startswith("m3"):
            _NC_CACHE[VARIANT] = _build_nc_m3()
        elif VARIANT.startswith("m2"):
            _NC_CACHE[VARIANT] = _build_nc_m2()
        elif VARIANT.startswith("d2"):
            _NC_CACHE[VARIANT] = _build_nc_d2()
        else:
            _NC_CACHE[VARIANT] = _build_nc()
    return _NC_CACHE[VARIANT]


def _prep_shared(W1, b1, W2, b2, Wout, bout, bout0):
    W1 = np.asarray(W1, np.float32)
    b1 = np.asarray(b1, np.float32)
    W2 = np.asarray(W2, np.float32)
    b2 = np.asarray(b2, np.float32)
    Wout = np.asarray(Wout, np.float32)
    bout = np.asarray(bout, np.float32)
    bout0 = np.asarray(bout0, np.float32)

    mask = np.tril(np.ones((L, L), np.float32))
    W1m = W1 * mask[:, None, :]  # [31, 128, 31]

    w1s = np.zeros((128, 16 * 128), F16)
    for g in range(8):
        for i in range(4):
            lyr = 4 * g + i
            if lyr >= L:
                break
            w1s[32 * i : 32 * i + L, 128 * g : 128 * (g + 1)] = W1m[lyr].T.astype(
                F16
            )
            w1s[32 * i + L, 128 * g : 128 * (g + 1)] = b1[lyr].astype(F16)
    b1s = np.ascontiguousarray(b1.T)  # [128, 31]

    w2s = np.zeros((128, 16 * 128), F16)
    b2s = np.zeros((128, 16), np.float32)
    for q, (a, bb) in enumerate(PAIRS):
        w2s[:, 128 * q : 128 * q + 64] = W2[a].T.astype(F16)
        w2s[:, 128 * q + 64 : 128 * (q + 1)] = W2[bb].T.astype(F16)
        b2s[0:64, q] = b2[a]
        b2s[64:128, q] = b2[bb]

    w3s = np.zeros((128, 16 * 64), F16)
    for q, (a, bb) in enumerate(PAIRS):
        blk = np.zeros((128, 64), np.float32)
        blk[0:64, 1 + a] = Wout[a][0]
        blk[0:64, 33 + a] = Wout[a][1]
        if 2 * q + 1 <= L - 1:  # real second layer (not the dup)
            blk[64:128, 1 + bb] = Wout[bb][0]
            blk[64:128, 33 + bb] = Wout[bb][1]
        w3s[:, 64 * q : 64 * (q + 1)] = blk.astype(F16)

    b3h = np.zeros(64, np.float32)
    b3h[0] = bout0[0]
    b3h[1 : 1 + L] = bout[:, 0]
    b3h[32] = bout0[1]
    b3h[33 : 33 + L] = bout[:, 1]
    b3s = np.concatenate([b3h, b3h]).reshape(128, 1)  # both L3 psum halves

    return w1s, w2s, w3s, b1s, b2s, b3s


def kernel(z, W1, b1, W2, b2, Wout, bout, bout0):
    global LAST_RESULT
    from concourse.bass_utils import run_bass_kernel_spmd

    z = np.asarray(z, np.float32)
    w1s, w2s, w3s, b1s, b2s, b3s = _prep_shared(W1, b1, W2, b2, Wout, bout, bout0)

    zin_T = np.ascontiguousarray(z[:, :L].T).astype(F16)  # [31, 65536]
    in_maps = []
    for c in range(NCORES):
        sl = zin_T[:, BC * c : BC * (c + 1)]
        zt4 = np.zeros((128, BC), F16)
        for i in range(4):
            zt4[32 * i : 32 * i + L] = sl
            zt4[32 * i + L] = 1.0  # ones row (feeds the K=32 bias fold in m1)
        in_maps.append(
            {
                "zT4": zt4,
                "w1s": w1s,
                "w2s": w2s,
                "w3s": w3s,
                "b1s": b1s,
                "b2s": b2s,
                "b3s": b3s,
            }
        )

    nc = _get_nc()
    try:
        res = run_bass_kernel_spmd(nc, in_maps, core_ids=list(range(NCORES)))
    except Exception:
        # transient device-unrecoverable states clear on the next attempt
        res = run_bass_kernel_spmd(nc, in_maps, core_ids=list(range(NCORES)))
    LAST_RESULT = res

    big = np.concatenate([res.results[c]["out"] for c in range(NCORES)], axis=1)
    mus = np.ascontiguousarray(big[:32].T).astype(np.float32, copy=False)
    lvs = np.ascontiguousarray(big[32:].T).astype(np.float32, copy=False)
    return mus, lvs



# revision 32
# speedup vs baseline: 1.0302x; 1.0302x over previous
"""Trainium2 Bass kernel for nn_ARPrior (stacked causal-prior MLPs).

Network (per sample, latent D=32, L=31 stacked layers):
    zin = z[:, :31]
    h1[l] = relu(W1m[l] @ zin + b1[l])   # [128], W1m causally masked
    h2[l] = relu(W2[l]  @ h1[l] + b2[l]) # [64]
    out[l] = Wout[l] @ h2[l] + bout[l]   # [2]  (mu, logvar)
    mus = [bout0[0], out[:,0]]; lvs = [bout0[1], out[:,1]]

Mapping (pure data parallel, batch 65536 sharded 8 ways -> 8192/core):
  - L1: K=31 -> 4 layers packed in the 128x128 PE array via row tiling
    (tile_position=(32i,0)), each writing its own PSUM bank.
  - L2: K=128, M=64 -> 2 layers packed via column tiling
    (tile_position=(0,0)/(0,64)) into one PSUM bank.
  - L3: M=2 per layer -> all 31 layers' output weights embedded in a
    block-diagonal [128,64] stationary per layer-pair, accumulated over
    16 matmuls into a single [64,512] PSUM tile per batch tile. Output
    columns are pre-arranged [mu(32) | logvar(32)]; bias adds bout/bout0.
  - The PSUM drain (bias+relu+fp16 cast, one pass over every h1/h2
    element) is the throughput wall (~283us/core combined across both
    drain-capable engines); ops are split greedily between ScalarE
    (activation Relu w/ per-partition bias AP) and VectorE (tensor_scalar
    add+max). Fine-grained single-bank PSUM rotation (4 L1 + 2 L2 + 2 L3
    slots) keeps the TensorE free of long stalls so the HAM clock gate
    stays at 2.4 GHz; coarser multi-bank drain variants measured slower
    despite fewer drain ops. Default variant: 5 L1 + 2 L2 + 1 shared L3
    psum slots, inline L3 accumulation. Final tuning: ACT-table prewarm,
    boot DMAs spread over the three DGE paths (w1 split so a 64KB first
    chunk ungates the first matmul; the zero half of w1s never
    transferred), final-tile output stores on two rings, and the greedy
    DVE/ACT split recalibrated to measured busy-per-op (682/604 ns, flag
    c2), and w2 split the same way as w1 (pairs 0-3 first) to kill a
    ~1.4us both-engine stall at t~14.5us waiting on the 512KB w2
    transfer: 297.2us/core (297,155/297,187 ns across two runs), engines
    balanced within 1.4us. Forcing drain-engine placement (pe2 flag, off)
    measured 308us - the greedy balancer's freedom beats deterministic
    pairing.
  - Profiled balance (per core, ~306us wall before the trims): TensorE
    union-busy 257us (mms ~203ns effective; tile_position pairs co-issue
    ~20-30% of the time), DVE 242us, ACT 249us. All three engines ~80-92%
    busy - a sharp local optimum. Two structural variants measured WORSE: d2 (2-bank
    [128,1024] L1 psums, bias folded via ones row -> half the L1 drain
    ops) hit 513us because halving the PSUM slot count starves the PE
    (mm latency ~633ns = slot head-wait/cold); l3cs (L3 split into two
    M=32 col-tiled chains sharing one rhs) hit 424us from doubled L3
    mm/sem traffic inflating every op ~20%. On TRN2 the drain floor is
    hard: only DVE (0.96GHz) + ACT (1.2GHz) can read PSUM at 1
    elem/lane/cyc (fp32 PSUM is 1x on DVE; 16-bit PSUM matmul output that
    would enable 2x_1P reads is TRN3-only; DMA/GPSIMD have no PSUM port).
  - All compute in fp16 (1 PE cycle/row vs 2 for fp32), fp32 PSUM/bias,
    rel err ~5e-4.
  - skw (current default, 262-264.5us/core, 11% over the 297us non-skw
    build): software-pipelined emission. L2 work of group g-1 and L3 work
    of group g-2 are emitted after group g's L1 section (queues cross tile
    boundaries; osb/output DMAs emit when the q==15 L3 of an odd tile is
    processed; post-loop flush drains the queues). This removes the
    in-order head-of-line blocking that previously idled all three
    engines ~13%: with skw, every instruction's dependencies were
    produced a full group (~2us) earlier, so the drain engines run
    saturated (mid-run idle 0.6-1.3us vs ~35us before; DVE 247us busy,
    ACT 249.7us busy, PE 215.8us on a 264.5us wall). Remaining idle is
    boot (~5-8us DMA/DGE init + HAM grant at ~17-22us) and tail (~8us
    final flush + output DMA receipts); both measured resistant: inlining
    the last tile cost +2us, tail-splitting osb is drain-work-negative.
  - Measured dead ends (2026-08-09 session), do not revisit without new
    evidence: (1) m2/m3 layer-paired [128,1024] L1 psums + relu-only
    1024-col drains (b1 folded via the zT4 ones row, 2-step skewed flat
    pipeline, parity-pinned drain engines): the 1024-col drain saves only
    5-8% engine time (DVE 1214ns vs 2x682, ACT 1106 vs 2x604) and the
    reduced PE load (<80% duty) makes the HAM K=8/8 grant a boot-time
    race that some cores lose entirely (whole run at 1.2GHz, 403us) plus
    mid-run K=4/8 relapses; best stable m3 = 299-311us. The HAM grant
    does NOT follow from continuous PE busy at K=4/8 (18 back-to-back
    zero-data warmup matmuls never tripped it; saturating real-data boot
    moved it only 60->26us); the fine-grained non-skw baseline earns it
    at ~17us reliably, and skw keeps PE >80% so it sticks. (2) fp8
    DoubleRow (2x PE): e4m3 quantization is ~2.2% rms per tensor and
    error propagates through the linear layers undiminished -> blows the
    2e-2 gate (fp16 build is at 5e-4). (3) c3 recalibration of the
    greedy split to raw measured per-op times (742/735) regressed to
    276us - the raw averages embed queue effects; c2's 682/604 ratio
    (~1.13 = true DVE/ACT throughput ratio) assigns correctly. (4)
    period-8 drain-engine patterns (DVE 3xh1+5xh2 / ACT 5xh1+3xh2):
    per-step makespan lumpiness beats the better period-average - 325us.
  - Roofline context: 48.8M hidden elems/core must cross PSUM->SBUF on
    exactly two engines (DVE 0.96GHz + ACT 1.2GHz, 1 fp32 elem/lane/cyc;
    DMA/GPSIMD have no PSUM port, 16-bit PSUM is TRN3-only) -> ~178us
    zero-overhead drain floor, ~225-245us with real per-op overhead.
    At 264.5us wall with drains 94% busy, remaining headroom is ~15-20us
    (boot+tail+per-op overhead), not a structural factor.

Host does all weight masking/stacking/transposing; device output is
[64, 8192] f32 per core, host concatenates and transposes.
"""

import sys

if "/opt/trn_rl_repo" not in sys.path:
    sys.path.insert(0, "/opt/trn_rl_repo")

import numpy as np

B = 65536
D = 32
L = 31
NCORES = 8
BC = B // NCORES  # 8192 per-core batch
FD = 1024             # legacy constant; HF = FD//2 = 512 is the batch tile
NT = BC // (FD // 2)  # 16 batch tiles of 512 per core

# layer pairs for L2/L3; last pair duplicates layer 30 (its W3 block is zero)
PAIRS = [(2 * q, min(2 * q + 1, L - 1)) for q in range(16)]

F16 = np.float16

DEDUP_LDW = False  # delete redundant InstLdweights post-schedule
# Tuned via interleaved A/B on hardware: l3s = one shared L3 psum bank
# (partition halves of one [128,512] tile serve two batch tiles) freeing a
# 5th L1 psum slot; sbp = deeper SBUF pools; l3i = L3 accumulation matmuls
# emitted inline after each pair drain instead of as a tail chain.
# d2* variants: 2-bank [128,1024] L1 psum tiles (2 layers each, bias folded
# into the matmul via the zT4 ones row) so PSUM slots free in bursts of two
# and adjacent row-tiled L1 matmuls co-issue into the PE array. Measured
# 513us: halving the PSUM slot count starves the PE in ~1.2us bursts and the
# HAM clock gate drops it to 1.2 GHz (mm dur ~633ns = cold latency). The
# 8-way single-bank rotation is load-bearing; keep it.
# pw = ACT spline-table prewarm overlapping input DMA; l3cs = L3 emitted as
# two M=32 col-tiled accumulation chains sharing one rhs (broadcast
# co-insert) — measured 424us: doubling L3 mm/sem count inflates every op's
# effective latency ~20%; do not use.
VARIANT = "l3s-sbq-l3i-zs-pw-c2-skw-dmv"

_NC_CACHE = {}
LAST_RESULT = None  # BassKernelResults of the most recent run (for test.py)


def _dedup_ldweights(nc):
    """Remove LDWEIGHTS that reload the exact weights already resident in the
    same PE-array region. Runs after Tile scheduling (instruction order and
    semaphores final) and before Bacc lowering. Conservative: any overlapping
    region load or tiling-mode change invalidates, and only sync-free
    duplicates are deleted.
    """
    import concourse.mybir as mybir

    PE = mybir.EngineType.PE
    removed = 0
    for bb in nc.m.functions[0].blocks:
        loaded = {}
        cur_mode = None
        todel = []
        for ins in bb.instructions:
            if getattr(ins, "engine", None) != PE:
                continue
            tn = type(ins).__name__
            if tn == "InstLdweights":
                tp = ins.tile_position or (0, 0)
                tsz = ins.tile_size or (128, 128)
                if tsz != cur_mode:
                    loaded.clear()
                    cur_mode = tsz
                region = (tp[0], tp[0] + tsz[0], tp[1], tp[1] + tsz[1])
                ap = ins.ins[0]
                sig = (
                    getattr(ap, "memref", None),
                    getattr(ap, "offset", None),
                    str(getattr(ap, "ap", None)),
                    str(getattr(ap, "dtype", None)),
                    tuple(tp),
                    tuple(tsz),
                )
                si = ins.sync_info
                clean = si is None or (not si.on_wait and not si.on_update)
                if loaded.get(region) == sig and clean:
                    todel.append(ins)
                    removed += 1
                    continue
                for rk in list(loaded):
                    if not (
                        rk[1] <= region[0]
                        or region[1] <= rk[0]
                        or rk[3] <= region[2]
                        or region[3] <= rk[2]
                    ):
                        del loaded[rk]
                loaded[region] = sig
            elif tn == "InstMatmult":
                tsz = ins.tile_size or (128, 128)
                if tuple(tsz) != (cur_mode and tuple(cur_mode)):
                    if tsz != cur_mode:
                        loaded.clear()
                        cur_mode = tsz
        for ins in todel:
            bb.instructions.remove(ins)
            nc.inst_map.pop(ins.name, None)
    return removed


def _merge_redundant_self_waits(nc):
    """Post-scheduling pass: on the strict-FIFO drain engines (ACT/DVE), a
    wait on the engine's OWN completion semaphore whose threshold is <= the
    number of increments already emitted earlier in the same queue is
    trivially satisfied (serial in-order execution also subsumes every
    same-engine data hazard). Tile emits such self-waits carried on the
    drain op while pushing the REAL cross-engine wait (psum-fill) onto a
    standalone InstEventSemaphore right before it (~250 per engine, ~40-60ns
    each of saturated engine-queue time). Drop the redundant self-wait,
    migrate the standalone's wait onto the drain, delete the standalone.
    """
    import concourse.mybir as mybir

    ENGS = (mybir.EngineType.Activation, mybir.EngineType.DVE)
    changed = 0
    for fn in nc.m.functions:
        for bb in fn.blocks:
            for ENG in ENGS:
                q = [
                    i
                    for i in bb.instructions
                    if getattr(i, "engine", None) == ENG
                ]
                own = set()
                bad = set()
                for ins in q:
                    si = ins.sync_info
                    for u in (si.on_update or []) if si else []:
                        if u.update_mode == "sem-inc":
                            own.add(u.id)
                        else:
                            bad.add(u.id)
                own -= bad  # only sems this queue monotonically increments
                counts = {}
                prev = None
                todel = []
                for ins in q:
                    si = ins.sync_info
                    waits = list(si.on_wait or []) if si else []
                    if (
                        si is not None
                        and len(waits) == 1
                        and type(ins).__name__ != "InstEventSemaphore"
                        and waits[0].wait_mode == "sem-ge-imm"
                        and waits[0].id in own
                        and waits[0].wait_value <= counts.get(waits[0].id, 0)
                    ):
                        psi = prev.sync_info if prev is not None else None
                        if (
                            prev is not None
                            and type(prev).__name__ == "InstEventSemaphore"
                            and psi is not None
                            and len(psi.on_wait or []) == 1
                            and not (psi.on_update or [])
                        ):
                            ins.sync_info = mybir.SyncInfo(
                                on_wait=list(psi.on_wait),
                                on_update=list(si.on_update or []),
                            )
                            todel.append(prev)
                        else:
                            ins.sync_info = mybir.SyncInfo(
                                on_wait=[],
                                on_update=list(si.on_update or []),
                            )
                        changed += 1
                    si2 = ins.sync_info
                    for u in (si2.on_update or []) if si2 else []:
                        if u.update_mode == "sem-inc" and u.id in own:
                            counts[u.id] = counts.get(u.id, 0) + (
                                u.update_value or 1
                            )
                    prev = ins
                for d in todel:
                    bb.instructions.remove(d)
                    nc.inst_map.pop(d.name, None)
    return changed


def _build_nc():
    import concourse.mybir as mybir
    from concourse import bacc, tile

    flags = set(VARIANT.split("-"))

    f32 = mybir.dt.float32
    f16 = mybir.dt.float16
    ADD = mybir.AluOpType.add
    MAX = mybir.AluOpType.max
    RELU = mybir.ActivationFunctionType.Relu

    nc = bacc.Bacc("TRN2", target_bir_lowering=False, debug=False)

    zT4_d = nc.declare_dram_parameter("zT4", [128, BC], f16, isOutput=False)
    w1_d = nc.declare_dram_parameter("w1s", [128, 16 * 128], f16, isOutput=False)
    w2_d = nc.declare_dram_parameter("w2s", [128, 16 * 128], f16, isOutput=False)
    w3_d = nc.declare_dram_parameter("w3s", [128, 16 * 64], f16, isOutput=False)
    b1_d = nc.declare_dram_parameter("b1s", [128, L], f32, isOutput=False)
    b2_d = nc.declare_dram_parameter("b2s", [128, 16], f32, isOutput=False)
    b3_d = nc.declare_dram_parameter("b3s", [128, 1], f32, isOutput=False)
    out_d = nc.declare_dram_parameter("out", [64, BC], f32, isOutput=True)

    # greedy DVE/ACT balance for PSUM drains (calibrated ns per op at FD)
    if "s1" in flags:
        eng_time = [0.0, 285.0]
    elif "s2" in flags:
        eng_time = [329.0, 0.0]
    else:
        eng_time = [0.0, 0.0]

    if "cal" in flags:
        DVE_NS, ACT_NS = 800.0, 683.0
    elif "rA" in flags:
        DVE_NS, ACT_NS = 700.0, 570.0
    elif "rD" in flags:
        DVE_NS, ACT_NS = 658.0, 610.0
    elif "c2" in flags:
        # measured engine-busy per op (incl. queue effects) from the
        # 2026-08-07 trace: DVE 240.8us/353 ops, ACT 246.3us/408 ops
        DVE_NS, ACT_NS = 682.0, 604.0
    else:
        DVE_NS, ACT_NS = 658.0, 570.0

    if "pd" in flags:
        # mixed 512/1024-col drains: fd-aware costs (decode+access+stream)
        def dve_cost(fd):
            return 170.0 + fd / 0.96

        def act_cost(fd):
            return 175.0 + fd / 1.2
    else:
        def dve_cost(fd):
            return DVE_NS

        def act_cost(fd):
            return ACT_NS

    HF = FD // 2  # single-matmul moving dim (PSUM bank limit)

    with tile.TileContext(nc) as tc:
        with (
            tc.tile_pool(name="const", bufs=1) as const,
            tc.tile_pool(
                name="l1ps",
                bufs=(
                    2 if "pd" in flags
                    else 2 if "m1" in flags
                    else 4 if "l2b3" in flags
                    else 6 if "l16" in flags
                    else 4 if "l3s2" in flags
                    else 5 if "l3s" in flags
                    else 4
                ),
                space="PSUM",
            ) as l1ps,
            tc.tile_pool(name="l1pB", bufs=1, space="PSUM") as l1pB,
            tc.tile_pool(name="h1sp", bufs=2) as h1sp,
            tc.tile_pool(
                name="l2ps",
                bufs=(3 if "l2b3" in flags else 1 if "l16" in flags else 2),
                space="PSUM",
            ) as l2ps,
            tc.tile_pool(
                name="l3ps",
                bufs=(2 if "l3s2" in flags else 1 if "l3s" in flags else 2),
                space="PSUM",
            ) as l3ps,
            tc.tile_pool(
                name="h1p",
                bufs=(
                    6 if "m1" in flags
                    else 18 if "sbq" in flags
                    else 14 if "sbp" in flags
                    else 10
                ),
            ) as h1p,
            tc.tile_pool(
                name="h2p",
                bufs=(26 if "sbq" in flags else 20 if "sbp" in flags else 18),
            ) as h2p,
            tc.tile_pool(
                name="outp",
                bufs=(6 if "sbq" in flags else 5 if "sbp" in flags else 3),
            ) as outp,
        ):
            if "pw" in flags:
                # ACT spline-table prewarm: the one-time ACT_TABLE_LOAD
                # (~2.7us) overlaps the input DMA instead of delaying the
                # first real drain.
                warm = const.tile([128, 1], f32, name="warm")
                nc.vector.memset(warm[:], 0.0)
                warm2 = const.tile([128, 1], f32, name="warm2")
                nc.scalar.activation(warm2[:], warm[:], RELU, bias=0.0)

            zslices = []
            if "zs" in flags:
                # per-tile z slices: first L1 matmul waits only on slice 0
                zt0 = const.tile([128, HF], f16, name="zt_s0")
                nc.sync.dma_start(zt0[:], zT4_d[:, 0:HF])
                zslices.append(zt0)
            else:
                zt_all = const.tile([128, BC], f16, name="zt_all")
                nc.sync.dma_start(zt_all[:], zT4_d[:, :])
            # Boot DMAs spread across the three DGE paths (sync HWDGE ring,
            # scalar HWDGE ring, gpsimd SWDGE) in first-use order so their
            # ~2us fixed completion latencies overlap: w1 gates the first L1
            # matmul, b1 the first L1 drain, w2+b2 the first L2. w1s only
            # populates group slots 0-7 (cols 0:1024); the zero half is never
            # transferred, and groups 0-1 ride a small first chunk so the
            # first matmul is gated by zt0, not the full weight load.
            w1ta = const.tile([128, 2 * 128], f16, name="w1ta")
            nc.scalar.dma_start(w1ta[:], w1_d[:, 0 : 2 * 128])

            def w1sl(g, r0, r1):
                if g < 2:
                    return w1ta[r0:r1, 128 * g : 128 * (g + 1)]
                return w1tb[r0:r1, 128 * (g - 2) : 128 * (g - 1)]
            b1t = const.tile([128, L], f32, name="b1t")
            nc.gpsimd.dma_start(b1t[:], b1_d[:, :])
            b2t = const.tile([128, 16], f32, name="b2t")
            nc.gpsimd.dma_start(b2t[:], b2_d[:, :])
            # w2 split like w1: pairs 0-3 (128KB) land before the w1 bulk
            # so the first L2 matmuls are not gated at ~15us by the full
            # 512KB transfer (both drain engines measured a ~1.4us stall
            # at t~14.5us waiting on it).
            w2ta = const.tile([128, 4 * 128], f16, name="w2ta")
            nc.scalar.dma_start(w2ta[:], w2_d[:, 0 : 4 * 128])
            # dmv: the bulk w1/w2 triggers ride the idle sync queue instead
            # of costing ~650ns each of drain-critical ACT time at ~t=9-15us
            bulk_eng = nc.sync if "dmv" in flags else nc.scalar
            w1tb = const.tile([128, 6 * 128], f16, name="w1tb")
            bulk_eng.dma_start(w1tb[:], w1_d[:, 2 * 128 : 8 * 128])
            w2tb = const.tile([128, 12 * 128], f16, name="w2tb")
            bulk_eng.dma_start(w2tb[:], w2_d[:, 4 * 128 : 16 * 128])

            def w2sl(q, c0, c1):
                if q < 4:
                    return w2ta[:, 128 * q + c0 : 128 * q + c1]
                return w2tb[:, 128 * (q - 4) + c0 : 128 * (q - 4) + c1]
            w3t = const.tile([128, 16 * 64], f16, name="w3t")
            nc.sync.dma_start(w3t[:], w3_d[:, :])
            b3t = const.tile([128, 1], f32, name="b3t")
            nc.gpsimd.dma_start(b3t[:], b3_d[:, :])
            if "zs" in flags:
                for ti in range(1, NT):
                    zti = const.tile([128, HF], f16, name=f"zt_s{ti}")
                    nc.sync.dma_start(zti[:], zT4_d[:, HF * ti : HF * (ti + 1)])
                    zslices.append(zti)

            alt_ctr = [0]

            def drain(dst, src, bias_ap, relu=True, force=None):
                fd = src.shape[-1]
                if force is not None:
                    use_dve = force == 0
                elif "alt" in flags:
                    alt_ctr[0] += 1
                    use_dve = alt_ctr[0] % 2 == 0
                else:
                    use_dve = (
                        eng_time[0] + dve_cost(fd) <= eng_time[1] + act_cost(fd)
                    )
                if use_dve:
                    eng_time[0] += dve_cost(fd)
                    if relu and bias_ap is None:
                        nc.vector.tensor_scalar(dst, src, 0.0, None, MAX)
                    elif relu:
                        nc.vector.tensor_scalar(dst, src, bias_ap, 0.0, ADD, MAX)
                    else:
                        nc.vector.tensor_scalar(dst, src, bias_ap, None, ADD)
                else:
                    eng_time[1] += act_cost(fd)
                    fn = RELU if relu else mybir.ActivationFunctionType.Identity
                    nc.scalar.activation(
                        dst, src, fn, bias=0.0 if bias_ap is None else bias_ap
                    )
                return 0 if use_dve else 1

            ps3s = None
            # skw: software-pipelined emission — L2 work of group g-1 and L3
            # work of group g-2 are emitted after group g's L1 section, so
            # neither the PE nor the drain engines head-of-line block on
            # drains emitted in the same group. Queues carry (t, q, ...)
            # across group and tile boundaries; osb/output DMAs emit when
            # the q==15 L3 of an odd tile is processed.
            skw = "skw" in flags
            pend_l2 = []
            pend_l3 = []
            ps3h = [None]

            def skw_emit_l2(ents):
                out = []
                for ent in ents:
                    tt, q, ha, hb = ent
                    ps2 = l2ps.tile(
                        [128, HF], f32, tag="l2", name=f"ps2_{tt}_{q}"
                    )
                    nc.tensor.matmul(
                        ps2[0:64, :],
                        lhsT=w2sl(q, 0, 64),
                        rhs=ha[:],
                        start=True,
                        stop=True,
                        tile_position=(0, 0),
                    )
                    nc.tensor.matmul(
                        ps2[64:128, :],
                        lhsT=w2sl(q, 64, 128),
                        rhs=hb[:],
                        start=True,
                        stop=True,
                        tile_position=(0, 64),
                    )
                    h2 = h2p.tile([128, HF], f16, tag="h2", name=f"h2_{tt}_{q}")
                    drain(h2[:], ps2[:], b2t[:, q : q + 1])
                    out.append((tt, q, h2))
                return out

            def skw_emit_l3(ents):
                for tt, q, h2 in ents:
                    po = 64 * (tt % 2)
                    if q == 0 and tt % 2 == 0:
                        ps3h[0] = l3ps.tile(
                            [128, HF], f32, tag="l3", name=f"ps3s_{tt}"
                        )
                    nc.tensor.matmul(
                        ps3h[0][po : po + 64, :],
                        lhsT=w3t[:, 64 * q : 64 * (q + 1)],
                        rhs=h2[:],
                        start=(q == 0),
                        stop=(q == 15),
                        tile_position=(0, po),
                    )
                    if q == 15 and tt % 2 == 1:
                        osb = outp.tile(
                            [128, HF], f32, tag="o", name=f"osb_{tt}"
                        )
                        drain(osb[:], ps3h[0][:], b3t[:, 0:1], relu=False)
                        nc.sync.dma_start(
                            out_d[:, HF * (tt - 1) : HF * tt], osb[0:64, :]
                        )
                        eng = nc.scalar if tt == NT - 1 else nc.sync
                        eng.dma_start(
                            out_d[:, HF * tt : HF * (tt + 1)], osb[64:128, :]
                        )

            for t in range(NT):
                zt = zslices[t][:] if "zs" in flags else zt_all[:, HF * t : HF * (t + 1)]
                h2_tiles = []
                if "l3s" in flags and t % 2 == 0 and not skw:
                    ps3s = l3ps.tile(
                        [128, HF], f32, tag="l3", name=f"ps3s_{t}"
                    )
                if "fg" in flags:
                    # fine-grain: L1 half-group (2 layers) -> its L2 pair ->
                    # inline L3, shortening the h1->L2 dependency distance
                    for q in range(16):
                        fl = [x for x in (2 * q, 2 * q + 1) if x < L]
                        h1f = []
                        for lyr in fl:
                            gg, ii = divmod(lyr, 4)
                            ro = 32 * ii
                            ps = l1ps.tile(
                                [128, HF], f32, tag="l1", name=f"ps1_{t}_{lyr}"
                            )
                            nc.tensor.matmul(
                                ps[:],
                                lhsT=w1sl(gg, ro, ro + L),
                                rhs=zt[ro : ro + L, :],
                                start=True,
                                stop=True,
                                tile_position=(ro, 0),
                            )
                            h1 = h1p.tile(
                                [128, HF], f16, tag="h1", name=f"h1_{t}_{lyr}"
                            )
                            drain(h1[:], ps[:], b1t[:, lyr : lyr + 1])
                            h1f.append(h1)
                        if len(fl) == 1:
                            h1f.append(h1f[0])
                        ha, hb = h1f
                        ps2 = l2ps.tile(
                            [128, HF], f32, tag="l2", name=f"ps2_{t}_{q}"
                        )
                        nc.tensor.matmul(
                            ps2[0:64, :],
                            lhsT=w2sl(q, 0, 64),
                            rhs=ha[:],
                            start=True,
                            stop=True,
                            tile_position=(0, 0),
                        )
                        nc.tensor.matmul(
                            ps2[64:128, :],
                            lhsT=w2sl(q, 64, 128),
                            rhs=hb[:],
                            start=True,
                            stop=True,
                            tile_position=(0, 64),
                        )
                        h2 = h2p.tile(
                            [128, HF], f16, tag="h2", name=f"h2_{t}_{q}"
                        )
                        drain(h2[:], ps2[:], b2t[:, q : q + 1])
                        h2_tiles.append(h2)
                        po = 64 * (t % 2)
                        nc.tensor.matmul(
                            ps3s[po : po + 64, :],
                            lhsT=w3t[:, 64 * q : 64 * (q + 1)],
                            rhs=h2[:],
                            start=(q == 0),
                            stop=(q == 15),
                            tile_position=(0, po),
                        )
                for g in ([] if "fg" in flags else range(8)):
                    lyrs = [x for x in range(4 * g, 4 * g + 4) if x < L]
                    h1_tiles = []
                    if "m1" in flags:
                        # 4 concurrent row-tiled MMs (K=32 incl. bias row)
                        # into two [128,1024] psum tiles; one bias-free relu
                        # drain per tile covers 2 layers.
                        h1d = []
                        for half in range(2):
                            hl = lyrs[2 * half : 2 * half + 2]
                            if not hl:
                                continue
                            ps = l1ps.tile(
                                [128, 2 * HF], f32, tag="l1",
                                name=f"ps1_{t}_{g}_{half}",
                            )
                            for k, lyr in enumerate(hl):
                                ro = 64 * half + 32 * k
                                nc.tensor.matmul(
                                    ps[:, HF * k : HF * (k + 1)],
                                    lhsT=w1sl(g, ro, ro + 32),
                                    rhs=zt[ro : ro + 32, :],
                                    start=True,
                                    stop=True,
                                    tile_position=(ro, 0),
                                )
                            h1 = h1p.tile(
                                [128, 2 * HF], f16, tag="h1",
                                name=f"h1_{t}_{g}_{half}",
                            )
                            drain(h1[:], ps[:], None)
                            h1d.append(h1)
                        h1_tiles = [h1d[0][:, 0:HF], h1d[0][:, HF : 2 * HF]]
                        if len(h1d) > 1:
                            h1_tiles += [h1d[1][:, 0:HF], h1d[1][:, HF : 2 * HF]]
                        else:
                            h1_tiles += [h1_tiles[0], h1_tiles[0]]
                        if len(lyrs) == 3:
                            h1_tiles[3] = h1d[1][:, 0:HF]
                    elif "pd" in flags:
                        # paired L1: layers 2j/2j+1 of the group share one
                        # [128,1024] 2-bank psum (b1 folded via the zT4 ones
                        # row, K=32) -> ONE relu-only drain per pair. Boot
                        # tiles drain per-half to keep the fine-grained
                        # cadence that earns the HAM K=8/8 grant.
                        for half in range(2):
                            hl = lyrs[2 * half : 2 * half + 2]
                            if len(hl) == 2:
                                ps = l1ps.tile(
                                    [128, 2 * HF], f32, tag="l1",
                                    name=f"ps1_{t}_{g}_{half}",
                                )
                                for k, lyr in enumerate(hl):
                                    ro = 32 * (2 * half + k)
                                    nc.tensor.matmul(
                                        ps[:, HF * k : HF * (k + 1)],
                                        lhsT=w1sl(g, ro, ro + 32),
                                        rhs=zt[ro : ro + 32, :],
                                        start=True,
                                        stop=True,
                                        tile_position=(ro, 0),
                                    )
                                h1 = h1p.tile(
                                    [128, 2 * HF], f16, tag="h1",
                                    name=f"h1_{t}_{g}_{half}",
                                )
                                if t < 2:
                                    drain(h1[:, 0:HF], ps[:, 0:HF], None)
                                    drain(
                                        h1[:, HF : 2 * HF],
                                        ps[:, HF : 2 * HF],
                                        None,
                                    )
                                else:
                                    drain(h1[:], ps[:], None)
                                h1_tiles.append(h1[:, 0:HF])
                                h1_tiles.append(h1[:, HF : 2 * HF])
                            elif len(hl) == 1:
                                ro = 32 * (2 * half)
                                ps = l1pB.tile(
                                    [128, HF], f32, tag="l1b",
                                    name=f"ps1b_{t}",
                                )
                                nc.tensor.matmul(
                                    ps[:],
                                    lhsT=w1sl(g, ro, ro + 32),
                                    rhs=zt[ro : ro + 32, :],
                                    start=True,
                                    stop=True,
                                    tile_position=(ro, 0),
                                )
                                h1 = h1sp.tile(
                                    [128, HF], f16, tag="h1b", name=f"h1b_{t}"
                                )
                                drain(h1[:], ps[:], None)
                                h1_tiles.append(h1[:])
                                h1_tiles.append(h1[:])
                    else:
                        prev_e = None
                        for i, lyr in enumerate(lyrs):
                            ro = 32 * i
                            ps = l1ps.tile(
                                [128, HF], f32, tag="l1", name=f"ps1_{t}_{lyr}"
                            )
                            nc.tensor.matmul(
                                ps[:],
                                lhsT=w1sl(g, ro, ro + L),
                                rhs=zt[ro : ro + L, :],
                                start=True,
                                stop=True,
                                tile_position=(ro, 0),
                            )
                            h1 = h1p.tile(
                                [128, HF], f16, tag="h1", name=f"h1_{t}_{lyr}"
                            )
                            # pe2: the two h1 drains feeding one L2 pair go to
                            # different engines so they finish ~together and
                            # the col-tiled L2 matmuls co-issue (co-insert).
                            fe = (
                                1 - prev_e
                                if ("pe2" in flags and i % 2 == 1)
                                else None
                            )
                            prev_e = drain(
                                h1[:], ps[:], b1t[:, lyr : lyr + 1], force=fe
                            )
                            h1_tiles.append(h1)
                        if len(lyrs) == 3:
                            h1_tiles.append(h1_tiles[2])
                    if skw:
                        new_l3 = skw_emit_l2(pend_l2)  # L2 of group g-1
                        del pend_l2[:]
                        skw_emit_l3(pend_l3)           # L3 of group g-2
                        del pend_l3[:]
                        pend_l3.extend(new_l3)
                        for j in range(2):
                            q = 2 * g + j
                            pend_l2.append(
                                (t, q, h1_tiles[2 * j], h1_tiles[2 * j + 1])
                            )
                        continue
                    for j in range(2):
                        q = 2 * g + j
                        ha = h1_tiles[2 * j]
                        hb = h1_tiles[2 * j + 1]
                        ps2 = l2ps.tile(
                            [128, HF], f32, tag="l2", name=f"ps2_{t}_{q}"
                        )
                        nc.tensor.matmul(
                            ps2[0:64, :],
                            lhsT=w2sl(q, 0, 64),
                            rhs=ha[:],
                            start=True,
                            stop=True,
                            tile_position=(0, 0),
                        )
                        if not ("p15" in flags and q == 15):
                            nc.tensor.matmul(
                                ps2[64:128, :],
                                lhsT=w2sl(q, 64, 128),
                                rhs=hb[:],
                                start=True,
                                stop=True,
                                tile_position=(0, 64),
                            )
                        h2 = h2p.tile(
                            [128, HF], f16, tag="h2", name=f"h2_{t}_{q}"
                        )
                        drain(h2[:], ps2[:], b2t[:, q : q + 1])
                        h2_tiles.append(h2)
                        if "l3i" in flags:
                            po = 64 * (t % 2)
                            if "l3cs" in flags:
                                # two M=32 col-tiled chains sharing one rhs:
                                # the insert column broadcasts across both
                                # col-groups, ~halving L3 PE time.
                                for cs in range(2):
                                    nc.tensor.matmul(
                                        ps3s[po + 32 * cs : po + 32 * (cs + 1), :],
                                        lhsT=w3t[
                                            :, 64 * q + 32 * cs : 64 * q + 32 * (cs + 1)
                                        ],
                                        rhs=h2[:],
                                        start=(q == 0),
                                        stop=(q == 15),
                                        tile_position=(0, po + 32 * cs),
                                    )
                            else:
                                nc.tensor.matmul(
                                    ps3s[po : po + 64, :],
                                    lhsT=w3t[:, 64 * q : 64 * (q + 1)],
                                    rhs=h2[:],
                                    start=(q == 0),
                                    stop=(q == 15),
                                    tile_position=(0, po),
                                )
                if "l3s" in flags and not skw:
                    po = 64 * (t % 2)
                    if "l3i" not in flags:
                        for q in range(16):
                            nc.tensor.matmul(
                                ps3s[po : po + 64, :],
                                lhsT=w3t[:, 64 * q : 64 * (q + 1)],
                                rhs=h2_tiles[q][:],
                                start=(q == 0),
                                stop=(q == 15),
                                tile_position=(0, po),
                            )
                    if "ose" in flags:
                        osb = outp.tile([64, HF], f32, tag="o", name=f"osb_{t}")
                        drain(
                            osb[:], ps3s[po : po + 64, :],
                            b3t[po : po + 64, 0:1], relu=False,
                        )
                        nc.sync.dma_start(
                            out_d[:, HF * t : HF * (t + 1)], osb[:]
                        )
                    elif t % 2 == 1:
                        osb = outp.tile([128, HF], f32, tag="o", name=f"osb_{t}")
                        drain(osb[:], ps3s[:], b3t[:, 0:1], relu=False)
                        nc.sync.dma_start(
                            out_d[:, HF * (t - 1) : HF * t], osb[0:64, :]
                        )
                        # final tile: second store on the scalar HWDGE ring so
                        # the two ~2us completion receipts overlap at the tail
                        # (earlier tiles stay off the ACT queue - a DMA trigger
                        # there costs ~650ns of drain-critical ACT time).
                        eng = nc.scalar if t == NT - 1 else nc.sync
                        eng.dma_start(
                            out_d[:, HF * t : HF * (t + 1)], osb[64:128, :]
                        )
                elif not skw:
                    ps3 = l3ps.tile([64, HF], f32, tag="l3", name=f"ps3_{t}")
                    for q in range(16):
                        nc.tensor.matmul(
                            ps3[:],
                            lhsT=w3t[:, 64 * q : 64 * (q + 1)],
                            rhs=h2_tiles[q][:],
                            start=(q == 0),
                            stop=(q == 15),
                        )
                    osb = outp.tile([64, HF], f32, tag="o", name=f"osb_{t}")
                    drain(osb[:], ps3[:], b3t[0:64, 0:1], relu=False)
                    nc.sync.dma_start(out_d[:, HF * t : HF * (t + 1)], osb[:])
            if skw:
                while pend_l2 or pend_l3:
                    new_l3 = skw_emit_l2(pend_l2)
                    del pend_l2[:]
                    skw_emit_l3(pend_l3)
                    del pend_l3[:]
                    pend_l3.extend(new_l3)

    if DEDUP_LDW:
        n = _dedup_ldweights(nc)
        print(f"dedup_ldweights removed {n}")
    if "sem1" in flags:
        # event semaphores only exist once compile() creates them; run the
        # merge pass right after the second generate_event_semaphores call
        # (sems final, still pre-ISA-codegen)
        orig_ges = nc.generate_event_semaphores
        ges_calls = [0]

        def _ges_wrapped():
            orig_ges()
            ges_calls[0] += 1
            if ges_calls[0] == 2:
                n = _merge_redundant_self_waits(nc)
                print(f"merge_redundant_self_waits: {n}")

        nc.generate_event_semaphores = _ges_wrapped
    nc.finalize()
    return nc


def _build_nc_m2():
    """Layer-paired L1 build: layers 2k/2k+1 share one [128,1024] 2-bank PSUM
    tile (column halves), b1 folded into the K=32 matmul via the ones row
    already present in zT4/w1s, so ONE relu-only drain covers both layers.
    Halves the dominant L1 drain-op count (31 -> 15 big + 1 single per batch
    tile) while keeping a fine-grained slot rotation (2x2-bank L1A + 1-bank
    L1B + 2 L2 + 1 shared L3 = 8 banks) so the PE never idles long enough to
    drop the HAM clock. Side effect: an h1 pair lands in one SBUF tile, so
    the two col-tiled L2 matmuls of pair q become ready together and co-issue.
    """
    import concourse.mybir as mybir
    from concourse import bacc, tile

    flags = set(VARIANT.split("-"))

    f32 = mybir.dt.float32
    f16 = mybir.dt.float16
    ADD = mybir.AluOpType.add
    MAX = mybir.AluOpType.max
    RELU = mybir.ActivationFunctionType.Relu

    nc = bacc.Bacc("TRN2", target_bir_lowering=False, debug=False)

    zT4_d = nc.declare_dram_parameter("zT4", [128, BC], f16, isOutput=False)
    w1_d = nc.declare_dram_parameter("w1s", [128, 16 * 128], f16, isOutput=False)
    w2_d = nc.declare_dram_parameter("w2s", [128, 16 * 128], f16, isOutput=False)
    w3_d = nc.declare_dram_parameter("w3s", [128, 16 * 64], f16, isOutput=False)
    b1_d = nc.declare_dram_parameter("b1s", [128, L], f32, isOutput=False)
    b2_d = nc.declare_dram_parameter("b2s", [128, 16], f32, isOutput=False)
    b3_d = nc.declare_dram_parameter("b3s", [128, 1], f32, isOutput=False)
    out_d = nc.declare_dram_parameter("out", [64, BC], f32, isOutput=True)

    # fd-aware greedy DVE/ACT drain balance (decode+access overhead + stream)
    eng_time = [0.0, 0.0]

    def dve_cost(fd):
        return 170.0 + fd / 0.96

    def act_cost(fd):
        return 175.0 + fd / 1.2

    HF = FD // 2

    with tile.TileContext(nc) as tc:
        with (
            tc.tile_pool(name="const", bufs=1) as const,
            tc.tile_pool(name="l1pA", bufs=2, space="PSUM") as l1pA,
            tc.tile_pool(name="l1pB", bufs=1, space="PSUM") as l1pB,
            tc.tile_pool(name="l2ps", bufs=2, space="PSUM") as l2ps,
            tc.tile_pool(name="l3ps", bufs=1, space="PSUM") as l3ps,
            tc.tile_pool(name="h1p", bufs=8) as h1p,
            tc.tile_pool(name="h1sp", bufs=2) as h1sp,
            tc.tile_pool(name="h2p", bufs=10) as h2p,
            tc.tile_pool(name="outp", bufs=4) as outp,
        ):
            # ACT spline-table prewarm overlapping input DMA
            warm = const.tile([128, 1], f32, name="warm")
            nc.vector.memset(warm[:], 0.0)
            warm2 = const.tile([128, 1], f32, name="warm2")
            nc.scalar.activation(warm2[:], warm[:], RELU, bias=0.0)

            zslices = []
            zt0 = const.tile([128, HF], f16, name="zt_s0")
            nc.sync.dma_start(zt0[:], zT4_d[:, 0:HF])
            zslices.append(zt0)
            # boot DMAs spread over the three DGE paths in first-use order
            w1ta = const.tile([128, 2 * 128], f16, name="w1ta")
            nc.scalar.dma_start(w1ta[:], w1_d[:, 0 : 2 * 128])

            def w1sl(g, r0, r1):
                if g < 2:
                    return w1ta[r0:r1, 128 * g : 128 * (g + 1)]
                return w1tb[r0:r1, 128 * (g - 2) : 128 * (g - 1)]

            b2t = const.tile([128, 16], f32, name="b2t")
            nc.gpsimd.dma_start(b2t[:], b2_d[:, :])
            w2ta = const.tile([128, 4 * 128], f16, name="w2ta")
            nc.scalar.dma_start(w2ta[:], w2_d[:, 0 : 4 * 128])
            w1tb = const.tile([128, 6 * 128], f16, name="w1tb")
            nc.scalar.dma_start(w1tb[:], w1_d[:, 2 * 128 : 8 * 128])
            w2tb = const.tile([128, 12 * 128], f16, name="w2tb")
            nc.scalar.dma_start(w2tb[:], w2_d[:, 4 * 128 : 16 * 128])

            def w2sl(q, c0, c1):
                if q < 4:
                    return w2ta[:, 128 * q + c0 : 128 * q + c1]
                return w2tb[:, 128 * (q - 4) + c0 : 128 * (q - 4) + c1]

            w3t = const.tile([128, 16 * 64], f16, name="w3t")
            nc.sync.dma_start(w3t[:], w3_d[:, :])
            b3t = const.tile([128, 1], f32, name="b3t")
            nc.gpsimd.dma_start(b3t[:], b3_d[:, :])
            for ti in range(1, NT):
                zti = const.tile([128, HF], f16, name=f"zt_s{ti}")
                nc.sync.dma_start(zti[:], zT4_d[:, HF * ti : HF * (ti + 1)])
                zslices.append(zti)

            def drain(dst, src, bias_ap, relu=True):
                fd = src.shape[-1]
                use_dve = eng_time[0] + dve_cost(fd) <= eng_time[1] + act_cost(fd)
                if use_dve:
                    eng_time[0] += dve_cost(fd)
                    if relu and bias_ap is None:
                        nc.vector.tensor_scalar(dst, src, 0.0, None, MAX)
                    elif relu:
                        nc.vector.tensor_scalar(dst, src, bias_ap, 0.0, ADD, MAX)
                    else:
                        nc.vector.tensor_scalar(dst, src, bias_ap, None, ADD)
                else:
                    eng_time[1] += act_cost(fd)
                    fn = RELU if relu else mybir.ActivationFunctionType.Identity
                    nc.scalar.activation(
                        dst, src, fn, bias=0.0 if bias_ap is None else bias_ap
                    )

            ps3s = None
            for t in range(NT):
                zt = zslices[t]
                po = 64 * (t % 2)
                if t % 2 == 0:
                    ps3s = l3ps.tile([128, HF], f32, tag="l3", name=f"ps3s_{t}")
                for q in range(16):
                    if q < 15:
                        a, b = 2 * q, 2 * q + 1
                        g = q // 2
                        roa = 32 * (a % 4)
                        rob = 32 * (b % 4)
                        ps = l1pA.tile(
                            [128, 2 * HF], f32, tag="l1", name=f"ps1_{t}_{q}"
                        )
                        # K=32 incl. ones row -> b1 folded; different row
                        # bands -> the two mms co-issue back-to-back
                        nc.tensor.matmul(
                            ps[:, 0:HF],
                            lhsT=w1sl(g, roa, roa + 32),
                            rhs=zt[roa : roa + 32, :],
                            start=True,
                            stop=True,
                            tile_position=(roa, 0),
                        )
                        nc.tensor.matmul(
                            ps[:, HF : 2 * HF],
                            lhsT=w1sl(g, rob, rob + 32),
                            rhs=zt[rob : rob + 32, :],
                            start=True,
                            stop=True,
                            tile_position=(rob, 0),
                        )
                        h1 = h1p.tile(
                            [128, 2 * HF], f16, tag="h1", name=f"h1_{t}_{q}"
                        )
                        drain(h1[:], ps[:], None)
                        ha = h1[:, 0:HF]
                        hb = h1[:, HF : 2 * HF]
                    else:
                        # layer 30 single via the 1-bank B slot; q=15 pair
                        # duplicates it (w3s zero-masks the dup's output)
                        ps = l1pB.tile([128, HF], f32, tag="l1b", name=f"ps1b_{t}")
                        nc.tensor.matmul(
                            ps[:],
                            lhsT=w1sl(7, 64, 96),
                            rhs=zt[64:96, :],
                            start=True,
                            stop=True,
                            tile_position=(64, 0),
                        )
                        h1 = h1sp.tile([128, HF], f16, tag="h1b", name=f"h1b_{t}")
                        drain(h1[:], ps[:], None)
                        ha = h1[:]
                        hb = h1[:]
                    ps2 = l2ps.tile([128, HF], f32, tag="l2", name=f"ps2_{t}_{q}")
                    nc.tensor.matmul(
                        ps2[0:64, :],
                        lhsT=w2sl(q, 0, 64),
                        rhs=ha,
                        start=True,
                        stop=True,
                        tile_position=(0, 0),
                    )
                    nc.tensor.matmul(
                        ps2[64:128, :],
                        lhsT=w2sl(q, 64, 128),
                        rhs=hb,
                        start=True,
                        stop=True,
                        tile_position=(0, 64),
                    )
                    h2 = h2p.tile([128, HF], f16, tag="h2", name=f"h2_{t}_{q}")
                    drain(h2[:], ps2[:], b2t[:, q : q + 1])
                    nc.tensor.matmul(
                        ps3s[po : po + 64, :],
                        lhsT=w3t[:, 64 * q : 64 * (q + 1)],
                        rhs=h2[:],
                        start=(q == 0),
                        stop=(q == 15),
                        tile_position=(0, po),
                    )
                if t % 2 == 1:
                    osb = outp.tile([128, HF], f32, tag="o", name=f"osb_{t}")
                    drain(osb[:], ps3s[:], b3t[:, 0:1], relu=False)
                    nc.sync.dma_start(
                        out_d[:, HF * (t - 1) : HF * t], osb[0:64, :]
                    )
                    eng = nc.scalar if t == NT - 1 else nc.sync
                    eng.dma_start(
                        out_d[:, HF * t : HF * (t + 1)], osb[64:128, :]
                    )

    nc.finalize()
    return nc


def _build_nc_m3():
    """m2 drains + 2-step software-pipelined emission.

    Flat stream over global steps s = 16*t + k (k = layer-pair index). At
    step s we emit: L1 fills for step s, the h1 drain for step s-1, and the
    L2 fills + h2 drain + inline L3 matmul for step s-2. Every PE
    instruction's dependencies were produced >=1 full step (~1.9us) earlier,
    so the in-order PE queue never head-of-line blocks on a drain, PE idle
    stays in thin slices (HAM keeps K=8/8), and each drain engine always has
    a ready op (one [128,1024] h1 drain + one [128,512] h2 drain per step).
    """
    import concourse.mybir as mybir
    from concourse import bacc, tile

    flags = set(VARIANT.split("-"))

    f32 = mybir.dt.float32
    f16 = mybir.dt.float16
    ADD = mybir.AluOpType.add
    MAX = mybir.AluOpType.max
    RELU = mybir.ActivationFunctionType.Relu

    nc = bacc.Bacc("TRN2", target_bir_lowering=False, debug=False)

    zT4_d = nc.declare_dram_parameter("zT4", [128, BC], f16, isOutput=False)
    w1_d = nc.declare_dram_parameter("w1s", [128, 16 * 128], f16, isOutput=False)
    w2_d = nc.declare_dram_parameter("w2s", [128, 16 * 128], f16, isOutput=False)
    w3_d = nc.declare_dram_parameter("w3s", [128, 16 * 64], f16, isOutput=False)
    b1_d = nc.declare_dram_parameter("b1s", [128, L], f32, isOutput=False)
    b2_d = nc.declare_dram_parameter("b2s", [128, 16], f32, isOutput=False)
    b3_d = nc.declare_dram_parameter("b3s", [128, 1], f32, isOutput=False)
    out_d = nc.declare_dram_parameter("out", [64, BC], f32, isOutput=True)

    eng_time = [0.0, 0.0]

    def dve_cost(fd):
        return 170.0 + fd / 0.96

    def act_cost(fd):
        return 175.0 + fd / 1.2

    HF = FD // 2

    with tile.TileContext(nc) as tc:
        with (
            tc.tile_pool(name="const", bufs=1) as const,
            tc.tile_pool(name="l1pA", bufs=2, space="PSUM") as l1pA,
            tc.tile_pool(name="l2ps", bufs=3, space="PSUM") as l2ps,
            tc.tile_pool(name="l3ps", bufs=1, space="PSUM") as l3ps,
            tc.tile_pool(name="h1p", bufs=8) as h1p,
            tc.tile_pool(name="h1sp", bufs=2) as h1sp,
            tc.tile_pool(name="h2p", bufs=10) as h2p,
            tc.tile_pool(name="outp", bufs=4) as outp,
        ):
            warm = const.tile([128, 1], f32, name="warm")
            nc.vector.memset(warm[:], 0.0)
            warm2 = const.tile([128, 1], f32, name="warm2")
            nc.scalar.activation(warm2[:], warm[:], RELU, bias=0.0)

            zslices = []
            zt0 = const.tile([128, HF], f16, name="zt_s0")
            nc.sync.dma_start(zt0[:], zT4_d[:, 0:HF])
            zslices.append(zt0)
            w1ta = const.tile([128, 2 * 128], f16, name="w1ta")
            nc.scalar.dma_start(w1ta[:], w1_d[:, 0 : 2 * 128])

            def w1sl(g, r0, r1):
                if g < 2:
                    return w1ta[r0:r1, 128 * g : 128 * (g + 1)]
                return w1tb[r0:r1, 128 * (g - 2) : 128 * (g - 1)]

            b2t = const.tile([128, 16], f32, name="b2t")
            nc.gpsimd.dma_start(b2t[:], b2_d[:, :])
            w2ta = const.tile([128, 4 * 128], f16, name="w2ta")
            nc.scalar.dma_start(w2ta[:], w2_d[:, 0 : 4 * 128])
            w1tb = const.tile([128, 6 * 128], f16, name="w1tb")
            nc.scalar.dma_start(w1tb[:], w1_d[:, 2 * 128 : 8 * 128])
            w2tb = const.tile([128, 12 * 128], f16, name="w2tb")
            nc.scalar.dma_start(w2tb[:], w2_d[:, 4 * 128 : 16 * 128])

            def w2sl(q, c0, c1):
                if q < 4:
                    return w2ta[:, 128 * q + c0 : 128 * q + c1]
                return w2tb[:, 128 * (q - 4) + c0 : 128 * (q - 4) + c1]

            w3t = const.tile([128, 16 * 64], f16, name="w3t")
            nc.sync.dma_start(w3t[:], w3_d[:, :])
            b3t = const.tile([128, 1], f32, name="b3t")
            nc.gpsimd.dma_start(b3t[:], b3_d[:, :])
            for ti in range(1, NT):
                zti = const.tile([128, HF], f16, name=f"zt_s{ti}")
                nc.sync.dma_start(zti[:], zT4_d[:, HF * ti : HF * (ti + 1)])
                zslices.append(zti)

            def drain(dst, src, bias_ap, relu=True, force=None):
                fd = src.shape[-1]
                if force is not None:
                    use_dve = force == 0
                else:
                    use_dve = (
                        eng_time[0] + dve_cost(fd) <= eng_time[1] + act_cost(fd)
                    )
                if use_dve:
                    eng_time[0] += dve_cost(fd)
                    if relu and bias_ap is None:
                        nc.vector.tensor_scalar(dst, src, 0.0, None, MAX)
                    elif relu:
                        nc.vector.tensor_scalar(dst, src, bias_ap, 0.0, ADD, MAX)
                    else:
                        nc.vector.tensor_scalar(dst, src, bias_ap, None, ADD)
                else:
                    eng_time[1] += act_cost(fd)
                    fn = RELU if relu else mybir.ActivationFunctionType.Identity
                    nc.scalar.activation(
                        dst, src, fn, bias=0.0 if bias_ap is None else bias_ap
                    )

            S = NT * 16
            BOOT_STEPS = 32  # first 2 tiles: PE-saturating redundant fills
            # + baseline-like single-bank drains to win the HAM K=8/8 grant
            pend = {}  # step -> (ps_tile, h1_tile or None yet, t, k)
            ps3s = None

            def emit_l1_fills(s):
                t, k = divmod(s, 16)
                zt = zslices[t]
                # during boot, emit each fill twice (redundant recompute,
                # same-band so strictly serial): keeps the PE 100% saturated
                # at K=4/8 so the HAM monitor grants K=8/8 early and
                # deterministically, like the baseline's PE-bound boot
                rep = 2 if s < 16 else 1
                if k < 15:
                    a = 2 * k
                    g = k // 2
                    roa = 32 * (a % 4)
                    rob = roa + 32
                    ps = l1pA.tile([128, 2 * HF], f32, tag="l1", name=f"ps1_{t}_{k}")
                    for _ in range(rep):
                        nc.tensor.matmul(
                            ps[:, 0:HF],
                            lhsT=w1sl(g, roa, roa + 32),
                            rhs=zt[roa : roa + 32, :],
                            start=True,
                            stop=True,
                            tile_position=(roa, 0),
                        )
                    for _ in range(rep):
                        nc.tensor.matmul(
                            ps[:, HF : 2 * HF],
                            lhsT=w1sl(g, rob, rob + 32),
                            rhs=zt[rob : rob + 32, :],
                            start=True,
                            stop=True,
                            tile_position=(rob, 0),
                        )
                else:
                    ps = l2ps.tile([128, HF], f32, tag="l2", name=f"ps1b_{t}")
                    nc.tensor.matmul(
                        ps[:],
                        lhsT=w1sl(7, 64, 96),
                        rhs=zt[64:96, :],
                        start=True,
                        stop=True,
                        tile_position=(64, 0),
                    )
                pend[s] = [ps, None, t, k]

            def emit_h1_drain(s):
                ent = pend[s]
                ps, _, t, k = ent
                f1 = s % 2
                if k < 15:
                    h1 = h1p.tile([128, 2 * HF], f16, tag="h1", name=f"h1_{t}_{k}")
                    if s < BOOT_STEPS:
                        drain(h1[:, 0:HF], ps[:, 0:HF], None, force=s % 2)
                        drain(
                            h1[:, HF : 2 * HF],
                            ps[:, HF : 2 * HF],
                            None,
                            force=(s + 1) % 2,
                        )
                    else:
                        drain(h1[:], ps[:], None, force=f1)
                else:
                    h1 = h1sp.tile([128, HF], f16, tag="h1b", name=f"h1b_{t}")
                    drain(h1[:], ps[:], None, force=f1)
                ent[1] = h1

            h2pend = {}  # step -> (h2_tile, t, q)

            def emit_l2(s):
                ps, h1, t, k = pend.pop(s)
                q = k
                if k < 15:
                    ha = h1[:, 0:HF]
                    hb = h1[:, HF : 2 * HF]
                else:
                    ha = h1[:]
                    hb = h1[:]
                ps2 = l2ps.tile([128, HF], f32, tag="l2", name=f"ps2_{t}_{q}")
                rep = 2 if s < 16 else 1
                for _ in range(rep):
                    nc.tensor.matmul(
                        ps2[0:64, :],
                        lhsT=w2sl(q, 0, 64),
                        rhs=ha,
                        start=True,
                        stop=True,
                        tile_position=(0, 0),
                    )
                for _ in range(rep):
                    nc.tensor.matmul(
                        ps2[64:128, :],
                        lhsT=w2sl(q, 64, 128),
                        rhs=hb,
                        start=True,
                        stop=True,
                        tile_position=(0, 64),
                    )
                h2 = h2p.tile([128, HF], f16, tag="h2", name=f"h2_{t}_{q}")
                drain(h2[:], ps2[:], b2t[:, q : q + 1], force=(s + 1) % 2)
                h2pend[s] = (h2, t, q)

            def emit_l3(s):
                nonlocal ps3s
                h2, t, q = h2pend.pop(s)
                po = 64 * (t % 2)
                if q == 0 and t % 2 == 0:
                    ps3s = l3ps.tile([128, HF], f32, tag="l3", name=f"ps3s_{t}")
                nc.tensor.matmul(
                    ps3s[po : po + 64, :],
                    lhsT=w3t[:, 64 * q : 64 * (q + 1)],
                    rhs=h2[:],
                    start=(q == 0),
                    stop=(q == 15),
                    tile_position=(0, po),
                )
                if q == 15 and t % 2 == 1:
                    osb = outp.tile([128, HF], f32, tag="o", name=f"osb_{t}")
                    drain(osb[:], ps3s[:], b3t[:, 0:1], relu=False)
                    nc.sync.dma_start(
                        out_d[:, HF * (t - 1) : HF * t], osb[0:64, :]
                    )
                    eng = nc.scalar if t == NT - 1 else nc.sync
                    eng.dma_start(
                        out_d[:, HF * t : HF * (t + 1)], osb[64:128, :]
                    )

            # Per-step emission order tuned for the in-order PE queue: L2
            # fills first (deps resolved 2 steps ago - never block), L1
            # fills second (slot waits overlap L2 streams), L3 last and 5
            # steps deep so its h2 is long-drained. Drains interleave on
            # the engine side in the same cadence.
            for s in range(S + 5):
                if 1 <= s <= S:
                    emit_h1_drain(s - 1)
                if 2 <= s <= S + 1:
                    emit_l2(s - 2)
                if s < S:
                    emit_l1_fills(s)
                if 5 <= s <= S + 4:
                    emit_l3(s - 5)

    nc.finalize()
    return nc


def _build_nc_d2():
    """Pair-granular build: L1 layers 2q/2q+1 share one 2-bank PSUM tile.

    Per batch tile of 512 and pair q: two row-tiled K=32 L1 matmuls (bias via
    ones row) -> one [128,1024] relu drain -> two col-tiled L2 matmuls ->
    [128,512] bias+relu drain -> inline L3 accumulation matmul. The paired
    slot-free events let consecutive PE matmuls co-issue (tile_position
    concurrency), and halving the drain op count trims DVE/ACT overhead.
    """
    import concourse.mybir as mybir
    from concourse import bacc, tile

    flags = set(VARIANT.split("-"))

    f32 = mybir.dt.float32
    f16 = mybir.dt.float16
    ADD = mybir.AluOpType.add
    MAX = mybir.AluOpType.max
    RELU = mybir.ActivationFunctionType.Relu

    nc = bacc.Bacc("TRN2", target_bir_lowering=False, debug=False)

    zT4_d = nc.declare_dram_parameter("zT4", [128, BC], f16, isOutput=False)
    w1_d = nc.declare_dram_parameter("w1s", [128, 16 * 128], f16, isOutput=False)
    w2_d = nc.declare_dram_parameter("w2s", [128, 16 * 128], f16, isOutput=False)
    w3_d = nc.declare_dram_parameter("w3s", [128, 16 * 64], f16, isOutput=False)
    b1_d = nc.declare_dram_parameter("b1s", [128, L], f32, isOutput=False)
    b2_d = nc.declare_dram_parameter("b2s", [128, 16], f32, isOutput=False)
    b3_d = nc.declare_dram_parameter("b3s", [128, 1], f32, isOutput=False)
    out_d = nc.declare_dram_parameter("out", [64, BC], f32, isOutput=True)

    HF = FD // 2

    # greedy DVE/ACT drain balance, fd-aware effective ns per op
    eng_time = [0.0, 0.0]
    PAD = 80.0 if "pad" in flags else 0.0

    def dve_cost(fd):
        return (120.0 + fd) / 0.96 + PAD

    def act_cost(fd):
        return (172.0 + fd) / 1.2 + PAD

    with tile.TileContext(nc) as tc:
        with (
            tc.tile_pool(name="const", bufs=1) as const,
            tc.tile_pool(
                name="l1ps", bufs=(3 if "l1b3" in flags else 2), space="PSUM"
            ) as l1ps,
            tc.tile_pool(
                name="l2ps", bufs=(2 if "l1b3" in flags else 3), space="PSUM"
            ) as l2ps,
            tc.tile_pool(name="l3ps", bufs=1, space="PSUM") as l3ps,
            tc.tile_pool(name="h1p", bufs=10) as h1p,
            tc.tile_pool(name="h2p", bufs=20) as h2p,
            tc.tile_pool(name="outp", bufs=4) as outp,
        ):
            # ACT spline-table prewarm: dummy relu so the one-time
            # ACT_TABLE_LOAD overlaps the input DMA instead of delaying the
            # first real drain.
            warm = const.tile([128, 1], f32, name="warm")
            nc.vector.memset(warm[:], 0.0)
            warm2 = const.tile([128, 1], f32, name="warm2")
            nc.scalar.activation(warm2[:], warm[:], RELU, bias=0.0)

            zslices = []
            zt0 = const.tile([128, HF], f16, name="zt_s0")
            nc.sync.dma_start(zt0[:], zT4_d[:, 0:HF])
            zslices.append(zt0)
            w1t = const.tile([128, 16 * 128], f16, name="w1t")
            nc.sync.dma_start(w1t[:], w1_d[:, :])
            w2t = const.tile([128, 16 * 128], f16, name="w2t")
            nc.sync.dma_start(w2t[:], w2_d[:, :])
            w3t = const.tile([128, 16 * 64], f16, name="w3t")
            nc.sync.dma_start(w3t[:], w3_d[:, :])
            b2t = const.tile([128, 16], f32, name="b2t")
            nc.sync.dma_start(b2t[:], b2_d[:, :])
            b3t = const.tile([128, 1], f32, name="b3t")
            nc.sync.dma_start(b3t[:], b3_d[:, :])
            for ti in range(1, NT):
                zti = const.tile([128, HF], f16, name=f"zt_s{ti}")
                nc.sync.dma_start(zti[:], zT4_d[:, HF * ti : HF * (ti + 1)])
                zslices.append(zti)

            def drain(dst, src, bias_ap, relu=True):
                fd = src.shape[-1]
                use_dve = eng_time[0] + dve_cost(fd) <= eng_time[1] + act_cost(fd)
                if use_dve:
                    eng_time[0] += dve_cost(fd)
                    if relu and bias_ap is None:
                        nc.vector.tensor_scalar(dst, src, 0.0, None, MAX)
                    elif relu:
                        nc.vector.tensor_scalar(dst, src, bias_ap, 0.0, ADD, MAX)
                    else:
                        nc.vector.tensor_scalar(dst, src, bias_ap, None, ADD)
                else:
                    eng_time[1] += act_cost(fd)
                    fn = RELU if relu else mybir.ActivationFunctionType.Identity
                    nc.scalar.activation(
                        dst, src, fn, bias=0.0 if bias_ap is None else bias_ap
                    )

            ps3s = None
            for t in range(NT):
                zt = zslices[t]
                po = 64 * (t % 2)
                if t % 2 == 0:
                    ps3s = l3ps.tile([128, HF], f32, tag="l3", name=f"ps3s_{t}")
                for q in range(16):
                    ps = l1ps.tile([128, 2 * HF], f32, tag="l1", name=f"ps1_{t}_{q}")
                    for k in range(2):
                        lyr = 2 * q + k  # lyr 31 hits the zero band of w1s
                        gg, ii = divmod(lyr, 4)
                        ro = 32 * ii
                        nc.tensor.matmul(
                            ps[:, HF * k : HF * (k + 1)],
                            lhsT=w1t[ro : ro + 32, 128 * gg : 128 * (gg + 1)],
                            rhs=zt[ro : ro + 32, :],
                            start=True,
                            stop=True,
                            tile_position=(ro, 0),
                        )
                    h1 = h1p.tile([128, 2 * HF], f16, tag="h1", name=f"h1_{t}_{q}")
                    drain(h1[:], ps[:], None)
                    ps2 = l2ps.tile([128, HF], f32, tag="l2", name=f"ps2_{t}_{q}")
                    nc.tensor.matmul(
                        ps2[0:64, :],
                        lhsT=w2sl(q, 0, 64),
                        rhs=h1[:, 0:HF],
                        start=True,
                        stop=True,
                        tile_position=(0, 0),
                    )
                    nc.tensor.matmul(
                        ps2[64:128, :],
                        lhsT=w2sl(q, 64, 128),
                        rhs=h1[:, HF : 2 * HF],
                        start=True,
                        stop=True,
                        tile_position=(0, 64),
                    )
                    h2 = h2p.tile([128, HF], f16, tag="h2", name=f"h2_{t}_{q}")
                    drain(h2[:], ps2[:], b2t[:, q : q + 1])
                    nc.tensor.matmul(
                        ps3s[po : po + 64, :],
                        lhsT=w3t[:, 64 * q : 64 * (q + 1)],
                        rhs=h2[:],
                        start=(q == 0),
                        stop=(q == 15),
                        tile_position=(0, po),
                    )
                if t % 2 == 1:
                    osb = outp.tile([128, HF], f32, tag="o", name=f"osb_{t}")
                    drain(osb[:], ps3s[:], b3t[:, 0:1], relu=False)
                    nc.sync.dma_start(out_d[:, HF * (t - 1) : HF * t], osb[0:64, :])
                    nc.sync.dma_start(
                        out_d[:, HF * t : HF * (t + 1)], osb[64:128, :]
                    )

    nc.finalize()
    return nc


def _get_nc():
    if VARIANT not in _NC_CACHE:
        if VARIANT.startswith("m3"):
            _NC_CACHE[VARIANT] = _build_nc_m3()
        elif VARIANT.startswith("m2"):
            _NC_CACHE[VARIANT] = _build_nc_m2()
        elif VARIANT.startswith("d2"):
            _NC_CACHE[VARIANT] = _build_nc_d2()
        else:
            _NC_CACHE[VARIANT] = _build_nc()
    return _NC_CACHE[VARIANT]


def _prep_shared(W1, b1, W2, b2, Wout, bout, bout0):
    W1 = np.asarray(W1, np.float32)
    b1 = np.asarray(b1, np.float32)
    W2 = np.asarray(W2, np.float32)
    b2 = np.asarray(b2, np.float32)
    Wout = np.asarray(Wout, np.float32)
    bout = np.asarray(bout, np.float32)
    bout0 = np.asarray(bout0, np.float32)

    mask = np.tril(np.ones((L, L), np.float32))
    W1m = W1 * mask[:, None, :]  # [31, 128, 31]

    w1s = np.zeros((128, 16 * 128), F16)
    for g in range(8):
        for i in range(4):
            lyr = 4 * g + i
            if lyr >= L:
                break
            w1s[32 * i : 32 * i + L, 128 * g : 128 * (g + 1)] = W1m[lyr].T.astype(
                F16
            )
            w1s[32 * i + L, 128 * g : 128 * (g + 1)] = b1[lyr].astype(F16)
    b1s = np.ascontiguousarray(b1.T)  # [128, 31]

    w2s = np.zeros((128, 16 * 128), F16)
    b2s = np.zeros((128, 16), np.float32)
    for q, (a, bb) in enumerate(PAIRS):
        w2s[:, 128 * q : 128 * q + 64] = W2[a].T.astype(F16)
        w2s[:, 128 * q + 64 : 128 * (q + 1)] = W2[bb].T.astype(F16)
        b2s[0:64, q] = b2[a]
        b2s[64:128, q] = b2[bb]

    w3s = np.zeros((128, 16 * 64), F16)
    for q, (a, bb) in enumerate(PAIRS):
        blk = np.zeros((128, 64), np.float32)
        blk[0:64, 1 + a] = Wout[a][0]
        blk[0:64, 33 + a] = Wout[a][1]
        if 2 * q + 1 <= L - 1:  # real second layer (not the dup)
            blk[64:128, 1 + bb] = Wout[bb][0]
            blk[64:128, 33 + bb] = Wout[bb][1]
        w3s[:, 64 * q : 64 * (q + 1)] = blk.astype(F16)

    b3h = np.zeros(64, np.float32)
    b3h[0] = bout0[0]
    b3h[1 : 1 + L] = bout[:, 0]
    b3h[32] = bout0[1]
    b3h[33 : 33 + L] = bout[:, 1]
    b3s = np.concatenate([b3h, b3h]).reshape(128, 1)  # both L3 psum halves

    return w1s, w2s, w3s, b1s, b2s, b3s


def kernel(z, W1, b1, W2, b2, Wout, bout, bout0):
    global LAST_RESULT
    from concourse.bass_utils import run_bass_kernel_spmd

    z = np.asarray(z, np.float32)
    w1s, w2s, w3s, b1s, b2s, b3s = _prep_shared(W1, b1, W2, b2, Wout, bout, bout0)

    zin_T = np.ascontiguousarray(z[:, :L].T).astype(F16)  # [31, 65536]
    in_maps = []
    for c in range(NCORES):
        sl = zin_T[:, BC * c : BC * (c + 1)]
        zt4 = np.zeros((128, BC), F16)
        for i in range(4):
            zt4[32 * i : 32 * i + L] = sl
            zt4[32 * i + L] = 1.0  # ones row (feeds the K=32 bias fold in m1)
        in_maps.append(
            {
                "zT4": zt4,
                "w1s": w1s,
                "w2s": w2s,
                "w3s": w3s,
                "b1s": b1s,
                "b2s": b2s,
                "b3s": b3s,
            }
        )

    nc = _get_nc()
    try:
        res = run_bass_kernel_spmd(nc, in_maps, core_ids=list(range(NCORES)))
    except Exception:
        # transient device-unrecoverable states clear on the next attempt
        res = run_bass_kernel_spmd(nc, in_maps, core_ids=list(range(NCORES)))
    LAST_RESULT = res

    big = np.concatenate([res.results[c]["out"] for c in range(NCORES)], axis=1)
    mus = np.ascontiguousarray(big[:32].T).astype(np.float32, copy=False)
    lvs = np.ascontiguousarray(big[32:].T).astype(np.float32, copy=False)
    return mus, lvs



# revision 33
# speedup vs baseline: 1.0331x; 1.0029x over previous
"""Trainium2 Bass kernel for nn_ARPrior (stacked causal-prior MLPs).

Network (per sample, latent D=32, L=31 stacked layers):
    zin = z[:, :31]
    h1[l] = relu(W1m[l] @ zin + b1[l])   # [128], W1m causally masked
    h2[l] = relu(W2[l]  @ h1[l] + b2[l]) # [64]
    out[l] = Wout[l] @ h2[l] + bout[l]   # [2]  (mu, logvar)
    mus = [bout0[0], out[:,0]]; lvs = [bout0[1], out[:,1]]

Mapping (pure data parallel, batch 65536 sharded 8 ways -> 8192/core):
  - L1: K=31 -> 4 layers packed in the 128x128 PE array via row tiling
    (tile_position=(32i,0)), each writing its own PSUM bank.
  - L2: K=128, M=64 -> 2 layers packed via column tiling
    (tile_position=(0,0)/(0,64)) into one PSUM bank.
  - L3: M=2 per layer -> all 31 layers' output weights embedded in a
    block-diagonal [128,64] stationary per layer-pair, accumulated over
    16 matmuls into a single [64,512] PSUM tile per batch tile. Output
    columns are pre-arranged [mu(32) | logvar(32)]; bias adds bout/bout0.
  - The PSUM drain (bias+relu+fp16 cast, one pass over every h1/h2
    element) is the throughput wall (~283us/core combined across both
    drain-capable engines); ops are split greedily between ScalarE
    (activation Relu w/ per-partition bias AP) and VectorE (tensor_scalar
    add+max). Fine-grained single-bank PSUM rotation (4 L1 + 2 L2 + 2 L3
    slots) keeps the TensorE free of long stalls so the HAM clock gate
    stays at 2.4 GHz; coarser multi-bank drain variants measured slower
    despite fewer drain ops. Default variant: 5 L1 + 2 L2 + 1 shared L3
    psum slots, inline L3 accumulation. Final tuning: ACT-table prewarm,
    boot DMAs spread over the three DGE paths (w1 split so a 64KB first
    chunk ungates the first matmul; the zero half of w1s never
    transferred), final-tile output stores on two rings, and the greedy
    DVE/ACT split recalibrated to measured busy-per-op (682/604 ns, flag
    c2), and w2 split the same way as w1 (pairs 0-3 first) to kill a
    ~1.4us both-engine stall at t~14.5us waiting on the 512KB w2
    transfer: 297.2us/core (297,155/297,187 ns across two runs), engines
    balanced within 1.4us. Forcing drain-engine placement (pe2 flag, off)
    measured 308us - the greedy balancer's freedom beats deterministic
    pairing.
  - Profiled balance (per core, ~306us wall before the trims): TensorE
    union-busy 257us (mms ~203ns effective; tile_position pairs co-issue
    ~20-30% of the time), DVE 242us, ACT 249us. All three engines ~80-92%
    busy - a sharp local optimum. Two structural variants measured WORSE: d2 (2-bank
    [128,1024] L1 psums, bias folded via ones row -> half the L1 drain
    ops) hit 513us because halving the PSUM slot count starves the PE
    (mm latency ~633ns = slot head-wait/cold); l3cs (L3 split into two
    M=32 col-tiled chains sharing one rhs) hit 424us from doubled L3
    mm/sem traffic inflating every op ~20%. On TRN2 the drain floor is
    hard: only DVE (0.96GHz) + ACT (1.2GHz) can read PSUM at 1
    elem/lane/cyc (fp32 PSUM is 1x on DVE; 16-bit PSUM matmul output that
    would enable 2x_1P reads is TRN3-only; DMA/GPSIMD have no PSUM port).
  - All compute in fp16 (1 PE cycle/row vs 2 for fp32), fp32 PSUM/bias,
    rel err ~5e-4.
  - skw (current default, 262-264.5us/core, 11% over the 297us non-skw
    build): software-pipelined emission. L2 work of group g-1 and L3 work
    of group g-2 are emitted after group g's L1 section (queues cross tile
    boundaries; osb/output DMAs emit when the q==15 L3 of an odd tile is
    processed; post-loop flush drains the queues). This removes the
    in-order head-of-line blocking that previously idled all three
    engines ~13%: with skw, every instruction's dependencies were
    produced a full group (~2us) earlier, so the drain engines run
    saturated (mid-run idle 0.6-1.3us vs ~35us before; DVE 247us busy,
    ACT 249.7us busy, PE 215.8us on a 264.5us wall). Remaining idle is
    boot (~5-8us DMA/DGE init + HAM grant at ~17-22us) and tail (~8us
    final flush + output DMA receipts); both measured resistant: inlining
    the last tile cost +2us, tail-splitting osb is drain-work-negative.
  - Measured dead ends (2026-08-09 session), do not revisit without new
    evidence: (1) m2/m3 layer-paired [128,1024] L1 psums + relu-only
    1024-col drains (b1 folded via the zT4 ones row, 2-step skewed flat
    pipeline, parity-pinned drain engines): the 1024-col drain saves only
    5-8% engine time (DVE 1214ns vs 2x682, ACT 1106 vs 2x604) and the
    reduced PE load (<80% duty) makes the HAM K=8/8 grant a boot-time
    race that some cores lose entirely (whole run at 1.2GHz, 403us) plus
    mid-run K=4/8 relapses; best stable m3 = 299-311us. The HAM grant
    does NOT follow from continuous PE busy at K=4/8 (18 back-to-back
    zero-data warmup matmuls never tripped it; saturating real-data boot
    moved it only 60->26us); the fine-grained non-skw baseline earns it
    at ~17us reliably, and skw keeps PE >80% so it sticks. (2) fp8
    DoubleRow (2x PE): e4m3 quantization is ~2.2% rms per tensor and
    error propagates through the linear layers undiminished -> blows the
    2e-2 gate (fp16 build is at 5e-4). (3) c3 recalibration of the
    greedy split to raw measured per-op times (742/735) regressed to
    276us - the raw averages embed queue effects; c2's 682/604 ratio
    (~1.13 = true DVE/ACT throughput ratio) assigns correctly. (4)
    period-8 drain-engine patterns (DVE 3xh1+5xh2 / ACT 5xh1+3xh2):
    per-step makespan lumpiness beats the better period-average - 325us.
  - Second-session iteration (same day): dmv flag (current default,
    262.7-263.6us max-core over three runs) moves the bulk w1tb/w2tb DMA
    triggers from the scalar (ACT) queue to the idle sync queue - their
    ~2us completion receipts were landing on ACT right as the first
    drains started. Additional measured dead ends: (5) sem1 post-pass
    (drop provably-satisfied self-engine waits on the strict-FIFO drain
    queues and merge the preceding standalone InstEventSemaphore's real
    wait into the drain, hooked after compile()'s second
    generate_event_semaphores): 144 merges, perf-neutral - standalone
    sem decode overlaps execution, so they were already free. Code kept
    (gated off). (6) pd flag (m2-style paired [128,1024] L1 drains inside
    the skw structure, boot tiles draining per-half): 261-271us with
    grant-race variance back (one core K=8/8 only at 63us) - merged
    drains reintroduce HAM fragility for ~zero drain-time gain; gated
    off. (7) l2b3 (4 L1 + 3 L2 banks): neutral (263.1).
  - Roofline context: 48.8M hidden elems/core must cross PSUM->SBUF on
    exactly two engines (DVE 0.96GHz + ACT 1.2GHz, 1 fp32 elem/lane/cyc;
    DMA/GPSIMD have no PSUM port, 16-bit PSUM is TRN3-only) -> ~178us
    zero-overhead drain floor, ~225-245us with real per-op overhead.
    At 264.5us wall with drains 94% busy, remaining headroom is ~15-20us
    (boot+tail+per-op overhead), not a structural factor.

Host does all weight masking/stacking/transposing; device output is
[64, 8192] f32 per core, host concatenates and transposes.
"""

import sys

if "/opt/trn_rl_repo" not in sys.path:
    sys.path.insert(0, "/opt/trn_rl_repo")

import numpy as np

B = 65536
D = 32
L = 31
NCORES = 8
BC = B // NCORES  # 8192 per-core batch
FD = 1024             # legacy constant; HF = FD//2 = 512 is the batch tile
NT = BC // (FD // 2)  # 16 batch tiles of 512 per core

# layer pairs for L2/L3; last pair duplicates layer 30 (its W3 block is zero)
PAIRS = [(2 * q, min(2 * q + 1, L - 1)) for q in range(16)]

F16 = np.float16

DEDUP_LDW = False  # delete redundant InstLdweights post-schedule
# Tuned via interleaved A/B on hardware: l3s = one shared L3 psum bank
# (partition halves of one [128,512] tile serve two batch tiles) freeing a
# 5th L1 psum slot; sbp = deeper SBUF pools; l3i = L3 accumulation matmuls
# emitted inline after each pair drain instead of as a tail chain.
# d2* variants: 2-bank [128,1024] L1 psum tiles (2 layers each, bias folded
# into the matmul via the zT4 ones row) so PSUM slots free in bursts of two
# and adjacent row-tiled L1 matmuls co-issue into the PE array. Measured
# 513us: halving the PSUM slot count starves the PE in ~1.2us bursts and the
# HAM clock gate drops it to 1.2 GHz (mm dur ~633ns = cold latency). The
# 8-way single-bank rotation is load-bearing; keep it.
# pw = ACT spline-table prewarm overlapping input DMA; l3cs = L3 emitted as
# two M=32 col-tiled accumulation chains sharing one rhs (broadcast
# co-insert) — measured 424us: doubling L3 mm/sem count inflates every op's
# effective latency ~20%; do not use.
VARIANT = "l3s-sbq-l3i-zs-pw-c2-skw-dmv"

_NC_CACHE = {}
LAST_RESULT = None  # BassKernelResults of the most recent run (for test.py)


def _dedup_ldweights(nc):
    """Remove LDWEIGHTS that reload the exact weights already resident in the
    same PE-array region. Runs after Tile scheduling (instruction order and
    semaphores final) and before Bacc lowering. Conservative: any overlapping
    region load or tiling-mode change invalidates, and only sync-free
    duplicates are deleted.
    """
    import concourse.mybir as mybir

    PE = mybir.EngineType.PE
    removed = 0
    for bb in nc.m.functions[0].blocks:
        loaded = {}
        cur_mode = None
        todel = []
        for ins in bb.instructions:
            if getattr(ins, "engine", None) != PE:
                continue
            tn = type(ins).__name__
            if tn == "InstLdweights":
                tp = ins.tile_position or (0, 0)
                tsz = ins.tile_size or (128, 128)
                if tsz != cur_mode:
                    loaded.clear()
                    cur_mode = tsz
                region = (tp[0], tp[0] + tsz[0], tp[1], tp[1] + tsz[1])
                ap = ins.ins[0]
                sig = (
                    getattr(ap, "memref", None),
                    getattr(ap, "offset", None),
                    str(getattr(ap, "ap", None)),
                    str(getattr(ap, "dtype", None)),
                    tuple(tp),
                    tuple(tsz),
                )
                si = ins.sync_info
                clean = si is None or (not si.on_wait and not si.on_update)
                if loaded.get(region) == sig and clean:
                    todel.append(ins)
                    removed += 1
                    continue
                for rk in list(loaded):
                    if not (
                        rk[1] <= region[0]
                        or region[1] <= rk[0]
                        or rk[3] <= region[2]
                        or region[3] <= rk[2]
                    ):
                        del loaded[rk]
                loaded[region] = sig
            elif tn == "InstMatmult":
                tsz = ins.tile_size or (128, 128)
                if tuple(tsz) != (cur_mode and tuple(cur_mode)):
                    if tsz != cur_mode:
                        loaded.clear()
                        cur_mode = tsz
        for ins in todel:
            bb.instructions.remove(ins)
            nc.inst_map.pop(ins.name, None)
    return removed


def _merge_redundant_self_waits(nc):
    """Post-scheduling pass: on the strict-FIFO drain engines (ACT/DVE), a
    wait on the engine's OWN completion semaphore whose threshold is <= the
    number of increments already emitted earlier in the same queue is
    trivially satisfied (serial in-order execution also subsumes every
    same-engine data hazard). Tile emits such self-waits carried on the
    drain op while pushing the REAL cross-engine wait (psum-fill) onto a
    standalone InstEventSemaphore right before it (~250 per engine, ~40-60ns
    each of saturated engine-queue time). Drop the redundant self-wait,
    migrate the standalone's wait onto the drain, delete the standalone.
    """
    import concourse.mybir as mybir

    ENGS = (mybir.EngineType.Activation, mybir.EngineType.DVE)
    changed = 0
    for fn in nc.m.functions:
        for bb in fn.blocks:
            for ENG in ENGS:
                q = [
                    i
                    for i in bb.instructions
                    if getattr(i, "engine", None) == ENG
                ]
                own = set()
                bad = set()
                for ins in q:
                    si = ins.sync_info
                    for u in (si.on_update or []) if si else []:
                        if u.update_mode == "sem-inc":
                            own.add(u.id)
                        else:
                            bad.add(u.id)
                own -= bad  # only sems this queue monotonically increments
                counts = {}
                prev = None
                todel = []
                for ins in q:
                    si = ins.sync_info
                    waits = list(si.on_wait or []) if si else []
                    if (
                        si is not None
                        and len(waits) == 1
                        and type(ins).__name__ != "InstEventSemaphore"
                        and waits[0].wait_mode == "sem-ge-imm"
                        and waits[0].id in own
                        and waits[0].wait_value <= counts.get(waits[0].id, 0)
                    ):
                        psi = prev.sync_info if prev is not None else None
                        if (
                            prev is not None
                            and type(prev).__name__ == "InstEventSemaphore"
                            and psi is not None
                            and len(psi.on_wait or []) == 1
                            and not (psi.on_update or [])
                        ):
                            ins.sync_info = mybir.SyncInfo(
                                on_wait=list(psi.on_wait),
                                on_update=list(si.on_update or []),
                            )
                            todel.append(prev)
                        else:
                            ins.sync_info = mybir.SyncInfo(
                                on_wait=[],
                                on_update=list(si.on_update or []),
                            )
                        changed += 1
                    si2 = ins.sync_info
                    for u in (si2.on_update or []) if si2 else []:
                        if u.update_mode == "sem-inc" and u.id in own:
                            counts[u.id] = counts.get(u.id, 0) + (
                                u.update_value or 1
                            )
                    prev = ins
                for d in todel:
                    bb.instructions.remove(d)
                    nc.inst_map.pop(d.name, None)
    return changed


def _build_nc():
    import concourse.mybir as mybir
    from concourse import bacc, tile

    flags = set(VARIANT.split("-"))

    f32 = mybir.dt.float32
    f16 = mybir.dt.float16
    ADD = mybir.AluOpType.add
    MAX = mybir.AluOpType.max
    RELU = mybir.ActivationFunctionType.Relu

    nc = bacc.Bacc("TRN2", target_bir_lowering=False, debug=False)

    zT4_d = nc.declare_dram_parameter("zT4", [128, BC], f16, isOutput=False)
    w1_d = nc.declare_dram_parameter("w1s", [128, 16 * 128], f16, isOutput=False)
    w2_d = nc.declare_dram_parameter("w2s", [128, 16 * 128], f16, isOutput=False)
    w3_d = nc.declare_dram_parameter("w3s", [128, 16 * 64], f16, isOutput=False)
    b1_d = nc.declare_dram_parameter("b1s", [128, L], f32, isOutput=False)
    b2_d = nc.declare_dram_parameter("b2s", [128, 16], f32, isOutput=False)
    b3_d = nc.declare_dram_parameter("b3s", [128, 1], f32, isOutput=False)
    out_d = nc.declare_dram_parameter("out", [64, BC], f32, isOutput=True)

    # greedy DVE/ACT balance for PSUM drains (calibrated ns per op at FD)
    if "s1" in flags:
        eng_time = [0.0, 285.0]
    elif "s2" in flags:
        eng_time = [329.0, 0.0]
    else:
        eng_time = [0.0, 0.0]

    if "cal" in flags:
        DVE_NS, ACT_NS = 800.0, 683.0
    elif "rA" in flags:
        DVE_NS, ACT_NS = 700.0, 570.0
    elif "rD" in flags:
        DVE_NS, ACT_NS = 658.0, 610.0
    elif "c2" in flags:
        # measured engine-busy per op (incl. queue effects) from the
        # 2026-08-07 trace: DVE 240.8us/353 ops, ACT 246.3us/408 ops
        DVE_NS, ACT_NS = 682.0, 604.0
    else:
        DVE_NS, ACT_NS = 658.0, 570.0

    if "pd" in flags:
        # mixed 512/1024-col drains: fd-aware costs (decode+access+stream)
        def dve_cost(fd):
            return 170.0 + fd / 0.96

        def act_cost(fd):
            return 175.0 + fd / 1.2
    else:
        def dve_cost(fd):
            return DVE_NS

        def act_cost(fd):
            return ACT_NS

    HF = FD // 2  # single-matmul moving dim (PSUM bank limit)

    with tile.TileContext(nc) as tc:
        with (
            tc.tile_pool(name="const", bufs=1) as const,
            tc.tile_pool(
                name="l1ps",
                bufs=(
                    2 if "pd" in flags
                    else 2 if "m1" in flags
                    else 4 if "l2b3" in flags
                    else 6 if "l16" in flags
                    else 4 if "l3s2" in flags
                    else 5 if "l3s" in flags
                    else 4
                ),
                space="PSUM",
            ) as l1ps,
            tc.tile_pool(name="l1pB", bufs=1, space="PSUM") as l1pB,
            tc.tile_pool(name="h1sp", bufs=2) as h1sp,
            tc.tile_pool(
                name="l2ps",
                bufs=(3 if "l2b3" in flags else 1 if "l16" in flags else 2),
                space="PSUM",
            ) as l2ps,
            tc.tile_pool(
                name="l3ps",
                bufs=(2 if "l3s2" in flags else 1 if "l3s" in flags else 2),
                space="PSUM",
            ) as l3ps,
            tc.tile_pool(
                name="h1p",
                bufs=(
                    6 if "m1" in flags
                    else 18 if "sbq" in flags
                    else 14 if "sbp" in flags
                    else 10
                ),
            ) as h1p,
            tc.tile_pool(
                name="h2p",
                bufs=(26 if "sbq" in flags else 20 if "sbp" in flags else 18),
            ) as h2p,
            tc.tile_pool(
                name="outp",
                bufs=(6 if "sbq" in flags else 5 if "sbp" in flags else 3),
            ) as outp,
        ):
            if "pw" in flags:
                # ACT spline-table prewarm: the one-time ACT_TABLE_LOAD
                # (~2.7us) overlaps the input DMA instead of delaying the
                # first real drain.
                warm = const.tile([128, 1], f32, name="warm")
                nc.vector.memset(warm[:], 0.0)
                warm2 = const.tile([128, 1], f32, name="warm2")
                nc.scalar.activation(warm2[:], warm[:], RELU, bias=0.0)

            zslices = []
            if "zs" in flags:
                # per-tile z slices: first L1 matmul waits only on slice 0
                zt0 = const.tile([128, HF], f16, name="zt_s0")
                nc.sync.dma_start(zt0[:], zT4_d[:, 0:HF])
                zslices.append(zt0)
            else:
                zt_all = const.tile([128, BC], f16, name="zt_all")
                nc.sync.dma_start(zt_all[:], zT4_d[:, :])
            # Boot DMAs spread across the three DGE paths (sync HWDGE ring,
            # scalar HWDGE ring, gpsimd SWDGE) in first-use order so their
            # ~2us fixed completion latencies overlap: w1 gates the first L1
            # matmul, b1 the first L1 drain, w2+b2 the first L2. w1s only
            # populates group slots 0-7 (cols 0:1024); the zero half is never
            # transferred, and groups 0-1 ride a small first chunk so the
            # first matmul is gated by zt0, not the full weight load.
            w1ta = const.tile([128, 2 * 128], f16, name="w1ta")
            nc.scalar.dma_start(w1ta[:], w1_d[:, 0 : 2 * 128])

            def w1sl(g, r0, r1):
                if g < 2:
                    return w1ta[r0:r1, 128 * g : 128 * (g + 1)]
                return w1tb[r0:r1, 128 * (g - 2) : 128 * (g - 1)]
            b1t = const.tile([128, L], f32, name="b1t")
            nc.gpsimd.dma_start(b1t[:], b1_d[:, :])
            b2t = const.tile([128, 16], f32, name="b2t")
            nc.gpsimd.dma_start(b2t[:], b2_d[:, :])
            # w2 split like w1: pairs 0-3 (128KB) land before the w1 bulk
            # so the first L2 matmuls are not gated at ~15us by the full
            # 512KB transfer (both drain engines measured a ~1.4us stall
            # at t~14.5us waiting on it).
            w2ta = const.tile([128, 4 * 128], f16, name="w2ta")
            nc.scalar.dma_start(w2ta[:], w2_d[:, 0 : 4 * 128])
            # dmv: the bulk w1/w2 triggers ride the idle sync queue instead
            # of costing ~650ns each of drain-critical ACT time at ~t=9-15us
            bulk_eng = nc.sync if "dmv" in flags else nc.scalar
            w1tb = const.tile([128, 6 * 128], f16, name="w1tb")
            bulk_eng.dma_start(w1tb[:], w1_d[:, 2 * 128 : 8 * 128])
            w2tb = const.tile([128, 12 * 128], f16, name="w2tb")
            bulk_eng.dma_start(w2tb[:], w2_d[:, 4 * 128 : 16 * 128])

            def w2sl(q, c0, c1):
                if q < 4:
                    return w2ta[:, 128 * q + c0 : 128 * q + c1]
                return w2tb[:, 128 * (q - 4) + c0 : 128 * (q - 4) + c1]
            w3t = const.tile([128, 16 * 64], f16, name="w3t")
            nc.sync.dma_start(w3t[:], w3_d[:, :])
            b3t = const.tile([128, 1], f32, name="b3t")
            nc.gpsimd.dma_start(b3t[:], b3_d[:, :])
            if "zs" in flags:
                for ti in range(1, NT):
                    zti = const.tile([128, HF], f16, name=f"zt_s{ti}")
                    nc.sync.dma_start(zti[:], zT4_d[:, HF * ti : HF * (ti + 1)])
                    zslices.append(zti)

            alt_ctr = [0]

            def drain(dst, src, bias_ap, relu=True, force=None):
                fd = src.shape[-1]
                if force is not None:
                    use_dve = force == 0
                elif "alt" in flags:
                    alt_ctr[0] += 1
                    use_dve = alt_ctr[0] % 2 == 0
                else:
                    use_dve = (
                        eng_time[0] + dve_cost(fd) <= eng_time[1] + act_cost(fd)
                    )
                if use_dve:
                    eng_time[0] += dve_cost(fd)
                    if relu and bias_ap is None:
                        nc.vector.tensor_scalar(dst, src, 0.0, None, MAX)
                    elif relu:
                        nc.vector.tensor_scalar(dst, src, bias_ap, 0.0, ADD, MAX)
                    else:
                        nc.vector.tensor_scalar(dst, src, bias_ap, None, ADD)
                else:
                    eng_time[1] += act_cost(fd)
                    fn = RELU if relu else mybir.ActivationFunctionType.Identity
                    nc.scalar.activation(
                        dst, src, fn, bias=0.0 if bias_ap is None else bias_ap
                    )
                return 0 if use_dve else 1

            ps3s = None
            # skw: software-pipelined emission — L2 work of group g-1 and L3
            # work of group g-2 are emitted after group g's L1 section, so
            # neither the PE nor the drain engines head-of-line block on
            # drains emitted in the same group. Queues carry (t, q, ...)
            # across group and tile boundaries; osb/output DMAs emit when
            # the q==15 L3 of an odd tile is processed.
            skw = "skw" in flags
            pend_l2 = []
            pend_l3 = []
            ps3h = [None]

            def skw_emit_l2(ents):
                out = []
                for ent in ents:
                    tt, q, ha, hb = ent
                    ps2 = l2ps.tile(
                        [128, HF], f32, tag="l2", name=f"ps2_{tt}_{q}"
                    )
                    nc.tensor.matmul(
                        ps2[0:64, :],
                        lhsT=w2sl(q, 0, 64),
                        rhs=ha[:],
                        start=True,
                        stop=True,
                        tile_position=(0, 0),
                    )
                    nc.tensor.matmul(
                        ps2[64:128, :],
                        lhsT=w2sl(q, 64, 128),
                        rhs=hb[:],
                        start=True,
                        stop=True,
                        tile_position=(0, 64),
                    )
                    h2 = h2p.tile([128, HF], f16, tag="h2", name=f"h2_{tt}_{q}")
                    drain(h2[:], ps2[:], b2t[:, q : q + 1])
                    out.append((tt, q, h2))
                return out

            def skw_emit_l3(ents):
                for tt, q, h2 in ents:
                    po = 64 * (tt % 2)
                    if q == 0 and tt % 2 == 0:
                        ps3h[0] = l3ps.tile(
                            [128, HF], f32, tag="l3", name=f"ps3s_{tt}"
                        )
                    nc.tensor.matmul(
                        ps3h[0][po : po + 64, :],
                        lhsT=w3t[:, 64 * q : 64 * (q + 1)],
                        rhs=h2[:],
                        start=(q == 0),
                        stop=(q == 15),
                        tile_position=(0, po),
                    )
                    if q == 15 and tt % 2 == 1:
                        osb = outp.tile(
                            [128, HF], f32, tag="o", name=f"osb_{tt}"
                        )
                        drain(osb[:], ps3h[0][:], b3t[:, 0:1], relu=False)
                        nc.sync.dma_start(
                            out_d[:, HF * (tt - 1) : HF * tt], osb[0:64, :]
                        )
                        eng = nc.scalar if tt == NT - 1 else nc.sync
                        eng.dma_start(
                            out_d[:, HF * tt : HF * (tt + 1)], osb[64:128, :]
                        )

            for t in range(NT):
                zt = zslices[t][:] if "zs" in flags else zt_all[:, HF * t : HF * (t + 1)]
                h2_tiles = []
                if "l3s" in flags and t % 2 == 0 and not skw:
                    ps3s = l3ps.tile(
                        [128, HF], f32, tag="l3", name=f"ps3s_{t}"
                    )
                if "fg" in flags:
                    # fine-grain: L1 half-group (2 layers) -> its L2 pair ->
                    # inline L3, shortening the h1->L2 dependency distance
                    for q in range(16):
                        fl = [x for x in (2 * q, 2 * q + 1) if x < L]
                        h1f = []
                        for lyr in fl:
                            gg, ii = divmod(lyr, 4)
                            ro = 32 * ii
                            ps = l1ps.tile(
                                [128, HF], f32, tag="l1", name=f"ps1_{t}_{lyr}"
                            )
                            nc.tensor.matmul(
                                ps[:],
                                lhsT=w1sl(gg, ro, ro + L),
                                rhs=zt[ro : ro + L, :],
                                start=True,
                                stop=True,
                                tile_position=(ro, 0),
                            )
                            h1 = h1p.tile(
                                [128, HF], f16, tag="h1", name=f"h1_{t}_{lyr}"
                            )
                            drain(h1[:], ps[:], b1t[:, lyr : lyr + 1])
                            h1f.append(h1)
                        if len(fl) == 1:
                            h1f.append(h1f[0])
                        ha, hb = h1f
                        ps2 = l2ps.tile(
                            [128, HF], f32, tag="l2", name=f"ps2_{t}_{q}"
                        )
                        nc.tensor.matmul(
                            ps2[0:64, :],
                            lhsT=w2sl(q, 0, 64),
                            rhs=ha[:],
                            start=True,
                            stop=True,
                            tile_position=(0, 0),
                        )
                        nc.tensor.matmul(
                            ps2[64:128, :],
                            lhsT=w2sl(q, 64, 128),
                            rhs=hb[:],
                            start=True,
                            stop=True,
                            tile_position=(0, 64),
                        )
                        h2 = h2p.tile(
                            [128, HF], f16, tag="h2", name=f"h2_{t}_{q}"
                        )
                        drain(h2[:], ps2[:], b2t[:, q : q + 1])
                        h2_tiles.append(h2)
                        po = 64 * (t % 2)
                        nc.tensor.matmul(
                            ps3s[po : po + 64, :],
                            lhsT=w3t[:, 64 * q : 64 * (q + 1)],
                            rhs=h2[:],
                            start=(q == 0),
                            stop=(q == 15),
                            tile_position=(0, po),
                        )
                for g in ([] if "fg" in flags else range(8)):
                    lyrs = [x for x in range(4 * g, 4 * g + 4) if x < L]
                    h1_tiles = []
                    if "m1" in flags:
                        # 4 concurrent row-tiled MMs (K=32 incl. bias row)
                        # into two [128,1024] psum tiles; one bias-free relu
                        # drain per tile covers 2 layers.
                        h1d = []
                        for half in range(2):
                            hl = lyrs[2 * half : 2 * half + 2]
                            if not hl:
                                continue
                            ps = l1ps.tile(
                                [128, 2 * HF], f32, tag="l1",
                                name=f"ps1_{t}_{g}_{half}",
                            )
                            for k, lyr in enumerate(hl):
                                ro = 64 * half + 32 * k
                                nc.tensor.matmul(
                                    ps[:, HF * k : HF * (k + 1)],
                                    lhsT=w1sl(g, ro, ro + 32),
                                    rhs=zt[ro : ro + 32, :],
                                    start=True,
                                    stop=True,
                                    tile_position=(ro, 0),
                                )
                            h1 = h1p.tile(
                                [128, 2 * HF], f16, tag="h1",
                                name=f"h1_{t}_{g}_{half}",
                            )
                            drain(h1[:], ps[:], None)
                            h1d.append(h1)
                        h1_tiles = [h1d[0][:, 0:HF], h1d[0][:, HF : 2 * HF]]
                        if len(h1d) > 1:
                            h1_tiles += [h1d[1][:, 0:HF], h1d[1][:, HF : 2 * HF]]
                        else:
                            h1_tiles += [h1_tiles[0], h1_tiles[0]]
                        if len(lyrs) == 3:
                            h1_tiles[3] = h1d[1][:, 0:HF]
                    elif "pd" in flags:
                        # paired L1: layers 2j/2j+1 of the group share one
                        # [128,1024] 2-bank psum (b1 folded via the zT4 ones
                        # row, K=32) -> ONE relu-only drain per pair. Boot
                        # tiles drain per-half to keep the fine-grained
                        # cadence that earns the HAM K=8/8 grant.
                        for half in range(2):
                            hl = lyrs[2 * half : 2 * half + 2]
                            if len(hl) == 2:
                                ps = l1ps.tile(
                                    [128, 2 * HF], f32, tag="l1",
                                    name=f"ps1_{t}_{g}_{half}",
                                )
                                for k, lyr in enumerate(hl):
                                    ro = 32 * (2 * half + k)
                                    nc.tensor.matmul(
                                        ps[:, HF * k : HF * (k + 1)],
                                        lhsT=w1sl(g, ro, ro + 32),
                                        rhs=zt[ro : ro + 32, :],
                                        start=True,
                                        stop=True,
                                        tile_position=(ro, 0),
                                    )
                                h1 = h1p.tile(
                                    [128, 2 * HF], f16, tag="h1",
                                    name=f"h1_{t}_{g}_{half}",
                                )
                                if t < 2:
                                    drain(h1[:, 0:HF], ps[:, 0:HF], None)
                                    drain(
                                        h1[:, HF : 2 * HF],
                                        ps[:, HF : 2 * HF],
                                        None,
                                    )
                                else:
                                    drain(h1[:], ps[:], None)
                                h1_tiles.append(h1[:, 0:HF])
                                h1_tiles.append(h1[:, HF : 2 * HF])
                            elif len(hl) == 1:
                                ro = 32 * (2 * half)
                                ps = l1pB.tile(
                                    [128, HF], f32, tag="l1b",
                                    name=f"ps1b_{t}",
                                )
                                nc.tensor.matmul(
                                    ps[:],
                                    lhsT=w1sl(g, ro, ro + 32),
                                    rhs=zt[ro : ro + 32, :],
                                    start=True,
                                    stop=True,
                                    tile_position=(ro, 0),
                                )
                                h1 = h1sp.tile(
                                    [128, HF], f16, tag="h1b", name=f"h1b_{t}"
                                )
                                drain(h1[:], ps[:], None)
                                h1_tiles.append(h1[:])
                                h1_tiles.append(h1[:])
                    else:
                        prev_e = None
                        for i, lyr in enumerate(lyrs):
                            ro = 32 * i
                            ps = l1ps.tile(
                                [128, HF], f32, tag="l1", name=f"ps1_{t}_{lyr}"
                            )
                            nc.tensor.matmul(
                                ps[:],
                                lhsT=w1sl(g, ro, ro + L),
                                rhs=zt[ro : ro + L, :],
                                start=True,
                                stop=True,
                                tile_position=(ro, 0),
                            )
                            h1 = h1p.tile(
                                [128, HF], f16, tag="h1", name=f"h1_{t}_{lyr}"
                            )
                            # pe2: the two h1 drains feeding one L2 pair go to
                            # different engines so they finish ~together and
                            # the col-tiled L2 matmuls co-issue (co-insert).
                            fe = (
                                1 - prev_e
                                if ("pe2" in flags and i % 2 == 1)
                                else None
                            )
                            prev_e = drain(
                                h1[:], ps[:], b1t[:, lyr : lyr + 1], force=fe
                            )
                            h1_tiles.append(h1)
                        if len(lyrs) == 3:
                            h1_tiles.append(h1_tiles[2])
                    if skw:
                        new_l3 = skw_emit_l2(pend_l2)  # L2 of group g-1
                        del pend_l2[:]
                        skw_emit_l3(pend_l3)           # L3 of group g-2
                        del pend_l3[:]
                        pend_l3.extend(new_l3)
                        for j in range(2):
                            q = 2 * g + j
                            pend_l2.append(
                                (t, q, h1_tiles[2 * j], h1_tiles[2 * j + 1])
                            )
                        continue
                    for j in range(2):
                        q = 2 * g + j
                        ha = h1_tiles[2 * j]
                        hb = h1_tiles[2 * j + 1]
                        ps2 = l2ps.tile(
                            [128, HF], f32, tag="l2", name=f"ps2_{t}_{q}"
                        )
                        nc.tensor.matmul(
                            ps2[0:64, :],
                            lhsT=w2sl(q, 0, 64),
                            rhs=ha[:],
                            start=True,
                            stop=True,
                            tile_position=(0, 0),
                        )
                        if not ("p15" in flags and q == 15):
                            nc.tensor.matmul(
                                ps2[64:128, :],
                                lhsT=w2sl(q, 64, 128),
                                rhs=hb[:],
                                start=True,
                                stop=True,
                                tile_position=(0, 64),
                            )
                        h2 = h2p.tile(
                            [128, HF], f16, tag="h2", name=f"h2_{t}_{q}"
                        )
                        drain(h2[:], ps2[:], b2t[:, q : q + 1])
                        h2_tiles.append(h2)
                        if "l3i" in flags:
                            po = 64 * (t % 2)
                            if "l3cs" in flags:
                                # two M=32 col-tiled chains sharing one rhs:
                                # the insert column broadcasts across both
                                # col-groups, ~halving L3 PE time.
                                for cs in range(2):
                                    nc.tensor.matmul(
                                        ps3s[po + 32 * cs : po + 32 * (cs + 1), :],
                                        lhsT=w3t[
                                            :, 64 * q + 32 * cs : 64 * q + 32 * (cs + 1)
                                        ],
                                        rhs=h2[:],
                                        start=(q == 0),
                                        stop=(q == 15),
                                        tile_position=(0, po + 32 * cs),
                                    )
                            else:
                                nc.tensor.matmul(
                                    ps3s[po : po + 64, :],
                                    lhsT=w3t[:, 64 * q : 64 * (q + 1)],
                                    rhs=h2[:],
                                    start=(q == 0),
                                    stop=(q == 15),
                                    tile_position=(0, po),
                                )
                if "l3s" in flags and not skw:
                    po = 64 * (t % 2)
                    if "l3i" not in flags:
                        for q in range(16):
                            nc.tensor.matmul(
                                ps3s[po : po + 64, :],
                                lhsT=w3t[:, 64 * q : 64 * (q + 1)],
                                rhs=h2_tiles[q][:],
                                start=(q == 0),
                                stop=(q == 15),
                                tile_position=(0, po),
                            )
                    if "ose" in flags:
                        osb = outp.tile([64, HF], f32, tag="o", name=f"osb_{t}")
                        drain(
                            osb[:], ps3s[po : po + 64, :],
                            b3t[po : po + 64, 0:1], relu=False,
                        )
                        nc.sync.dma_start(
                            out_d[:, HF * t : HF * (t + 1)], osb[:]
                        )
                    elif t % 2 == 1:
                        osb = outp.tile([128, HF], f32, tag="o", name=f"osb_{t}")
                        drain(osb[:], ps3s[:], b3t[:, 0:1], relu=False)
                        nc.sync.dma_start(
                            out_d[:, HF * (t - 1) : HF * t], osb[0:64, :]
                        )
                        # final tile: second store on the scalar HWDGE ring so
                        # the two ~2us completion receipts overlap at the tail
                        # (earlier tiles stay off the ACT queue - a DMA trigger
                        # there costs ~650ns of drain-critical ACT time).
                        eng = nc.scalar if t == NT - 1 else nc.sync
                        eng.dma_start(
                            out_d[:, HF * t : HF * (t + 1)], osb[64:128, :]
                        )
                elif not skw:
                    ps3 = l3ps.tile([64, HF], f32, tag="l3", name=f"ps3_{t}")
                    for q in range(16):
                        nc.tensor.matmul(
                            ps3[:],
                            lhsT=w3t[:, 64 * q : 64 * (q + 1)],
                            rhs=h2_tiles[q][:],
                            start=(q == 0),
                            stop=(q == 15),
                        )
                    osb = outp.tile([64, HF], f32, tag="o", name=f"osb_{t}")
                    drain(osb[:], ps3[:], b3t[0:64, 0:1], relu=False)
                    nc.sync.dma_start(out_d[:, HF * t : HF * (t + 1)], osb[:])
            if skw:
                while pend_l2 or pend_l3:
                    new_l3 = skw_emit_l2(pend_l2)
                    del pend_l2[:]
                    skw_emit_l3(pend_l3)
                    del pend_l3[:]
                    pend_l3.extend(new_l3)

    if DEDUP_LDW:
        n = _dedup_ldweights(nc)
        print(f"dedup_ldweights removed {n}")
    if "sem1" in flags:
        # event semaphores only exist once compile() creates them; run the
        # merge pass right after the second generate_event_semaphores call
        # (sems final, still pre-ISA-codegen)
        orig_ges = nc.generate_event_semaphores
        ges_calls = [0]

        def _ges_wrapped():
            orig_ges()
            ges_calls[0] += 1
            if ges_calls[0] == 2:
                n = _merge_redundant_self_waits(nc)
                print(f"merge_redundant_self_waits: {n}")

        nc.generate_event_semaphores = _ges_wrapped
    nc.finalize()
    return nc


def _build_nc_m2():
    """Layer-paired L1 build: layers 2k/2k+1 share one [128,1024] 2-bank PSUM
    tile (column halves), b1 folded into the K=32 matmul via the ones row
    already present in zT4/w1s, so ONE relu-only drain covers both layers.
    Halves the dominant L1 drain-op count (31 -> 15 big + 1 single per batch
    tile) while keeping a fine-grained slot rotation (2x2-bank L1A + 1-bank
    L1B + 2 L2 + 1 shared L3 = 8 banks) so the PE never idles long enough to
    drop the HAM clock. Side effect: an h1 pair lands in one SBUF tile, so
    the two col-tiled L2 matmuls of pair q become ready together and co-issue.
    """
    import concourse.mybir as mybir
    from concourse import bacc, tile

    flags = set(VARIANT.split("-"))

    f32 = mybir.dt.float32
    f16 = mybir.dt.float16
    ADD = mybir.AluOpType.add
    MAX = mybir.AluOpType.max
    RELU = mybir.ActivationFunctionType.Relu

    nc = bacc.Bacc("TRN2", target_bir_lowering=False, debug=False)

    zT4_d = nc.declare_dram_parameter("zT4", [128, BC], f16, isOutput=False)
    w1_d = nc.declare_dram_parameter("w1s", [128, 16 * 128], f16, isOutput=False)
    w2_d = nc.declare_dram_parameter("w2s", [128, 16 * 128], f16, isOutput=False)
    w3_d = nc.declare_dram_parameter("w3s", [128, 16 * 64], f16, isOutput=False)
    b1_d = nc.declare_dram_parameter("b1s", [128, L], f32, isOutput=False)
    b2_d = nc.declare_dram_parameter("b2s", [128, 16], f32, isOutput=False)
    b3_d = nc.declare_dram_parameter("b3s", [128, 1], f32, isOutput=False)
    out_d = nc.declare_dram_parameter("out", [64, BC], f32, isOutput=True)

    # fd-aware greedy DVE/ACT drain balance (decode+access overhead + stream)
    eng_time = [0.0, 0.0]

    def dve_cost(fd):
        return 170.0 + fd / 0.96

    def act_cost(fd):
        return 175.0 + fd / 1.2

    HF = FD // 2

    with tile.TileContext(nc) as tc:
        with (
            tc.tile_pool(name="const", bufs=1) as const,
            tc.tile_pool(name="l1pA", bufs=2, space="PSUM") as l1pA,
            tc.tile_pool(name="l1pB", bufs=1, space="PSUM") as l1pB,
            tc.tile_pool(name="l2ps", bufs=2, space="PSUM") as l2ps,
            tc.tile_pool(name="l3ps", bufs=1, space="PSUM") as l3ps,
            tc.tile_pool(name="h1p", bufs=8) as h1p,
            tc.tile_pool(name="h1sp", bufs=2) as h1sp,
            tc.tile_pool(name="h2p", bufs=10) as h2p,
            tc.tile_pool(name="outp", bufs=4) as outp,
        ):
            # ACT spline-table prewarm overlapping input DMA
            warm = const.tile([128, 1], f32, name="warm")
            nc.vector.memset(warm[:], 0.0)
            warm2 = const.tile([128, 1], f32, name="warm2")
            nc.scalar.activation(warm2[:], warm[:], RELU, bias=0.0)

            zslices = []
            zt0 = const.tile([128, HF], f16, name="zt_s0")
            nc.sync.dma_start(zt0[:], zT4_d[:, 0:HF])
            zslices.append(zt0)
            # boot DMAs spread over the three DGE paths in first-use order
            w1ta = const.tile([128, 2 * 128], f16, name="w1ta")
            nc.scalar.dma_start(w1ta[:], w1_d[:, 0 : 2 * 128])

            def w1sl(g, r0, r1):
                if g < 2:
                    return w1ta[r0:r1, 128 * g : 128 * (g + 1)]
                return w1tb[r0:r1, 128 * (g - 2) : 128 * (g - 1)]

            b2t = const.tile([128, 16], f32, name="b2t")
            nc.gpsimd.dma_start(b2t[:], b2_d[:, :])
            w2ta = const.tile([128, 4 * 128], f16, name="w2ta")
            nc.scalar.dma_start(w2ta[:], w2_d[:, 0 : 4 * 128])
            w1tb = const.tile([128, 6 * 128], f16, name="w1tb")
            nc.scalar.dma_start(w1tb[:], w1_d[:, 2 * 128 : 8 * 128])
            w2tb = const.tile([128, 12 * 128], f16, name="w2tb")
            nc.scalar.dma_start(w2tb[:], w2_d[:, 4 * 128 : 16 * 128])

            def w2sl(q, c0, c1):
                if q < 4:
                    return w2ta[:, 128 * q + c0 : 128 * q + c1]
                return w2tb[:, 128 * (q - 4) + c0 : 128 * (q - 4) + c1]

            w3t = const.tile([128, 16 * 64], f16, name="w3t")
            nc.sync.dma_start(w3t[:], w3_d[:, :])
            b3t = const.tile([128, 1], f32, name="b3t")
            nc.gpsimd.dma_start(b3t[:], b3_d[:, :])
            for ti in range(1, NT):
                zti = const.tile([128, HF], f16, name=f"zt_s{ti}")
                nc.sync.dma_start(zti[:], zT4_d[:, HF * ti : HF * (ti + 1)])
                zslices.append(zti)

            def drain(dst, src, bias_ap, relu=True):
                fd = src.shape[-1]
                use_dve = eng_time[0] + dve_cost(fd) <= eng_time[1] + act_cost(fd)
                if use_dve:
                    eng_time[0] += dve_cost(fd)
                    if relu and bias_ap is None:
                        nc.vector.tensor_scalar(dst, src, 0.0, None, MAX)
                    elif relu:
                        nc.vector.tensor_scalar(dst, src, bias_ap, 0.0, ADD, MAX)
                    else:
                        nc.vector.tensor_scalar(dst, src, bias_ap, None, ADD)
                else:
                    eng_time[1] += act_cost(fd)
                    fn = RELU if relu else mybir.ActivationFunctionType.Identity
                    nc.scalar.activation(
                        dst, src, fn, bias=0.0 if bias_ap is None else bias_ap
                    )

            ps3s = None
            for t in range(NT):
                zt = zslices[t]
                po = 64 * (t % 2)
                if t % 2 == 0:
                    ps3s = l3ps.tile([128, HF], f32, tag="l3", name=f"ps3s_{t}")
                for q in range(16):
                    if q < 15:
                        a, b = 2 * q, 2 * q + 1
                        g = q // 2
                        roa = 32 * (a % 4)
                        rob = 32 * (b % 4)
                        ps = l1pA.tile(
                            [128, 2 * HF], f32, tag="l1", name=f"ps1_{t}_{q}"
                        )
                        # K=32 incl. ones row -> b1 folded; different row
                        # bands -> the two mms co-issue back-to-back
                        nc.tensor.matmul(
                            ps[:, 0:HF],
                            lhsT=w1sl(g, roa, roa + 32),
                            rhs=zt[roa : roa + 32, :],
                            start=True,
                            stop=True,
                            tile_position=(roa, 0),
                        )
                        nc.tensor.matmul(
                            ps[:, HF : 2 * HF],
                            lhsT=w1sl(g, rob, rob + 32),
                            rhs=zt[rob : rob + 32, :],
                            start=True,
                            stop=True,
                            tile_position=(rob, 0),
                        )
                        h1 = h1p.tile(
                            [128, 2 * HF], f16, tag="h1", name=f"h1_{t}_{q}"
                        )
                        drain(h1[:], ps[:], None)
                        ha = h1[:, 0:HF]
                        hb = h1[:, HF : 2 * HF]
                    else:
                        # layer 30 single via the 1-bank B slot; q=15 pair
                        # duplicates it (w3s zero-masks the dup's output)
                        ps = l1pB.tile([128, HF], f32, tag="l1b", name=f"ps1b_{t}")
                        nc.tensor.matmul(
                            ps[:],
                            lhsT=w1sl(7, 64, 96),
                            rhs=zt[64:96, :],
                            start=True,
                            stop=True,
                            tile_position=(64, 0),
                        )
                        h1 = h1sp.tile([128, HF], f16, tag="h1b", name=f"h1b_{t}")
                        drain(h1[:], ps[:], None)
                        ha = h1[:]
                        hb = h1[:]
                    ps2 = l2ps.tile([128, HF], f32, tag="l2", name=f"ps2_{t}_{q}")
                    nc.tensor.matmul(
                        ps2[0:64, :],
                        lhsT=w2sl(q, 0, 64),
                        rhs=ha,
                        start=True,
                        stop=True,
                        tile_position=(0, 0),
                    )
                    nc.tensor.matmul(
                        ps2[64:128, :],
                        lhsT=w2sl(q, 64, 128),
                        rhs=hb,
                        start=True,
                        stop=True,
                        tile_position=(0, 64),
                    )
                    h2 = h2p.tile([128, HF], f16, tag="h2", name=f"h2_{t}_{q}")
                    drain(h2[:], ps2[:], b2t[:, q : q + 1])
                    nc.tensor.matmul(
                        ps3s[po : po + 64, :],
                        lhsT=w3t[:, 64 * q : 64 * (q + 1)],
                        rhs=h2[:],
                        start=(q == 0),
                        stop=(q == 15),
                        tile_position=(0, po),
                    )
                if t % 2 == 1:
                    osb = outp.tile([128, HF], f32, tag="o", name=f"osb_{t}")
                    drain(osb[:], ps3s[:], b3t[:, 0:1], relu=False)
                    nc.sync.dma_start(
                        out_d[:, HF * (t - 1) : HF * t], osb[0:64, :]
                    )
                    eng = nc.scalar if t == NT - 1 else nc.sync
                    eng.dma_start(
                        out_d[:, HF * t : HF * (t + 1)], osb[64:128, :]
                    )

    nc.finalize()
    return nc


def _build_nc_m3():
    """m2 drains + 2-step software-pipelined emission.

    Flat stream over global steps s = 16*t + k (k = layer-pair index). At
    step s we emit: L1 fills for step s, the h1 drain for step s-1, and the
    L2 fills + h2 drain + inline L3 matmul for step s-2. Every PE
    instruction's dependencies were produced >=1 full step (~1.9us) earlier,
    so the in-order PE queue never head-of-line blocks on a drain, PE idle
    stays in thin slices (HAM keeps K=8/8), and each drain engine always has
    a ready op (one [128,1024] h1 drain + one [128,512] h2 drain per step).
    """
    import concourse.mybir as mybir
    from concourse import bacc, tile

    flags = set(VARIANT.split("-"))

    f32 = mybir.dt.float32
    f16 = mybir.dt.float16
    ADD = mybir.AluOpType.add
    MAX = mybir.AluOpType.max
    RELU = mybir.ActivationFunctionType.Relu

    nc = bacc.Bacc("TRN2", target_bir_lowering=False, debug=False)

    zT4_d = nc.declare_dram_parameter("zT4", [128, BC], f16, isOutput=False)
    w1_d = nc.declare_dram_parameter("w1s", [128, 16 * 128], f16, isOutput=False)
    w2_d = nc.declare_dram_parameter("w2s", [128, 16 * 128], f16, isOutput=False)
    w3_d = nc.declare_dram_parameter("w3s", [128, 16 * 64], f16, isOutput=False)
    b1_d = nc.declare_dram_parameter("b1s", [128, L], f32, isOutput=False)
    b2_d = nc.declare_dram_parameter("b2s", [128, 16], f32, isOutput=False)
    b3_d = nc.declare_dram_parameter("b3s", [128, 1], f32, isOutput=False)
    out_d = nc.declare_dram_parameter("out", [64, BC], f32, isOutput=True)

    eng_time = [0.0, 0.0]

    def dve_cost(fd):
        return 170.0 + fd / 0.96

    def act_cost(fd):
        return 175.0 + fd / 1.2

    HF = FD // 2

    with tile.TileContext(nc) as tc:
        with (
            tc.tile_pool(name="const", bufs=1) as const,
            tc.tile_pool(name="l1pA", bufs=2, space="PSUM") as l1pA,
            tc.tile_pool(name="l2ps", bufs=3, space="PSUM") as l2ps,
            tc.tile_pool(name="l3ps", bufs=1, space="PSUM") as l3ps,
            tc.tile_pool(name="h1p", bufs=8) as h1p,
            tc.tile_pool(name="h1sp", bufs=2) as h1sp,
            tc.tile_pool(name="h2p", bufs=10) as h2p,
            tc.tile_pool(name="outp", bufs=4) as outp,
        ):
            warm = const.tile([128, 1], f32, name="warm")
            nc.vector.memset(warm[:], 0.0)
            warm2 = const.tile([128, 1], f32, name="warm2")
            nc.scalar.activation(warm2[:], warm[:], RELU, bias=0.0)

            zslices = []
            zt0 = const.tile([128, HF], f16, name="zt_s0")
            nc.sync.dma_start(zt0[:], zT4_d[:, 0:HF])
            zslices.append(zt0)
            w1ta = const.tile([128, 2 * 128], f16, name="w1ta")
            nc.scalar.dma_start(w1ta[:], w1_d[:, 0 : 2 * 128])

            def w1sl(g, r0, r1):
                if g < 2:
                    return w1ta[r0:r1, 128 * g : 128 * (g + 1)]
                return w1tb[r0:r1, 128 * (g - 2) : 128 * (g - 1)]

            b2t = const.tile([128, 16], f32, name="b2t")
            nc.gpsimd.dma_start(b2t[:], b2_d[:, :])
            w2ta = const.tile([128, 4 * 128], f16, name="w2ta")
            nc.scalar.dma_start(w2ta[:], w2_d[:, 0 : 4 * 128])
            w1tb = const.tile([128, 6 * 128], f16, name="w1tb")
            nc.scalar.dma_start(w1tb[:], w1_d[:, 2 * 128 : 8 * 128])
            w2tb = const.tile([128, 12 * 128], f16, name="w2tb")
            nc.scalar.dma_start(w2tb[:], w2_d[:, 4 * 128 : 16 * 128])

            def w2sl(q, c0, c1):
                if q < 4:
                    return w2ta[:, 128 * q + c0 : 128 * q + c1]
                return w2tb[:, 128 * (q - 4) + c0 : 128 * (q - 4) + c1]

            w3t = const.tile([128, 16 * 64], f16, name="w3t")
            nc.sync.dma_start(w3t[:], w3_d[:, :])
            b3t = const.tile([128, 1], f32, name="b3t")
            nc.gpsimd.dma_start(b3t[:], b3_d[:, :])
            for ti in range(1, NT):
                zti = const.tile([128, HF], f16, name=f"zt_s{ti}")
                nc.sync.dma_start(zti[:], zT4_d[:, HF * ti : HF * (ti + 1)])
                zslices.append(zti)

            def drain(dst, src, bias_ap, relu=True, force=None):
                fd = src.shape[-1]
                if force is not None:
                    use_dve = force == 0
                else:
                    use_dve = (
                        eng_time[0] + dve_cost(fd) <= eng_time[1] + act_cost(fd)
                    )
                if use_dve:
                    eng_time[0] += dve_cost(fd)
                    if relu and bias_ap is None:
                        nc.vector.tensor_scalar(dst, src, 0.0, None, MAX)
                    elif relu:
                        nc.vector.tensor_scalar(dst, src, bias_ap, 0.0, ADD, MAX)
                    else:
                        nc.vector.tensor_scalar(dst, src, bias_ap, None, ADD)
                else:
                    eng_time[1] += act_cost(fd)
                    fn = RELU if relu else mybir.ActivationFunctionType.Identity
                    nc.scalar.activation(
                        dst, src, fn, bias=0.0 if bias_ap is None else bias_ap
                    )

            S = NT * 16
            BOOT_STEPS = 32  # first 2 tiles: PE-saturating redundant fills
            # + baseline-like single-bank drains to win the HAM K=8/8 grant
            pend = {}  # step -> (ps_tile, h1_tile or None yet, t, k)
            ps3s = None

            def emit_l1_fills(s):
                t, k = divmod(s, 16)
                zt = zslices[t]
                # during boot, emit each fill twice (redundant recompute,
                # same-band so strictly serial): keeps the PE 100% saturated
                # at K=4/8 so the HAM monitor grants K=8/8 early and
                # deterministically, like the baseline's PE-bound boot
                rep = 2 if s < 16 else 1
                if k < 15:
                    a = 2 * k
                    g = k // 2
                    roa = 32 * (a % 4)
                    rob = roa + 32
                    ps = l1pA.tile([128, 2 * HF], f32, tag="l1", name=f"ps1_{t}_{k}")
                    for _ in range(rep):
                        nc.tensor.matmul(
                            ps[:, 0:HF],
                            lhsT=w1sl(g, roa, roa + 32),
                            rhs=zt[roa : roa + 32, :],
                            start=True,
                            stop=True,
                            tile_position=(roa, 0),
                        )
                    for _ in range(rep):
                        nc.tensor.matmul(
                            ps[:, HF : 2 * HF],
                            lhsT=w1sl(g, rob, rob + 32),
                            rhs=zt[rob : rob + 32, :],
                            start=True,
                            stop=True,
                            tile_position=(rob, 0),
                        )
                else:
                    ps = l2ps.tile([128, HF], f32, tag="l2", name=f"ps1b_{t}")
                    nc.tensor.matmul(
                        ps[:],
                        lhsT=w1sl(7, 64, 96),
                        rhs=zt[64:96, :],
                        start=True,
                        stop=True,
                        tile_position=(64, 0),
                    )
                pend[s] = [ps, None, t, k]

            def emit_h1_drain(s):
                ent = pend[s]
                ps, _, t, k = ent
                f1 = s % 2
                if k < 15:
                    h1 = h1p.tile([128, 2 * HF], f16, tag="h1", name=f"h1_{t}_{k}")
                    if s < BOOT_STEPS:
                        drain(h1[:, 0:HF], ps[:, 0:HF], None, force=s % 2)
                        drain(
                            h1[:, HF : 2 * HF],
                            ps[:, HF : 2 * HF],
                            None,
                            force=(s + 1) % 2,
                        )
                    else:
                        drain(h1[:], ps[:], None, force=f1)
                else:
                    h1 = h1sp.tile([128, HF], f16, tag="h1b", name=f"h1b_{t}")
                    drain(h1[:], ps[:], None, force=f1)
                ent[1] = h1

            h2pend = {}  # step -> (h2_tile, t, q)

            def emit_l2(s):
                ps, h1, t, k = pend.pop(s)
                q = k
                if k < 15:
                    ha = h1[:, 0:HF]
                    hb = h1[:, HF : 2 * HF]
                else:
                    ha = h1[:]
                    hb = h1[:]
                ps2 = l2ps.tile([128, HF], f32, tag="l2", name=f"ps2_{t}_{q}")
                rep = 2 if s < 16 else 1
                for _ in range(rep):
                    nc.tensor.matmul(
                        ps2[0:64, :],
                        lhsT=w2sl(q, 0, 64),
                        rhs=ha,
                        start=True,
                        stop=True,
                        tile_position=(0, 0),
                    )
                for _ in range(rep):
                    nc.tensor.matmul(
                        ps2[64:128, :],
                        lhsT=w2sl(q, 64, 128),
                        rhs=hb,
                        start=True,
                        stop=True,
                        tile_position=(0, 64),
                    )
                h2 = h2p.tile([128, HF], f16, tag="h2", name=f"h2_{t}_{q}")
                drain(h2[:], ps2[:], b2t[:, q : q + 1], force=(s + 1) % 2)
                h2pend[s] = (h2, t, q)

            def emit_l3(s):
                nonlocal ps3s
                h2, t, q = h2pend.pop(s)
                po = 64 * (t % 2)
                if q == 0 and t % 2 == 0:
                    ps3s = l3ps.tile([128, HF], f32, tag="l3", name=f"ps3s_{t}")
                nc.tensor.matmul(
                    ps3s[po : po + 64, :],
                    lhsT=w3t[:, 64 * q : 64 * (q + 1)],
                    rhs=h2[:],
                    start=(q == 0),
                    stop=(q == 15),
                    tile_position=(0, po),
                )
                if q == 15 and t % 2 == 1:
                    osb = outp.tile([128, HF], f32, tag="o", name=f"osb_{t}")
                    drain(osb[:], ps3s[:], b3t[:, 0:1], relu=False)
                    nc.sync.dma_start(
                        out_d[:, HF * (t - 1) : HF * t], osb[0:64, :]
                    )
                    eng = nc.scalar if t == NT - 1 else nc.sync
                    eng.dma_start(
                        out_d[:, HF * t : HF * (t + 1)], osb[64:128, :]
                    )

            # Per-step emission order tuned for the in-order PE queue: L2
            # fills first (deps resolved 2 steps ago - never block), L1
            # fills second (slot waits overlap L2 streams), L3 last and 5
            # steps deep so its h2 is long-drained. Drains interleave on
            # the engine side in the same cadence.
            for s in range(S + 5):
                if 1 <= s <= S:
                    emit_h1_drain(s - 1)
                if 2 <= s <= S + 1:
                    emit_l2(s - 2)
                if s < S:
                    emit_l1_fills(s)
                if 5 <= s <= S + 4:
                    emit_l3(s - 5)

    nc.finalize()
    return nc


def _build_nc_d2():
    """Pair-granular build: L1 layers 2q/2q+1 share one 2-bank PSUM tile.

    Per batch tile of 512 and pair q: two row-tiled K=32 L1 matmuls (bias via
    ones row) -> one [128,1024] relu drain -> two col-tiled L2 matmuls ->
    [128,512] bias+relu drain -> inline L3 accumulation matmul. The paired
    slot-free events let consecutive PE matmuls co-issue (tile_position
    concurrency), and halving the drain op count trims DVE/ACT overhead.
    """
    import concourse.mybir as mybir
    from concourse import bacc, tile

    flags = set(VARIANT.split("-"))

    f32 = mybir.dt.float32
    f16 = mybir.dt.float16
    ADD = mybir.AluOpType.add
    MAX = mybir.AluOpType.max
    RELU = mybir.ActivationFunctionType.Relu

    nc = bacc.Bacc("TRN2", target_bir_lowering=False, debug=False)

    zT4_d = nc.declare_dram_parameter("zT4", [128, BC], f16, isOutput=False)
    w1_d = nc.declare_dram_parameter("w1s", [128, 16 * 128], f16, isOutput=False)
    w2_d = nc.declare_dram_parameter("w2s", [128, 16 * 128], f16, isOutput=False)
    w3_d = nc.declare_dram_parameter("w3s", [128, 16 * 64], f16, isOutput=False)
    b1_d = nc.declare_dram_parameter("b1s", [128, L], f32, isOutput=False)
    b2_d = nc.declare_dram_parameter("b2s", [128, 16], f32, isOutput=False)
    b3_d = nc.declare_dram_parameter("b3s", [128, 1], f32, isOutput=False)
    out_d = nc.declare_dram_parameter("out", [64, BC], f32, isOutput=True)

    HF = FD // 2

    # greedy DVE/ACT drain balance, fd-aware effective ns per op
    eng_time = [0.0, 0.0]
    PAD = 80.0 if "pad" in flags else 0.0

    def dve_cost(fd):
        return (120.0 + fd) / 0.96 + PAD

    def act_cost(fd):
        return (172.0 + fd) / 1.2 + PAD

    with tile.TileContext(nc) as tc:
        with (
            tc.tile_pool(name="const", bufs=1) as const,
            tc.tile_pool(
                name="l1ps", bufs=(3 if "l1b3" in flags else 2), space="PSUM"
            ) as l1ps,
            tc.tile_pool(
                name="l2ps", bufs=(2 if "l1b3" in flags else 3), space="PSUM"
            ) as l2ps,
            tc.tile_pool(name="l3ps", bufs=1, space="PSUM") as l3ps,
            tc.tile_pool(name="h1p", bufs=10) as h1p,
            tc.tile_pool(name="h2p", bufs=20) as h2p,
            tc.tile_pool(name="outp", bufs=4) as outp,
        ):
            # ACT spline-table prewarm: dummy relu so the one-time
            # ACT_TABLE_LOAD overlaps the input DMA instead of delaying the
            # first real drain.
            warm = const.tile([128, 1], f32, name="warm")
            nc.vector.memset(warm[:], 0.0)
            warm2 = const.tile([128, 1], f32, name="warm2")
            nc.scalar.activation(warm2[:], warm[:], RELU, bias=0.0)

            zslices = []
            zt0 = const.tile([128, HF], f16, name="zt_s0")
            nc.sync.dma_start(zt0[:], zT4_d[:, 0:HF])
            zslices.append(zt0)
            w1t = const.tile([128, 16 * 128], f16, name="w1t")
            nc.sync.dma_start(w1t[:], w1_d[:, :])
            w2t = const.tile([128, 16 * 128], f16, name="w2t")
            nc.sync.dma_start(w2t[:], w2_d[:, :])
            w3t = const.tile([128, 16 * 64], f16, name="w3t")
            nc.sync.dma_start(w3t[:], w3_d[:, :])
            b2t = const.tile([128, 16], f32, name="b2t")
            nc.sync.dma_start(b2t[:], b2_d[:, :])
            b3t = const.tile([128, 1], f32, name="b3t")
            nc.sync.dma_start(b3t[:], b3_d[:, :])
            for ti in range(1, NT):
                zti = const.tile([128, HF], f16, name=f"zt_s{ti}")
                nc.sync.dma_start(zti[:], zT4_d[:, HF * ti : HF * (ti + 1)])
                zslices.append(zti)

            def drain(dst, src, bias_ap, relu=True):
                fd = src.shape[-1]
                use_dve = eng_time[0] + dve_cost(fd) <= eng_time[1] + act_cost(fd)
                if use_dve:
                    eng_time[0] += dve_cost(fd)
                    if relu and bias_ap is None:
                        nc.vector.tensor_scalar(dst, src, 0.0, None, MAX)
                    elif relu:
                        nc.vector.tensor_scalar(dst, src, bias_ap, 0.0, ADD, MAX)
                    else:
                        nc.vector.tensor_scalar(dst, src, bias_ap, None, ADD)
                else:
                    eng_time[1] += act_cost(fd)
                    fn = RELU if relu else mybir.ActivationFunctionType.Identity
                    nc.scalar.activation(
                        dst, src, fn, bias=0.0 if bias_ap is None else bias_ap
                    )

            ps3s = None
            for t in range(NT):
                zt = zslices[t]
                po = 64 * (t % 2)
                if t % 2 == 0:
                    ps3s = l3ps.tile([128, HF], f32, tag="l3", name=f"ps3s_{t}")
                for q in range(16):
                    ps = l1ps.tile([128, 2 * HF], f32, tag="l1", name=f"ps1_{t}_{q}")
                    for k in range(2):
                        lyr = 2 * q + k  # lyr 31 hits the zero band of w1s
                        gg, ii = divmod(lyr, 4)
                        ro = 32 * ii
                        nc.tensor.matmul(
                            ps[:, HF * k : HF * (k + 1)],
                            lhsT=w1t[ro : ro + 32, 128 * gg : 128 * (gg + 1)],
                            rhs=zt[ro : ro + 32, :],
                            start=True,
                            stop=True,
                            tile_position=(ro, 0),
                        )
                    h1 = h1p.tile([128, 2 * HF], f16, tag="h1", name=f"h1_{t}_{q}")
                    drain(h1[:], ps[:], None)
                    ps2 = l2ps.tile([128, HF], f32, tag="l2", name=f"ps2_{t}_{q}")
                    nc.tensor.matmul(
                        ps2[0:64, :],
                        lhsT=w2sl(q, 0, 64),
                        rhs=h1[:, 0:HF],
                        start=True,
                        stop=True,
                        tile_position=(0, 0),
                    )
                    nc.tensor.matmul(
                        ps2[64:128, :],
                        lhsT=w2sl(q, 64, 128),
                        rhs=h1[:, HF : 2 * HF],
                        start=True,
                        stop=True,
                        tile_position=(0, 64),
                    )
                    h2 = h2p.tile([128, HF], f16, tag="h2", name=f"h2_{t}_{q}")
                    drain(h2[:], ps2[:], b2t[:, q : q + 1])
                    nc.tensor.matmul(
                        ps3s[po : po + 64, :],
                        lhsT=w3t[:, 64 * q : 64 * (q + 1)],
                        rhs=h2[:],
                        start=(q == 0),
                        stop=(q == 15),
                        tile_position=(0, po),
                    )
                if t % 2 == 1:
                    osb = outp.tile([128, HF], f32, tag="o", name=f"osb_{t}")
                    drain(osb[:], ps3s[:], b3t[:, 0:1], relu=False)
                    nc.sync.dma_start(out_d[:, HF * (t - 1) : HF * t], osb[0:64, :])
                    nc.sync.dma_start(
                        out_d[:, HF * t : HF * (t + 1)], osb[64:128, :]
                    )

    nc.finalize()
    return nc


def _get_nc():
    if VARIANT not in _NC_CACHE:
        if VARIANT.startswith("m3"):
            _NC_CACHE[VARIANT] = _build_nc_m3()
        elif VARIANT.startswith("m2"):
            _NC_CACHE[VARIANT] = _build_nc_m2()
        elif VARIANT.startswith("d2"):
            _NC_CACHE[VARIANT] = _build_nc_d2()
        else:
            _NC_CACHE[VARIANT] = _build_nc()
    return _NC_CACHE[VARIANT]


def _prep_shared(W1, b1, W2, b2, Wout, bout, bout0):
    W1 = np.asarray(W1, np.float32)
    b1 = np.asarray(b1, np.float32)
    W2 = np.asarray(W2, np.float32)
    b2 = np.asarray(b2, np.float32)
    Wout = np.asarray(Wout, np.float32)
    bout = np.asarray(bout, np.float32)
    bout0 = np.asarray(bout0, np.float32)

    mask = np.tril(np.ones((L, L), np.float32))
    W1m = W1 * mask[:, None, :]  # [31, 128, 31]

    w1s = np.zeros((128, 16 * 128), F16)
    for g in range(8):
        for i in range(4):
            lyr = 4 * g + i
            if lyr >= L:
                break
            w1s[32 * i : 32 * i + L, 128 * g : 128 * (g + 1)] = W1m[lyr].T.astype(
                F16
            )
            w1s[32 * i + L, 128 * g : 128 * (g + 1)] = b1[lyr].astype(F16)
    b1s = np.ascontiguousarray(b1.T)  # [128, 31]

    w2s = np.zeros((128, 16 * 128), F16)
    b2s = np.zeros((128, 16), np.float32)
    for q, (a, bb) in enumerate(PAIRS):
        w2s[:, 128 * q : 128 * q + 64] = W2[a].T.astype(F16)
        w2s[:, 128 * q + 64 : 128 * (q + 1)] = W2[bb].T.astype(F16)
        b2s[0:64, q] = b2[a]
        b2s[64:128, q] = b2[bb]

    w3s = np.zeros((128, 16 * 64), F16)
    for q, (a, bb) in enumerate(PAIRS):
        blk = np.zeros((128, 64), np.float32)
        blk[0:64, 1 + a] = Wout[a][0]
        blk[0:64, 33 + a] = Wout[a][1]
        if 2 * q + 1 <= L - 1:  # real second layer (not the dup)
            blk[64:128, 1 + bb] = Wout[bb][0]
            blk[64:128, 33 + bb] = Wout[bb][1]
        w3s[:, 64 * q : 64 * (q + 1)] = blk.astype(F16)

    b3h = np.zeros(64, np.float32)
    b3h[0] = bout0[0]
    b3h[1 : 1 + L] = bout[:, 0]
    b3h[32] = bout0[1]
    b3h[33 : 33 + L] = bout[:, 1]
    b3s = np.concatenate([b3h, b3h]).reshape(128, 1)  # both L3 psum halves

    return w1s, w2s, w3s, b1s, b2s, b3s


def kernel(z, W1, b1, W2, b2, Wout, bout, bout0):
    global LAST_RESULT
    from concourse.bass_utils import run_bass_kernel_spmd

    z = np.asarray(z, np.float32)
    w1s, w2s, w3s, b1s, b2s, b3s = _prep_shared(W1, b1, W2, b2, Wout, bout, bout0)

    zin_T = np.ascontiguousarray(z[:, :L].T).astype(F16)  # [31, 65536]
    in_maps = []
    for c in range(NCORES):
        sl = zin_T[:, BC * c : BC * (c + 1)]
        zt4 = np.zeros((128, BC), F16)
        for i in range(4):
            zt4[32 * i : 32 * i + L] = sl
            zt4[32 * i + L] = 1.0  # ones row (feeds the K=32 bias fold in m1)
        in_maps.append(
            {
                "zT4": zt4,
                "w1s": w1s,
                "w2s": w2s,
                "w3s": w3s,
                "b1s": b1s,
                "b2s": b2s,
                "b3s": b3s,
            }
        )

    nc = _get_nc()
    try:
        res = run_bass_kernel_spmd(nc, in_maps, core_ids=list(range(NCORES)))
    except Exception:
        # transient device-unrecoverable states clear on the next attempt
        res = run_bass_kernel_spmd(nc, in_maps, core_ids=list(range(NCORES)))
    LAST_RESULT = res

    big = np.concatenate([res.results[c]["out"] for c in range(NCORES)], axis=1)
    mus = np.ascontiguousarray(big[:32].T).astype(np.float32, copy=False)
    lvs = np.ascontiguousarray(big[32:].T).astype(np.float32, copy=False)
    return mus, lvs

